# revision 25
# baseline (speedup 1.0000x reference)
"""Trainium2 Bass kernel for causal Lorentz self-attention.

Problem: B=4, L=4096, D=64 single-head self-attention where
  scores = (2 + 2*<q,k>_L) / scale + bias   (Lorentz inner product)
  causal mask (strict upper triangle) + per-query pad-mask
  attn = softmax(scores);  mu = attn @ v
  out = mu / sqrt(max(|<mu,mu>_L|, eps))

Key algebraic fact used: the softmax denominator cancels in the final
normalization (out = mu_raw / sqrt(|<mu_raw,mu_raw>_L|)), so no row-sum
is computed on device.

Sharding: 2 cores per batch. Each core runs an IDENTICAL static program of
4 "slots" (512 queries each) with static k-extents (8,16,24,32) steps of 128
keys. Which query tiles a slot owns, and where the causal boundary falls, is
encoded purely in host-prepared per-core input data:
  - k iterated DESCENDING from the diagonal, so the 4 boundary steps are
    always steps 0..3 of a slot (static affine_select masks),
  - slots whose causal extent is shorter than the static extent get
    "poison" K columns (huge negative score -> exp underflows to 0) and
    zero V rows.

Precision strategy "split" (default): all matmuls run in bf16 with hi/lo
decomposition (x = bf16(x) + bf16(x - bf16(x)), ~2^-17 operand precision):
  - scores: lhsT = [K_hi; K_lo] stacked on the contraction dim (K=128)
    against Q_hi replicated, plus a K=64 correction matmul K_hi x Q_lo.
  - attn@V: lhsT = [V_hi | V_lo] stacked on the output dim (M=128),
    two moving passes with P_hi and P_lo; the hi/lo output halves are
    summed once per slot (linearity lets them accumulate separately).
Strategy "f32" is the exact-fp32 fallback (4x slower matmuls).
"""

import os
import numpy as np
import ml_dtypes

import concourse.bass as bass
import concourse.bacc as bacc
import concourse.tile as tile
from concourse import mybir
from concourse import masks as cmasks
from concourse import bass_utils
from concourse._compat import with_exitstack
from contextlib import ExitStack

B, L, D = 4, 4096, 64
EPS = 1e-8
N_CORES = 8
QT = 128                       # queries per q-tile / keys per k-step
SLOT_Q = 512                   # queries per slot (4 q-tiles)
SLOTS = 4                      # slots per core
NQ_CORE = SLOTS * SLOT_Q       # 2048 queries per core
NT_CORE = NQ_CORE // QT        # 16 q-tiles per core
SLOT_EXTENTS = (8, 16, 24, 32)  # static k-steps per slot
TOTAL_STEPS = sum(SLOT_EXTENTS)  # 80
# groups of 4 consecutive q-tiles; group g covers q-tiles 4g..4g+3 and needs
# 4g+4 k-tiles. Half 0 gets groups (0,3,4,7) -> extents (4,16,20,32), half 1
# gets (1,2,5,6) -> (8,12,24,28); both fit elementwise under SLOT_EXTENTS.
HALF_GROUPS = ((0, 3, 4, 7), (1, 2, 5, 6))

_F32 = mybir.dt.float32
_BF16 = mybir.dt.bfloat16
_FP16 = mybir.dt.float16
_U16 = mybir.dt.uint16
_BF16_NP = ml_dtypes.bfloat16
_LOG2E = 1.4426950408889634
_SCHRAUDOLPH_A = 1024.0 * _LOG2E          # fp16-bits slope
_SCHRAUDOLPH_B0 = 1024.0 * 15.0 - 44.0    # fp16-bits intercept (C=44 minimax)
_ACT_COLS = 640                           # exp cols on ACT; rest on DVE
# strategy:
#   "k66"   - exploit Lorentz structure: time component (the only large
#             score term) as bf16 hi/lo cross-terms, spatial components as
#             single bf16 -> ONE K=66 score matmul. P fp16 from ACT, V fp16
#             hi/lo stack. 2 MMs/step total.
#   "mixed" - bf16 hi/lo pairs for K/Q, P fp16, V fp16 stack. 3 MMs/step.
#   "fp16"  - fp16 hi/lo pairs for K/Q, P fp16, V fp16 stack. 3 MMs/step,
#             but fp16 matmuls are half-rate on PE.
#   "split" - bf16 hi/lo everywhere incl. P (4 MMs/step + DVE splits)
#   "f32"   - exact fp32 fallback (4x slower matmuls)
_STRATEGY = os.environ.get("KERNEL_MM_DT", "v2")
_KSTACK = 66  # rows: [-k0h, -k0l, -k0h, k_space(63)] x [q0h, q0h, q0l, q_space]

_cache = {}


def _ensure_ntff_hook():
    """The agent image lacks ``antenv.axon_hooks``; synthesize it using the
    ctypes NTFF driver from trn_agent_boot so trace=True works."""
    import sys as _sys
    if "antenv.axon_hooks" in _sys.modules:
        return
    try:
        import types as _types
        import antenv  # noqa: F401
        from trn_agent_boot.trn_boot import _ntff_profile_via_ctypes
        hook = _ntff_profile_via_ctypes("/opt/axon/libaxon_pjrt.so")
        m = _types.ModuleType("antenv.axon_hooks")
        m.get_axon_ntff_profile_hook = lambda: hook
        m.set_axon_ntff_profile_hook = lambda h: None
        _sys.modules["antenv.axon_hooks"] = m
    except Exception:
        pass


@with_exitstack
def _body_v2(ctx: ExitStack, tc, aps, bias_val):
    """Fully SBUF-resident K/V/Q, K=66 Lorentz-structured score matmul,
    single-fp16 V. Exp of each 2-step round is split between ACT (true exp,
    cols 0:ACT_COLS) and DVE (Schraudolph fp16-bits exp, rest). Diagonal
    (masked) steps sit at slot END (pads first) so slot starts never stall;
    rounds stream across slot boundaries with a 1-round skew; epilogues are
    spread in small chunks between rounds. rsqrt via DVE bit trick + one
    Newton step keeps the ACT Exp table resident the whole kernel."""
    nc = tc.nc
    PSUM = bass.MemorySpace.PSUM

    const = ctx.enter_context(tc.tile_pool(name="const", bufs=1))
    datap = ctx.enter_context(tc.tile_pool(name="datap", bufs=1))
    expp = ctx.enter_context(tc.tile_pool(name="expp", bufs=3))
    stp = ctx.enter_context(tc.tile_pool(name="stp", bufs=3, space=PSUM))
    mup = ctx.enter_context(tc.tile_pool(name="mup", bufs=2, space=PSUM))
    sbp = ctx.enter_context(tc.tile_pool(name="sbp", bufs=1))
    smallp = ctx.enter_context(tc.tile_pool(name="smallp", bufs=4))
    outp = ctx.enter_context(tc.tile_pool(name="outp", bufs=1))

    ident = const.tile([64, 64], _F32)
    cmasks.make_identity(nc, ident[:])
    bias_t = const.tile([128, 1], _F32)
    nc.vector.memset(bias_t[:], float(bias_val))
    # DVE Schraudolph intercept: bits = A*(ps + b) + B0 = A*ps + (B0 + A*b)
    dve_b = _SCHRAUDOLPH_B0 + _SCHRAUDOLPH_A * float(bias_val)

    # ---- bulk preloads; first two triggers cover slot 0's working set ----
    slot_base = [sum(SLOT_EXTENTS[:i]) for i in range(SLOTS + 1)]
    kd_sb = datap.tile([_KSTACK, TOTAL_STEPS * QT], _BF16)
    nc.sync.dma_start(kd_sb[:, 0:slot_base[1] * QT],
                      aps["kd66"][:, 0:slot_base[1] * QT])
    qd_sb = datap.tile([_KSTACK, NQ_CORE], _BF16)
    nc.sync.dma_start(qd_sb[:, 0:SLOT_Q], aps["qd66"][:, 0:SLOT_Q])
    vn_sb = datap.tile([QT, TOTAL_STEPS * D], _FP16)
    nc.sync.dma_start(vn_sb[:, 0:slot_base[1] * D],
                      aps["vn"][:, 0:slot_base[1] * D])
    nc.sync.dma_start(qd_sb[:, SLOT_Q:], aps["qd66"][:, SLOT_Q:])
    for s0 in range(1, SLOTS):
        nc.sync.dma_start(
            kd_sb[:, slot_base[s0] * QT:slot_base[s0 + 1] * QT],
            aps["kd66"][:, slot_base[s0] * QT:slot_base[s0 + 1] * QT],
        )
        nc.sync.dma_start(
            vn_sb[:, slot_base[s0] * D:slot_base[s0 + 1] * D],
            aps["vn"][:, slot_base[s0] * D:slot_base[s0 + 1] * D],
        )

    # ---- PE warm-up: short matmuls ramp the clock during the DMA fill ----
    wsrc = const.tile([QT, SLOT_Q], _BF16)
    nc.vector.memset(wsrc[:], 0.0)
    n_warm = int(os.environ.get("KERNEL_N_WARM", "6"))
    for w in range(n_warm):
        wps = stp.tile([QT, 2 * SLOT_Q], _F32, tag="ps", name=f"warm{w}")
        nc.tensor.matmul(wps[:, 0:QT], lhsT=wsrc[:, 0:QT],
                         rhs=wsrc[:, 0:QT], start=True, stop=True)

    mu_all = sbp.tile([64, NQ_CORE], _F32)
    muq_all = sbp.tile([128, NT_CORE * D], _F32)
    ln_all = sbp.tile([128, NT_CORE], _F32)
    invs_all = sbp.tile([128, NT_CORE], _F32)
    out_sb = outp.tile([128, NT_CORE * D], _F32)

    def score_round(s, r):
        ps = stp.tile([QT, 2 * SLOT_Q], _F32)
        qblk = qd_sb[:, s * SLOT_Q:(s + 1) * SLOT_Q]
        for h in (0, 1):
            st = slot_base[s] + 2 * r + h
            nc.tensor.matmul(
                ps[:, h * SLOT_Q:(h + 1) * SLOT_Q],
                lhsT=kd_sb[:, st * QT:(st + 1) * QT],
                rhs=qblk,
                start=True, stop=True,
            )
        return ps

    def exp_round(s, r, ps):
        ext = SLOT_EXTENTS[s]
        et = expp.tile([QT, 2 * SLOT_Q], _FP16)
        # step h=0 -> ACT true exp, step h=1 -> DVE Schraudolph bits exp.
        # Diagonal (masked) steps: columns q' < 128*m are fully above the
        # diagonal — skip their exp; the affine_select zero-fills them.
        m0 = 2 * r - (ext - 4)
        if m0 < 0:
            # unmasked round: ACT takes step 0 plus 128 cols of step 1
            act_lo, act_hi = 0, _ACT_COLS
            dve_lo = _ACT_COLS
        else:
            act_lo, act_hi = QT * m0, SLOT_Q
            dve_lo = SLOT_Q + QT * (m0 + 1)
        nc.scalar.activation(
            et[:, act_lo:act_hi], ps[:, act_lo:act_hi],
            mybir.ActivationFunctionType.Exp,
            bias=bias_t[:], scale=1.0,
        )
        nc.vector.tensor_scalar(
            out=et[:, dve_lo:].bitcast(_U16),
            in0=ps[:, dve_lo:],
            scalar1=_SCHRAUDOLPH_A,
            scalar2=dve_b,
            op0=mybir.AluOpType.mult,
            op1=mybir.AluOpType.add,
        )
        for h in (0, 1):
            m = 2 * r + h - (ext - 4)
            if m >= 0:
                nc.gpsimd.affine_select(
                    out=et[:, h * SLOT_Q:(h + 1) * SLOT_Q],
                    in_=et[:, h * SLOT_Q:(h + 1) * SLOT_Q],
                    compare_op=mybir.AluOpType.is_ge,
                    fill=0.0,
                    base=-QT * m,
                    pattern=[[1, SLOT_Q]],
                    channel_multiplier=-1,
                )
        return et

    mu_tiles = {}

    def av_round(s, r, et):
        ext = SLOT_EXTENTS[s]
        if r == 0:
            # [128, 512] tile: AV accumulates mu into rows 0:64; after the
            # drain, the same bank is reused for the epilogue transposes.
            mu_tiles[s] = mup.tile([QT, SLOT_Q], _F32, tag="mu", name=f"mu{s}")
        mu_ps = mu_tiles[s]
        for h in (0, 1):
            st = slot_base[s] + 2 * r + h
            nc.tensor.matmul(
                mu_ps[0:64, :],
                lhsT=vn_sb[:, st * D:(st + 1) * D],
                rhs=et[:, h * SLOT_Q:(h + 1) * SLOT_Q],
                start=(r == 0 and h == 0),
                stop=(2 * r + h == ext - 1),
            )
        if 2 * r + 1 == ext - 1:
            # drain mu to SBUF on ACT (it has the most slack)
            nc.scalar.copy(
                mu_all[:, s * SLOT_Q:(s + 1) * SLOT_Q], mu_ps[0:64, :]
            )

    def epi_lnorm(sp, q):
        """|l| = 2*mu0^2 - sum(mu_d^2) for one q-tile (l is always < 0)."""
        qt_i = sp * 4 + q
        muq = muq_all[:, qt_i * D:(qt_i + 1) * D]
        sq = smallp.tile([QT, D], _F32)
        nc.vector.tensor_mul(sq[:], muq, muq)
        red = smallp.tile([QT, 1], _F32)
        nc.vector.reduce_sum(red[:], sq[:], axis=mybir.AxisListType.X)
        nc.vector.scalar_tensor_tensor(
            out=ln_all[:, qt_i:qt_i + 1],
            in0=sq[:, 0:1],
            scalar=2.0,
            in1=red[:],
            op0=mybir.AluOpType.mult,
            op1=mybir.AluOpType.subtract,
        )

    def epi_invs(sp):
        """invs = exp(-0.5*ln(x)); Ln/Exp/Square/Copy share one ACT table."""
        lns = ln_all[:, sp * 4:(sp + 1) * 4]
        lnt = smallp.tile([128, 4], _F32, tag="lnt")
        nc.scalar.activation(lnt[:], lns, mybir.ActivationFunctionType.Ln)
        nc.scalar.activation(
            invs_all[:, sp * 4:(sp + 1) * 4], lnt[:],
            mybir.ActivationFunctionType.Exp, bias=0.0, scale=-0.5,
        )

    def epi_scale(sp):
        for q in range(4):
            qt_i = sp * 4 + q
            nc.vector.tensor_scalar_mul(
                out_sb[:, qt_i * D:(qt_i + 1) * D],
                muq_all[:, qt_i * D:(qt_i + 1) * D],
                invs_all[:, qt_i:qt_i + 1],
            )
        nc.sync.dma_start(
            aps["out"][:, sp * 256:(sp + 1) * 256],
            out_sb[:, sp * 256:(sp + 1) * 256],
        )

    def epilogue_piece(sp, r):
        """Spread one slot's tail over rounds r=0..5 of the next slot."""
        if r <= 1:
            for q in (2 * r, 2 * r + 1):  # transposes into drained mu bank
                qt_i = sp * 4 + q
                nc.tensor.transpose(
                    mu_tiles[sp][:, q * 64:(q + 1) * 64],
                    mu_all[:, qt_i * QT:(qt_i + 1) * QT], ident[:],
                )
        elif r == 2:
            # all 4 tp's -> SBUF in one DVE copy
            nc.vector.tensor_copy(
                muq_all[:, sp * 256:(sp + 1) * 256], mu_tiles[sp][:, 0:256]
            )
        elif r == 3:
            epi_lnorm(sp, 0)
            epi_lnorm(sp, 1)
        elif r == 4:
            epi_lnorm(sp, 2)
            epi_lnorm(sp, 3)
            epi_invs(sp)
        elif r == 5:
            epi_scale(sp)

    # ---- flat pipelined stream over all rounds, 2-round AV skew ----
    rounds = [(s, r) for s in range(SLOTS) for r in range(SLOT_EXTENTS[s] // 2)]
    pend = []
    for s, r in rounds:
        ps = score_round(s, r)
        if len(pend) >= 2:
            av_round(*pend.pop(0))
        if s > 0 and 2 <= r <= 7:
            epilogue_piece(s - 1, r - 2)
        et = exp_round(s, r, ps)
        pend.append((s, r, et))
    for p in pend:
        av_round(*p)
    for r in range(6):
        epilogue_piece(SLOTS - 1, r)


@with_exitstack
def _body_k66(ctx: ExitStack, tc, aps, bias_val):
    """Single K=66 score matmul per step (Lorentz-structured hi/lo),
    fp16 P/V attention matmul. Skewed pipeline."""
    nc = tc.nc
    PSUM = bass.MemorySpace.PSUM

    const = ctx.enter_context(tc.tile_pool(name="const", bufs=1))
    qdp = ctx.enter_context(tc.tile_pool(name="qdp", bufs=1))
    kdp = ctx.enter_context(tc.tile_pool(name="kdp", bufs=4))
    vnp = ctx.enter_context(tc.tile_pool(name="vnp", bufs=4))
    expp = ctx.enter_context(tc.tile_pool(name="expp", bufs=3))
    stp = ctx.enter_context(tc.tile_pool(name="stp", bufs=3, space=PSUM))
    mup = ctx.enter_context(tc.tile_pool(name="mup", bufs=2, space=PSUM))
    tpp = ctx.enter_context(tc.tile_pool(name="tpp", bufs=2, space=PSUM))
    sbp = ctx.enter_context(tc.tile_pool(name="sbp", bufs=1))
    smallp = ctx.enter_context(tc.tile_pool(name="smallp", bufs=4))
    outp = ctx.enter_context(tc.tile_pool(name="outp", bufs=3))

    ident = const.tile([64, 64], _F32)
    cmasks.make_identity(nc, ident[:])
    bias_t = const.tile([128, 1], _F32)
    nc.vector.memset(bias_t[:], float(bias_val))

    qd_sb = qdp.tile([_KSTACK, NQ_CORE], _BF16)
    for s0 in range(SLOTS):
        c0, c1 = s0 * SLOT_Q, (s0 + 1) * SLOT_Q
        nc.sync.dma_start(qd_sb[:, c0:c1], aps["qd66"][:, c0:c1])

    mu_sb = sbp.tile([64, NQ_CORE], _F32)
    muq_all = sbp.tile([128, NT_CORE * D], _F32)
    ln_all = sbp.tile([128, NT_CORE], _F32)

    step_base = 0
    for s in range(SLOTS):
        ext = SLOT_EXTENTS[s]
        q_lo = s * SLOT_Q
        mu_ps = mup.tile([QT, SLOT_Q], _F32)  # rows 0-63 hi, 64-127 lo

        def st_step(i):
            st = step_base + i
            kt = kdp.tile([_KSTACK, QT], _BF16)
            nc.sync.dma_start(kt[:], aps["kd66"][:, st * QT:(st + 1) * QT])
            ps = stp.tile([QT, SLOT_Q], _F32)
            nc.tensor.matmul(
                ps[:], lhsT=kt[:], rhs=qd_sb[:, q_lo:q_lo + SLOT_Q],
                start=True, stop=True,
            )
            return ps

        def av_step(i, ps):
            st = step_base + i
            vt = vnp.tile([QT, QT], _FP16)
            nc.sync.dma_start(vt[:], aps["vn"][st * QT:(st + 1) * QT, :])
            et = expp.tile([QT, SLOT_Q], _FP16)
            nc.scalar.activation(
                et[:], ps[:], mybir.ActivationFunctionType.Exp,
                bias=bias_t[:], scale=1.0,
            )
            if i < 4:
                nc.gpsimd.affine_select(
                    out=et[:], in_=et[:],
                    compare_op=mybir.AluOpType.is_ge,
                    fill=0.0,
                    base=-QT * (3 - i),
                    pattern=[[1, SLOT_Q]],
                    channel_multiplier=-1,
                )
            nc.tensor.matmul(
                mu_ps[:], lhsT=vt[:], rhs=et[:],
                start=(i == 0), stop=(i == ext - 1),
            )

        ps_prev = st_step(0)
        for i in range(1, ext):
            ps_i = st_step(i)
            av_step(i - 1, ps_prev)
            ps_prev = ps_i
        av_step(ext - 1, ps_prev)
        step_base += ext

        lo_sb = smallp.tile([64, SLOT_Q], _F32, tag="losb")
        nc.scalar.copy(lo_sb[:], mu_ps[64:128, :])
        nc.vector.tensor_add(mu_sb[:, q_lo:q_lo + SLOT_Q], mu_ps[0:64, :], lo_sb[:])

        for q in range(SLOT_Q // QT):
            qt_i = s * (SLOT_Q // QT) + q
            tp = tpp.tile([QT, 64], _F32)
            nc.tensor.transpose(
                tp[:], mu_sb[:, qt_i * QT:(qt_i + 1) * QT], ident[:]
            )
            muq = muq_all[:, qt_i * D:(qt_i + 1) * D]
            nc.scalar.copy(muq, tp[:, :D])
            sq = smallp.tile([QT, D], _F32)
            nc.vector.tensor_mul(sq[:], muq, muq)
            red = smallp.tile([QT, 1], _F32)
            nc.vector.reduce_sum(red[:], sq[:], axis=mybir.AxisListType.X)
            nc.vector.scalar_tensor_tensor(
                out=ln_all[:, qt_i:qt_i + 1],
                in0=sq[:, 0:1],
                scalar=2.0,
                in1=red[:],
                op0=mybir.AluOpType.mult,
                op1=mybir.AluOpType.subtract,
            )

        # per-slot normalize: 1/sqrt(x) = exp(-0.5*ln(x)); Ln and Exp share
        # one ACT table set, so no table switch and no end-of-kernel phase.
        lns = ln_all[:, s * 4:(s + 1) * 4]
        lnt = smallp.tile([128, 4], _F32, tag="lnt")
        nc.scalar.activation(lnt[:], lns, mybir.ActivationFunctionType.Ln)
        invs = smallp.tile([128, 4], _F32, tag="invs")
        nc.scalar.activation(
            invs[:], lnt[:], mybir.ActivationFunctionType.Exp,
            bias=0.0, scale=-0.5,
        )
        for q in range(SLOT_Q // QT):
            qt_i = s * (SLOT_Q // QT) + q
            ot = outp.tile([QT, D], _F32)
            nc.vector.tensor_scalar_mul(
                ot[:], muq_all[:, qt_i * D:(qt_i + 1) * D], invs[:, q:q + 1]
            )
            nc.sync.dma_start(aps["out"][qt_i * QT:(qt_i + 1) * QT, :], ot[:])


@with_exitstack
def _body_fp16(ctx: ExitStack, tc, aps, bias_val, kq_dt=_FP16):
    """hi/lo-pair strategy with software-pipelined (skewed) step loop and
    per-slot preloaded K/V (per-step DMA triggers serialize on the sync
    sequencer at ~590ns each, so they must be batched).
    kq_dt: dtype of the K/Q score operands (bf16 = PE full rate)."""
    nc = tc.nc
    PSUM = bass.MemorySpace.PSUM

    const = ctx.enter_context(tc.tile_pool(name="const", bufs=1))
    qdp = ctx.enter_context(tc.tile_pool(name="qdp", bufs=1))
    kdp = ctx.enter_context(tc.tile_pool(name="kdp", bufs=1))
    vnp = ctx.enter_context(tc.tile_pool(name="vnp", bufs=1))
    expp = ctx.enter_context(tc.tile_pool(name="expp", bufs=3))
    stp = ctx.enter_context(tc.tile_pool(name="stp", bufs=3, space=PSUM))
    mup = ctx.enter_context(tc.tile_pool(name="mup", bufs=2, space=PSUM))
    tpp = ctx.enter_context(tc.tile_pool(name="tpp", bufs=2, space=PSUM))
    sbp = ctx.enter_context(tc.tile_pool(name="sbp", bufs=1))
    smallp = ctx.enter_context(tc.tile_pool(name="smallp", bufs=4))
    outp = ctx.enter_context(tc.tile_pool(name="outp", bufs=3))

    ident = const.tile([64, 64], _F32)
    cmasks.make_identity(nc, ident[:])
    bias_t = const.tile([128, 1], _F32)
    nc.vector.memset(bias_t[:], float(bias_val))

    # PE warm-up: ~16 dummy matmuls during the initial DMA window so the
    # HAM clock-gate reaches 2.4 GHz before the first real matmul.
    wsrc = const.tile([QT, SLOT_Q], kq_dt)
    nc.gpsimd.memset(wsrc[:], 0.0)
    for w in range(16):
        wps = tpp.tile([QT, SLOT_Q], _F32, tag="warm", bufs=1)
        nc.tensor.matmul(wps[:], lhsT=wsrc[:, 0:QT], rhs=wsrc[:],
                         start=True, stop=True)

    # causal boundary masks (fp16 ones/zeros incl. diagonal triangle),
    # applied with a DVE multiply instead of a gpsimd affine_select on the
    # exp->AV critical path.
    bmask = const.tile([QT, 4, SLOT_Q], _FP16)
    nc.vector.memset(bmask[:], 1.0)
    for i in range(4):
        nc.gpsimd.affine_select(
            out=bmask[:, i, :], in_=bmask[:, i, :],
            compare_op=mybir.AluOpType.is_ge,
            fill=0.0,
            base=-QT * (3 - i),
            pattern=[[1, SLOT_Q]],
            channel_multiplier=-1,
        )

    qdh_sb = qdp.tile([128, NQ_CORE], kq_dt)
    qdl_sb = qdp.tile([64, NQ_CORE], kq_dt)
    kd_sb = {}
    vn_sb = {}
    base = 0
    for s0 in range(SLOTS):
        ext = SLOT_EXTENTS[s0]
        c0, c1 = s0 * SLOT_Q, (s0 + 1) * SLOT_Q
        nc.sync.dma_start(qdh_sb[:, c0:c1], aps["qdh"][:, c0:c1])
        nc.sync.dma_start(qdl_sb[:, c0:c1], aps["qdl"][:, c0:c1])
        kd_sb[s0] = kdp.tile([QT, ext * QT], kq_dt, tag=f"kd{s0}", name=f"kd_sb{s0}")
        nc.sync.dma_start(kd_sb[s0][:], aps["kd"][:, base * QT:(base + ext) * QT])
        vn_sb[s0] = vnp.tile([QT, ext, QT], _FP16, tag=f"vn{s0}", name=f"vn_sb{s0}")
        vsrc = aps["vn"][base * QT:(base + ext) * QT, :].rearrange(
            "(t p) c -> p t c", p=QT)
        nc.sync.dma_start(vn_sb[s0][:], vsrc)
        base += ext

    mu_sb = sbp.tile([64, NQ_CORE], _F32)
    muq_all = sbp.tile([128, NT_CORE * D], _F32)
    ln_all = sbp.tile([128, NT_CORE], _F32)

    step_base = 0
    for s in range(SLOTS):
        ext = SLOT_EXTENTS[s]
        q_lo = s * SLOT_Q
        mu_ps = mup.tile([QT, SLOT_Q], _F32)  # rows 0-63 hi, 64-127 lo

        def st_step(i):
            kt = kd_sb[s][:, i * QT:(i + 1) * QT]
            ps = stp.tile([QT, SLOT_Q], _F32)
            nc.tensor.matmul(
                ps[:], lhsT=kt, rhs=qdh_sb[:, q_lo:q_lo + SLOT_Q],
                start=True, stop=False,
            )
            nc.tensor.matmul(
                ps[:], lhsT=kt[0:64, :], rhs=qdl_sb[:, q_lo:q_lo + SLOT_Q],
                start=False, stop=True,
            )
            return ps

        def av_step(i, ps):
            vt = vn_sb[s][:, i, :]
            et = expp.tile([QT, SLOT_Q], _FP16)
            nc.scalar.activation(
                et[:], ps[:], mybir.ActivationFunctionType.Exp,
                bias=bias_t[:], scale=1.0,
            )
            if i < 4:
                # step i's k-tile is the (3-i)'th q-tile block's diagonal
                nc.vector.tensor_mul(et[:], et[:], bmask[:, i, :])
            nc.tensor.matmul(
                mu_ps[:], lhsT=vt, rhs=et[:],
                start=(i == 0), stop=(i == ext - 1),
            )

        # 2-deep skewed pipeline: S_T(i+2) runs on PE before AV(i), covering
        # the exp latency (and the boundary-mask multiply) on ACT/DVE.
        pending = [st_step(0), st_step(1)]
        for i in range(2, ext):
            pending.append(st_step(i))
            av_step(i - 2, pending.pop(0))
        av_step(ext - 2, pending.pop(0))
        av_step(ext - 1, pending.pop(0))
        step_base += ext

        # mu = hi half + lo half (one PSUM operand max per DVE op)
        lo_sb = smallp.tile([64, SLOT_Q], _F32, tag="losb")
        nc.scalar.copy(lo_sb[:], mu_ps[64:128, :])
        nc.vector.tensor_add(mu_sb[:, q_lo:q_lo + SLOT_Q], mu_ps[0:64, :], lo_sb[:])

        for q in range(SLOT_Q // QT):
            qt_i = s * (SLOT_Q // QT) + q
            tp = tpp.tile([QT, 64], _F32)
            nc.tensor.transpose(
                tp[:], mu_sb[:, qt_i * QT:(qt_i + 1) * QT], ident[:]
            )
            muq = muq_all[:, qt_i * D:(qt_i + 1) * D]
            nc.scalar.copy(muq, tp[:, :D])
            sq = smallp.tile([QT, D], _F32)
            nc.vector.tensor_mul(sq[:], muq, muq)
            red = smallp.tile([QT, 1], _F32)
            nc.vector.reduce_sum(red[:], sq[:], axis=mybir.AxisListType.X)
            # |l| = -l = 2*mu0^2 - sum(mu_d^2)  (l is always < 0 here)
            nc.vector.scalar_tensor_tensor(
                out=ln_all[:, qt_i:qt_i + 1],
                in0=sq[:, 0:1],
                scalar=2.0,
                in1=red[:],
                op0=mybir.AluOpType.mult,
                op1=mybir.AluOpType.subtract,
            )

        # per-slot normalize: 1/sqrt(x) = exp(-0.5*ln(x)); Ln and Exp share
        # one ACT table set, so no table switch and no end-of-kernel phase.
        lns = ln_all[:, s * 4:(s + 1) * 4]
        lnt = smallp.tile([128, 4], _F32, tag="lnt")
        nc.scalar.activation(lnt[:], lns, mybir.ActivationFunctionType.Ln)
        invs = smallp.tile([128, 4], _F32, tag="invs")
        nc.scalar.activation(
            invs[:], lnt[:], mybir.ActivationFunctionType.Exp,
            bias=0.0, scale=-0.5,
        )
        for q in range(SLOT_Q // QT):
            qt_i = s * (SLOT_Q // QT) + q
            ot = outp.tile([QT, D], _F32)
            nc.vector.tensor_scalar_mul(
                ot[:], muq_all[:, qt_i * D:(qt_i + 1) * D], invs[:, q:q + 1]
            )
            nc.sync.dma_start(aps["out"][qt_i * QT:(qt_i + 1) * QT, :], ot[:])


@with_exitstack
def _body_split(ctx: ExitStack, tc, aps, bias_val):
    """bf16 hi/lo strategy. aps: dict of DRAM APs."""
    nc = tc.nc
    PSUM = bass.MemorySpace.PSUM

    const = ctx.enter_context(tc.tile_pool(name="const", bufs=1))
    qdp = ctx.enter_context(tc.tile_pool(name="qdp", bufs=1))
    kdp = ctx.enter_context(tc.tile_pool(name="kdp", bufs=4))
    vnp = ctx.enter_context(tc.tile_pool(name="vnp", bufs=4))
    expp = ctx.enter_context(tc.tile_pool(name="expp", bufs=3))
    ehp = ctx.enter_context(tc.tile_pool(name="ehp", bufs=3))
    elp = ctx.enter_context(tc.tile_pool(name="elp", bufs=3))
    stp = ctx.enter_context(tc.tile_pool(name="stp", bufs=2, space=PSUM))
    mup = ctx.enter_context(tc.tile_pool(name="mup", bufs=2, space=PSUM))
    tpp = ctx.enter_context(tc.tile_pool(name="tpp", bufs=2, space=PSUM))
    sbp = ctx.enter_context(tc.tile_pool(name="sbp", bufs=1))
    smallp = ctx.enter_context(tc.tile_pool(name="smallp", bufs=4))
    outp = ctx.enter_context(tc.tile_pool(name="outp", bufs=3))

    ident = const.tile([64, 64], _F32)
    cmasks.make_identity(nc, ident[:])
    bias_t = const.tile([128, 1], _F32)
    nc.vector.memset(bias_t[:], float(bias_val))

    qdh_sb = qdp.tile([128, NQ_CORE], _BF16)
    nc.sync.dma_start(qdh_sb[:], aps["qdh"][:])
    qdl_sb = qdp.tile([64, NQ_CORE], _BF16)
    nc.sync.dma_start(qdl_sb[:], aps["qdl"][:])

    mu_sb = sbp.tile([64, NQ_CORE], _F32)
    muq_all = sbp.tile([128, NT_CORE * D], _F32)
    ln_all = sbp.tile([128, NT_CORE], _F32)

    step_base = 0
    for s in range(SLOTS):
        ext = SLOT_EXTENTS[s]
        q_lo = s * SLOT_Q
        mu_ps = mup.tile([QT, SLOT_Q], _F32)  # rows 0-63 hi, 64-127 lo
        for i in range(ext):
            st = step_base + i
            kt = kdp.tile([QT, QT], _BF16)
            nc.sync.dma_start(kt[:], aps["kd"][:, st * QT:(st + 1) * QT])
            vt = vnp.tile([QT, QT], _BF16)
            nc.sync.dma_start(vt[:], aps["vn"][st * QT:(st + 1) * QT, :])

            ps = stp.tile([QT, SLOT_Q], _F32)
            nc.tensor.matmul(
                ps[:], lhsT=kt[:], rhs=qdh_sb[:, q_lo:q_lo + SLOT_Q],
                start=True, stop=False,
            )
            nc.tensor.matmul(
                ps[:], lhsT=kt[0:64, :], rhs=qdl_sb[:, q_lo:q_lo + SLOT_Q],
                start=False, stop=True,
            )
            et = expp.tile([QT, SLOT_Q], _F32)
            nc.scalar.activation(
                et[:], ps[:], mybir.ActivationFunctionType.Exp,
                bias=bias_t[:], scale=1.0,
            )
            if i < 4:
                # step i's k-tile is the (3-i)'th q-tile block's diagonal:
                # keep element (k, q) iff q - k - 128*(3-i) >= 0
                nc.gpsimd.affine_select(
                    out=et[:], in_=et[:],
                    compare_op=mybir.AluOpType.is_ge,
                    fill=0.0,
                    base=-QT * (3 - i),
                    pattern=[[1, SLOT_Q]],
                    channel_multiplier=-1,
                )
            eth = ehp.tile([QT, SLOT_Q], _BF16)
            nc.vector.tensor_copy(eth[:], et[:])
            etl = elp.tile([QT, SLOT_Q], _BF16)
            nc.vector.tensor_sub(etl[:], et[:], eth[:])
            nc.tensor.matmul(
                mu_ps[:], lhsT=vt[:], rhs=eth[:],
                start=(i == 0), stop=False,
            )
            nc.tensor.matmul(
                mu_ps[:], lhsT=vt[:], rhs=etl[:],
                start=False, stop=(i == ext - 1),
            )
        step_base += ext

        # mu = hi half + lo half (one PSUM operand max per DVE op)
        lo_sb = smallp.tile([64, SLOT_Q], _F32, tag="losb")
        nc.scalar.copy(lo_sb[:], mu_ps[64:128, :])
        nc.vector.tensor_add(mu_sb[:, q_lo:q_lo + SLOT_Q], mu_ps[0:64, :], lo_sb[:])

        for q in range(SLOT_Q // QT):
            qt_i = s * (SLOT_Q // QT) + q
            tp = tpp.tile([QT, 64], _F32)
            nc.tensor.transpose(
                tp[:], mu_sb[:, qt_i * QT:(qt_i + 1) * QT], ident[:]
            )
            muq = muq_all[:, qt_i * D:(qt_i + 1) * D]
            nc.scalar.copy(muq, tp[:, :D])
            sq = smallp.tile([QT, D], _F32)
            nc.vector.tensor_mul(sq[:], muq, muq)
            red = smallp.tile([QT, 1], _F32)
            nc.vector.reduce_sum(red[:], sq[:], axis=mybir.AxisListType.X)
            # |l| = -l = 2*mu0^2 - sum(mu_d^2)  (l is always < 0 here)
            nc.vector.scalar_tensor_tensor(
                out=ln_all[:, qt_i:qt_i + 1],
                in0=sq[:, 0:1],
                scalar=2.0,
                in1=red[:],
                op0=mybir.AluOpType.mult,
                op1=mybir.AluOpType.subtract,
            )

    # grouped sqrt (single ACT table switch) + reciprocal + final scale
    sqv = sbp.tile([128, NT_CORE], _F32)
    nc.scalar.activation(
        sqv[:], ln_all[:], mybir.ActivationFunctionType.Sqrt,
        bias=0.0, scale=1.0,
    )
    inv = sbp.tile([128, NT_CORE], _F32)
    nc.vector.reciprocal(inv[:], sqv[:])
    for qt_i in range(NT_CORE):
        ot = outp.tile([QT, D], _F32)
        nc.vector.tensor_scalar_mul(
            ot[:], muq_all[:, qt_i * D:(qt_i + 1) * D], inv[:, qt_i:qt_i + 1]
        )
        nc.sync.dma_start(aps["out"][qt_i * QT:(qt_i + 1) * QT, :], ot[:])


@with_exitstack
def _body_f32(ctx: ExitStack, tc, aps, bias_val):
    """Exact-fp32 fallback strategy."""
    nc = tc.nc
    PSUM = bass.MemorySpace.PSUM

    const = ctx.enter_context(tc.tile_pool(name="const", bufs=1))
    qdp = ctx.enter_context(tc.tile_pool(name="qdp", bufs=1))
    kdp = ctx.enter_context(tc.tile_pool(name="kdp", bufs=4))
    vnp = ctx.enter_context(tc.tile_pool(name="vnp", bufs=4))
    expp = ctx.enter_context(tc.tile_pool(name="expp", bufs=3))
    stp = ctx.enter_context(tc.tile_pool(name="stp", bufs=2, space=PSUM))
    mup = ctx.enter_context(tc.tile_pool(name="mup", bufs=2, space=PSUM))
    tpp = ctx.enter_context(tc.tile_pool(name="tpp", bufs=2, space=PSUM))
    sbp = ctx.enter_context(tc.tile_pool(name="sbp", bufs=1))
    smallp = ctx.enter_context(tc.tile_pool(name="smallp", bufs=4))
    outp = ctx.enter_context(tc.tile_pool(name="outp", bufs=3))

    ident = const.tile([64, 64], _F32)
    cmasks.make_identity(nc, ident[:])
    bias_t = const.tile([128, 1], _F32)
    nc.vector.memset(bias_t[:], float(bias_val))

    qd_sb = qdp.tile([64, NQ_CORE], _F32)
    nc.sync.dma_start(qd_sb[:], aps["qd"][:])

    mu_sb = sbp.tile([64, NQ_CORE], _F32)
    muq_all = sbp.tile([128, NT_CORE * D], _F32)
    ln_all = sbp.tile([128, NT_CORE], _F32)

    step_base = 0
    for s in range(SLOTS):
        ext = SLOT_EXTENTS[s]
        q_lo = s * SLOT_Q
        mu_ps = mup.tile([64, SLOT_Q], _F32)
        for i in range(ext):
            st = step_base + i
            kt = kdp.tile([64, QT], _F32)
            nc.sync.dma_start(kt[:], aps["kd"][:, st * QT:(st + 1) * QT])
            vt = vnp.tile([QT, D], _F32)
            nc.sync.dma_start(vt[:], aps["vn"][st * QT:(st + 1) * QT, :])

            ps = stp.tile([QT, SLOT_Q], _F32)
            nc.tensor.matmul(
                ps[:], lhsT=kt[:], rhs=qd_sb[:, q_lo:q_lo + SLOT_Q],
                start=True, stop=True,
            )
            et = expp.tile([QT, SLOT_Q], _F32)
            nc.scalar.activation(
                et[:], ps[:], mybir.ActivationFunctionType.Exp,
                bias=bias_t[:], scale=1.0,
            )
            if i < 4:
                nc.gpsimd.affine_select(
                    out=et[:], in_=et[:],
                    compare_op=mybir.AluOpType.is_ge,
                    fill=0.0,
                    base=-QT * (3 - i),
                    pattern=[[1, SLOT_Q]],
                    channel_multiplier=-1,
                )
            nc.tensor.matmul(
                mu_ps[:], lhsT=vt[:], rhs=et[:],
                start=(i == 0), stop=(i == ext - 1),
            )
        step_base += ext

        nc.vector.tensor_copy(mu_sb[:, q_lo:q_lo + SLOT_Q], mu_ps[:])
        for q in range(SLOT_Q // QT):
            qt_i = s * (SLOT_Q // QT) + q
            tp = tpp.tile([QT, 64], _F32)
            nc.tensor.transpose(
                tp[:], mu_sb[:, qt_i * QT:(qt_i + 1) * QT], ident[:]
            )
            muq = muq_all[:, qt_i * D:(qt_i + 1) * D]
            nc.scalar.copy(muq, tp[:, :D])
            sq = smallp.tile([QT, D], _F32)
            nc.vector.tensor_mul(sq[:], muq, muq)
            red = smallp.tile([QT, 1], _F32)
            nc.vector.reduce_sum(red[:], sq[:], axis=mybir.AxisListType.X)
            nc.vector.scalar_tensor_tensor(
                out=ln_all[:, qt_i:qt_i + 1],
                in0=sq[:, 0:1],
                scalar=2.0,
                in1=red[:],
                op0=mybir.AluOpType.mult,
                op1=mybir.AluOpType.subtract,
            )

    sqv = sbp.tile([128, NT_CORE], _F32)
    nc.scalar.activation(
        sqv[:], ln_all[:], mybir.ActivationFunctionType.Sqrt,
        bias=0.0, scale=1.0,
    )
    inv = sbp.tile([128, NT_CORE], _F32)
    nc.vector.reciprocal(inv[:], sqv[:])
    for qt_i in range(NT_CORE):
        ot = outp.tile([QT, D], _F32)
        nc.vector.tensor_scalar_mul(
            ot[:], muq_all[:, qt_i * D:(qt_i + 1) * D], inv[:, qt_i:qt_i + 1]
        )
        nc.sync.dma_start(aps["out"][qt_i * QT:(qt_i + 1) * QT, :], ot[:])


def _build_program(bias_val):
    key = (round(float(bias_val), 12), _STRATEGY)
    if key in _cache:
        return _cache[key]
    nc = bacc.Bacc(
        "TRN2",
        target_bir_lowering=False,
        debug=False,
        enable_asserts=False,
    )
    aps = {}
    if _STRATEGY == "v2":
        aps["qd66"] = nc.dram_tensor("qd66", [_KSTACK, NQ_CORE], _BF16, kind="ExternalInput").ap()
        aps["kd66"] = nc.dram_tensor("kd66", [_KSTACK, TOTAL_STEPS * QT], _BF16, kind="ExternalInput").ap()
        aps["vn"] = nc.dram_tensor("vn", [QT, TOTAL_STEPS * D], _FP16, kind="ExternalInput").ap()
        aps["out"] = nc.dram_tensor("out", [128, NT_CORE * D], _F32, kind="ExternalOutput").ap()
        with tile.TileContext(nc) as tc:
            _body_v2(tc, aps, bias_val)
        nc.compile()
        _cache[key] = nc
        return nc
    if _STRATEGY == "k66":
        aps["qd66"] = nc.dram_tensor("qd66", [_KSTACK, NQ_CORE], _BF16, kind="ExternalInput").ap()
        aps["kd66"] = nc.dram_tensor("kd66", [_KSTACK, TOTAL_STEPS * QT], _BF16, kind="ExternalInput").ap()
        aps["vn"] = nc.dram_tensor("vn", [TOTAL_STEPS * QT, 128], _FP16, kind="ExternalInput").ap()
    elif _STRATEGY in ("split", "fp16", "mixed"):
        kq_dt = _BF16 if _STRATEGY in ("split", "mixed") else _FP16
        pv_dt = _BF16 if _STRATEGY == "split" else _FP16
        aps["qdh"] = nc.dram_tensor("qdh", [128, NQ_CORE], kq_dt, kind="ExternalInput").ap()
        aps["qdl"] = nc.dram_tensor("qdl", [64, NQ_CORE], kq_dt, kind="ExternalInput").ap()
        aps["kd"] = nc.dram_tensor("kd", [128, TOTAL_STEPS * QT], kq_dt, kind="ExternalInput").ap()
        aps["vn"] = nc.dram_tensor("vn", [TOTAL_STEPS * QT, 128], pv_dt, kind="ExternalInput").ap()
    else:
        aps["qd"] = nc.dram_tensor("qd", [64, NQ_CORE], _F32, kind="ExternalInput").ap()
        aps["kd"] = nc.dram_tensor("kd", [64, TOTAL_STEPS * QT], _F32, kind="ExternalInput").ap()
        aps["vn"] = nc.dram_tensor("vn", [TOTAL_STEPS * QT, D], _F32, kind="ExternalInput").ap()
    aps["out"] = nc.dram_tensor("out", [NQ_CORE, D], _F32, kind="ExternalOutput").ap()
    with tile.TileContext(nc) as tc:
        if _STRATEGY == "k66":
            _body_k66(tc, aps, bias_val)
        elif _STRATEGY == "mixed":
            _body_fp16(tc, aps, bias_val, kq_dt=_BF16)
        elif _STRATEGY == "fp16":
            _body_fp16(tc, aps, bias_val, kq_dt=_FP16)
        elif _STRATEGY == "split":
            _body_split(tc, aps, bias_val)
        else:
            _body_f32(tc, aps, bias_val)
    nc.compile()
    _cache[key] = nc
    return nc


def _hilo(x, np_dt):
    hi = x.astype(np_dt)
    lo = (x - hi.astype(np.float32)).astype(np_dt)
    return hi, lo


def _prep_core_inputs_v2(Q, b, half, a_scale):
    """v2 layouts: kd66 as in k66 but pads reuse a real k-tile (scores stay
    in the normal range); vn transposed to [128, steps*64] fp16 with zero
    pads (zero V rows nullify pad steps, so no score poison is needed)."""
    groups = HALF_GROUPS[half]
    Qb = Q[b]  # [L, D]
    qd = np.empty((64, NQ_CORE), np.float32)
    kd = np.empty((64, TOTAL_STEPS * QT), np.float32)
    vn = np.zeros((TOTAL_STEPS, QT, D), np.float32)
    blk0 = Qb[0:QT, :]  # pad k-tile: any real tile keeps scores bounded
    kdb0 = blk0.T.copy()
    kdb0[0, :] = -kdb0[0, :]
    step_base = 0
    for s, g in enumerate(groups):
        ext = SLOT_EXTENTS[s]
        qd[:, s * SLOT_Q:(s + 1) * SLOT_Q] = (
            Qb[g * SLOT_Q:(g + 1) * SLOT_Q, :].T * a_scale
        )
        n_real = 4 * g + 4
        pads = ext - n_real
        # pads FIRST (zero V nullifies them), then k-tiles ascending so the
        # 4 diagonal tiles land at static steps ext-4..ext-1 (mask steps).
        for i in range(ext):
            st = step_base + i
            c0 = st * QT
            if i >= pads:
                j = i - pads  # ascending 0..n_real-1
                blk = Qb[j * QT:(j + 1) * QT, :]
                kdb = blk.T.copy()
                kdb[0, :] = -kdb[0, :]  # Lorentz signature on time component
                kd[:, c0:c0 + QT] = kdb
                vn[st] = blk
            else:
                kd[:, c0:c0 + QT] = kdb0
                # vn stays zero
        step_base += ext
    k0h, k0l = _hilo(kd[0:1], _BF16_NP)
    q0h, q0l = _hilo(qd[0:1], _BF16_NP)
    kd66 = np.empty((_KSTACK, TOTAL_STEPS * QT), _BF16_NP)
    kd66[0] = k0h
    kd66[1] = k0l
    kd66[2] = k0h
    kd66[3:] = kd[1:].astype(_BF16_NP)
    qd66 = np.empty((_KSTACK, NQ_CORE), _BF16_NP)
    qd66[0] = q0h
    qd66[1] = q0h
    qd66[2] = q0l
    qd66[3:] = qd[1:].astype(_BF16_NP)
    # [steps, 128k, 64d] -> [128k, steps*64]
    vn_pm = np.ascontiguousarray(
        vn.transpose(1, 0, 2).reshape(QT, TOTAL_STEPS * D)
    ).astype(np.float16)
    return {"qd66": qd66, "kd66": kd66, "vn": vn_pm}


def _prep_core_inputs(Q, b, half, a_scale, poison):
    """Build per-core input arrays. a_scale folded into q."""
    groups = HALF_GROUPS[half]
    Qb = Q[b]  # [L, D]
    qd = np.empty((64, NQ_CORE), np.float32)
    kd = np.empty((64, TOTAL_STEPS * QT), np.float32)
    vn = np.zeros((TOTAL_STEPS * QT, D), np.float32)
    step_base = 0
    for s, g in enumerate(groups):
        ext = SLOT_EXTENTS[s]
        qd[:, s * SLOT_Q:(s + 1) * SLOT_Q] = (
            Qb[g * SLOT_Q:(g + 1) * SLOT_Q, :].T * a_scale
        )
        n_real = 4 * g + 4  # causal extent of this group in k-tiles
        for i in range(ext):
            c0 = (step_base + i) * QT
            if i < n_real:
                j = 4 * g + 3 - i  # descending from the diagonal
                blk = Qb[j * QT:(j + 1) * QT, :]  # [128, 64]
                kdb = blk.T.copy()
                kdb[0, :] = -kdb[0, :]  # Lorentz signature on time component
                kd[:, c0:c0 + QT] = kdb
                vn[c0:c0 + QT, :] = blk
            else:
                kd[:, c0:c0 + QT] = 0.0
                kd[0, c0:c0 + QT] = poison
                # vn stays zero
        step_base += ext
    if _STRATEGY == "k66":
        # kd rows already carry the Lorentz sign on row 0 (time).
        k0h, k0l = _hilo(kd[0:1], _BF16_NP)      # signed time component
        q0h, q0l = _hilo(qd[0:1], _BF16_NP)
        kd66 = np.empty((_KSTACK, TOTAL_STEPS * QT), _BF16_NP)
        kd66[0] = k0h
        kd66[1] = k0l
        kd66[2] = k0h
        kd66[3:] = kd[1:].astype(_BF16_NP)
        qd66 = np.empty((_KSTACK, NQ_CORE), _BF16_NP)
        qd66[0] = q0h
        qd66[1] = q0h
        qd66[2] = q0l
        qd66[3:] = qd[1:].astype(_BF16_NP)
        vh, vl = _hilo(vn, np.float16)
        vns = np.concatenate([vh, vl], axis=1)   # [steps*128, 128]
        return {"qd66": qd66, "kd66": kd66, "vn": np.ascontiguousarray(vns)}
    if _STRATEGY not in ("split", "fp16", "mixed"):
        return {"qd": qd, "kd": kd, "vn": vn}
    np_dt = _BF16_NP if _STRATEGY in ("split", "mixed") else np.float16
    pv_np = _BF16_NP if _STRATEGY == "split" else np.float16
    qh, ql = _hilo(qd, np_dt)
    kh, kl = _hilo(kd, np_dt)
    vh, vl = _hilo(vn, pv_np)
    qdh = np.empty((128, NQ_CORE), np_dt)
    qdh[0:64] = qh
    qdh[64:128] = qh  # replicated: both halves of the K-stack see Q_hi
    kds = np.concatenate([kh, kl], axis=0)       # [128, steps*128]
    vns = np.concatenate([vh, vl], axis=1)       # [steps*128, 128]
    return {"qdh": qdh, "qdl": ql, "kd": np.ascontiguousarray(kds),
            "vn": np.ascontiguousarray(vns)}


def _mask_fixup(out, Q, mask, scale_v, bias_v):
    """Reference masks by QUERY row: a masked row becomes a uniform softmax
    over ALL L keys (causal entries equally -inf). Recompute those rows."""
    for b in range(B):
        rows = np.nonzero(mask[b])[0]
        if len(rows) == 0:
            continue
        mu = Q[b].mean(axis=0)  # uniform attention over all keys
        l_norm = -mu[0] ** 2 + np.sum(mu[1:] ** 2)
        denom = np.sqrt(max(abs(l_norm), EPS))
        out[b, rows, :] = (mu / denom)[None, :]
    return out


LAST_EXEC_NS = None
LAST_RESULTS = None


def kernel(Q, mask, scale, bias, _trace=False):
    global LAST_EXEC_NS, LAST_RESULTS
    Q = np.ascontiguousarray(np.asarray(Q, dtype=np.float32))
    mask_np = np.asarray(mask).astype(bool).reshape(B, L)
    scale_v = float(np.asarray(scale).reshape(-1)[0])
    bias_v = float(np.asarray(bias).reshape(-1)[0]) if np.asarray(bias).size else float(bias)

    a_scale = 2.0 / scale_v              # folded into q host-side
    b0 = 2.0 / scale_v + bias_v          # activation bias immediate
    poison = -(500.0 + abs(b0)) / a_scale

    if _trace:
        _ensure_ntff_hook()
    nc = _build_program(b0)

    in_maps = []
    for c in range(N_CORES):
        if _STRATEGY == "v2":
            in_maps.append(_prep_core_inputs_v2(Q, c // 2, c % 2, a_scale))
        else:
            in_maps.append(_prep_core_inputs(Q, c // 2, c % 2, a_scale, poison))

    res = bass_utils.run_bass_kernel_spmd(
        nc, in_maps, core_ids=list(range(N_CORES)), trace=_trace
    )
    LAST_EXEC_NS = res.exec_time_ns
    LAST_RESULTS = res

    out = np.empty((B, L, D), np.float32)
    for c in range(N_CORES):
        o = res.results[c]["out"]
        if _STRATEGY == "v2":
            # [128, 16*64] p-major -> [2048, 64]
            o = o.reshape(QT, NT_CORE, D).transpose(1, 0, 2).reshape(NQ_CORE, D)
        b, half = c // 2, c % 2
        for s, g in enumerate(HALF_GROUPS[half]):
            out[b, g * SLOT_Q:(g + 1) * SLOT_Q, :] = o[s * SLOT_Q:(s + 1) * SLOT_Q, :]

    if mask_np.any():
        out = _mask_fixup(out, Q, mask_np, scale_v, bias_v)
    return out



# revision 26
# speedup vs baseline: 1.0061x; 1.0061x over previous
"""Trainium2 Bass kernel for causal Lorentz self-attention.

Problem: B=4, L=4096, D=64 single-head self-attention where
  scores = (2 + 2*<q,k>_L) / scale + bias   (Lorentz inner product)
  causal mask (strict upper triangle) + per-query pad-mask
  attn = softmax(scores);  mu = attn @ v
  out = mu / sqrt(max(|<mu,mu>_L|, eps))

Key algebraic fact used: the softmax denominator cancels in the final
normalization (out = mu_raw / sqrt(|<mu_raw,mu_raw>_L|)), so no row-sum
is computed on device.

Sharding: 2 cores per batch. Each core runs an IDENTICAL static program of
4 "slots" (512 queries each) with static k-extents (8,16,24,32) steps of 128
keys. Which query tiles a slot owns, and where the causal boundary falls, is
encoded purely in host-prepared per-core input data:
  - k iterated DESCENDING from the diagonal, so the 4 boundary steps are
    always steps 0..3 of a slot (static affine_select masks),
  - slots whose causal extent is shorter than the static extent get
    "poison" K columns (huge negative score -> exp underflows to 0) and
    zero V rows.

Precision strategy "split" (default): all matmuls run in bf16 with hi/lo
decomposition (x = bf16(x) + bf16(x - bf16(x)), ~2^-17 operand precision):
  - scores: lhsT = [K_hi; K_lo] stacked on the contraction dim (K=128)
    against Q_hi replicated, plus a K=64 correction matmul K_hi x Q_lo.
  - attn@V: lhsT = [V_hi | V_lo] stacked on the output dim (M=128),
    two moving passes with P_hi and P_lo; the hi/lo output halves are
    summed once per slot (linearity lets them accumulate separately).
Strategy "f32" is the exact-fp32 fallback (4x slower matmuls).
"""

import os
import numpy as np
import ml_dtypes

import concourse.bass as bass
import concourse.bacc as bacc
import concourse.tile as tile
from concourse import mybir
from concourse import masks as cmasks
from concourse import bass_utils
from concourse._compat import with_exitstack
from contextlib import ExitStack

B, L, D = 4, 4096, 64
EPS = 1e-8
N_CORES = 8
QT = 128                       # queries per q-tile / keys per k-step
SLOT_Q = 512                   # queries per slot (4 q-tiles)
SLOTS = 4                      # slots per core
NQ_CORE = SLOTS * SLOT_Q       # 2048 queries per core
NT_CORE = NQ_CORE // QT        # 16 q-tiles per core
SLOT_EXTENTS = (8, 16, 24, 32)  # static k-steps per slot
TOTAL_STEPS = sum(SLOT_EXTENTS)  # 80
# groups of 4 consecutive q-tiles; group g covers q-tiles 4g..4g+3 and needs
# 4g+4 k-tiles. Half 0 gets groups (0,3,4,7) -> extents (4,16,20,32), half 1
# gets (1,2,5,6) -> (8,12,24,28); both fit elementwise under SLOT_EXTENTS.
HALF_GROUPS = ((0, 3, 4, 7), (1, 2, 5, 6))

_F32 = mybir.dt.float32
_BF16 = mybir.dt.bfloat16
_FP16 = mybir.dt.float16
_U16 = mybir.dt.uint16
_BF16_NP = ml_dtypes.bfloat16
_LOG2E = 1.4426950408889634
_SCHRAUDOLPH_A = 1024.0 * _LOG2E          # fp16-bits slope
_SCHRAUDOLPH_B0 = 1024.0 * 15.0 - 44.0    # fp16-bits intercept (C=44 minimax)
_ACT_COLS = 640                           # exp cols on ACT; rest on DVE
# strategy:
#   "k66"   - exploit Lorentz structure: time component (the only large
#             score term) as bf16 hi/lo cross-terms, spatial components as
#             single bf16 -> ONE K=66 score matmul. P fp16 from ACT, V fp16
#             hi/lo stack. 2 MMs/step total.
#   "mixed" - bf16 hi/lo pairs for K/Q, P fp16, V fp16 stack. 3 MMs/step.
#   "fp16"  - fp16 hi/lo pairs for K/Q, P fp16, V fp16 stack. 3 MMs/step,
#             but fp16 matmuls are half-rate on PE.
#   "split" - bf16 hi/lo everywhere incl. P (4 MMs/step + DVE splits)
#   "f32"   - exact fp32 fallback (4x slower matmuls)
_STRATEGY = os.environ.get("KERNEL_MM_DT", "v2")
_KSTACK = 66  # rows: [-k0h, -k0l, -k0h, k_space(63)] x [q0h, q0h, q0l, q_space]

_cache = {}


def _ensure_ntff_hook():
    """The agent image lacks ``antenv.axon_hooks``; synthesize it using the
    ctypes NTFF driver from trn_agent_boot so trace=True works."""
    import sys as _sys
    if "antenv.axon_hooks" in _sys.modules:
        return
    try:
        import types as _types
        import antenv  # noqa: F401
        from trn_agent_boot.trn_boot import _ntff_profile_via_ctypes
        hook = _ntff_profile_via_ctypes("/opt/axon/libaxon_pjrt.so")
        m = _types.ModuleType("antenv.axon_hooks")
        m.get_axon_ntff_profile_hook = lambda: hook
        m.set_axon_ntff_profile_hook = lambda h: None
        _sys.modules["antenv.axon_hooks"] = m
    except Exception:
        pass


@with_exitstack
def _body_v2(ctx: ExitStack, tc, aps, bias_val):
    """Fully SBUF-resident K/V/Q, K=66 Lorentz-structured score matmul,
    single-fp16 V. Exp of each 2-step round is split between ACT (true exp,
    cols 0:ACT_COLS) and DVE (Schraudolph fp16-bits exp, rest). Diagonal
    (masked) steps sit at slot END (pads first) so slot starts never stall;
    rounds stream across slot boundaries with a 1-round skew; epilogues are
    spread in small chunks between rounds. rsqrt via DVE bit trick + one
    Newton step keeps the ACT Exp table resident the whole kernel."""
    nc = tc.nc
    PSUM = bass.MemorySpace.PSUM

    const = ctx.enter_context(tc.tile_pool(name="const", bufs=1))
    datap = ctx.enter_context(tc.tile_pool(name="datap", bufs=1))
    expp = ctx.enter_context(tc.tile_pool(name="expp", bufs=3))
    stp = ctx.enter_context(tc.tile_pool(name="stp", bufs=3, space=PSUM))
    mup = ctx.enter_context(tc.tile_pool(name="mup", bufs=2, space=PSUM))
    sbp = ctx.enter_context(tc.tile_pool(name="sbp", bufs=1))
    smallp = ctx.enter_context(tc.tile_pool(name="smallp", bufs=4))
    outp = ctx.enter_context(tc.tile_pool(name="outp", bufs=1))

    ident = const.tile([64, 64], _F32)
    cmasks.make_identity(nc, ident[:])
    bias_t = const.tile([128, 1], _F32)
    nc.vector.memset(bias_t[:], float(bias_val))
    # DVE Schraudolph intercept: bits = A*(ps + b) + B0 = A*ps + (B0 + A*b)
    dve_b = _SCHRAUDOLPH_B0 + _SCHRAUDOLPH_A * float(bias_val)

    # ---- bulk preloads; first two triggers cover slot 0's working set ----
    slot_base = [sum(SLOT_EXTENTS[:i]) for i in range(SLOTS + 1)]
    kd_sb = datap.tile([_KSTACK, TOTAL_STEPS * QT], _BF16)
    nc.sync.dma_start(kd_sb[:, 0:slot_base[1] * QT],
                      aps["kd66"][:, 0:slot_base[1] * QT])
    qd_sb = datap.tile([_KSTACK, NQ_CORE], _BF16)
    nc.sync.dma_start(qd_sb[:, 0:SLOT_Q], aps["qd66"][:, 0:SLOT_Q])
    vn_sb = datap.tile([QT, TOTAL_STEPS * D], _FP16)
    nc.sync.dma_start(vn_sb[:, 0:slot_base[1] * D],
                      aps["vn"][:, 0:slot_base[1] * D])
    nc.sync.dma_start(qd_sb[:, SLOT_Q:], aps["qd66"][:, SLOT_Q:])
    for s0 in range(1, SLOTS):
        nc.sync.dma_start(
            kd_sb[:, slot_base[s0] * QT:slot_base[s0 + 1] * QT],
            aps["kd66"][:, slot_base[s0] * QT:slot_base[s0 + 1] * QT],
        )
        nc.sync.dma_start(
            vn_sb[:, slot_base[s0] * D:slot_base[s0 + 1] * D],
            aps["vn"][:, slot_base[s0] * D:slot_base[s0 + 1] * D],
        )

    # ---- PE warm-up: short matmuls ramp the clock during the DMA fill ----
    wsrc = const.tile([QT, SLOT_Q], _BF16)
    nc.vector.memset(wsrc[:], 0.0)
    n_warm = int(os.environ.get("KERNEL_N_WARM", "6"))
    for w in range(n_warm):
        wps = stp.tile([QT, 2 * SLOT_Q], _F32, tag="ps", name=f"warm{w}")
        nc.tensor.matmul(wps[:, 0:QT], lhsT=wsrc[:, 0:QT],
                         rhs=wsrc[:, 0:QT], start=True, stop=True)

    mu_all = sbp.tile([64, NQ_CORE], _F32)
    muq_all = sbp.tile([128, NT_CORE * D], _F32)
    ln_all = sbp.tile([128, NT_CORE], _F32)
    invs_all = sbp.tile([128, NT_CORE], _F32)
    out_sb = outp.tile([128, NT_CORE * D], _F32)

    def score_round(s, r):
        ps = stp.tile([QT, 2 * SLOT_Q], _F32)
        qblk = qd_sb[:, s * SLOT_Q:(s + 1) * SLOT_Q]
        for h in (0, 1):
            st = slot_base[s] + 2 * r + h
            nc.tensor.matmul(
                ps[:, h * SLOT_Q:(h + 1) * SLOT_Q],
                lhsT=kd_sb[:, st * QT:(st + 1) * QT],
                rhs=qblk,
                start=True, stop=True,
            )
        return ps

    def exp_round(s, r, ps):
        ext = SLOT_EXTENTS[s]
        et = expp.tile([QT, 2 * SLOT_Q], _FP16)
        # step h=0 -> ACT true exp, step h=1 -> DVE Schraudolph bits exp.
        # Diagonal (masked) steps: columns q' < 128*m are fully above the
        # diagonal — skip their exp; the affine_select zero-fills them.
        m0 = 2 * r - (ext - 4)
        if m0 < 0:
            # unmasked round: ACT takes step 0 plus 128 cols of step 1
            act_lo, act_hi = 0, _ACT_COLS
            dve_lo = _ACT_COLS
        else:
            act_lo, act_hi = QT * m0, SLOT_Q
            dve_lo = SLOT_Q + QT * (m0 + 1)
        nc.scalar.activation(
            et[:, act_lo:act_hi], ps[:, act_lo:act_hi],
            mybir.ActivationFunctionType.Exp,
            bias=bias_t[:], scale=1.0,
        )
        nc.vector.tensor_scalar(
            out=et[:, dve_lo:].bitcast(_U16),
            in0=ps[:, dve_lo:],
            scalar1=_SCHRAUDOLPH_A,
            scalar2=dve_b,
            op0=mybir.AluOpType.mult,
            op1=mybir.AluOpType.add,
        )
        for h in (0, 1):
            m = 2 * r + h - (ext - 4)
            if m >= 0:
                nc.gpsimd.affine_select(
                    out=et[:, h * SLOT_Q:(h + 1) * SLOT_Q],
                    in_=et[:, h * SLOT_Q:(h + 1) * SLOT_Q],
                    compare_op=mybir.AluOpType.is_ge,
                    fill=0.0,
                    base=-QT * m,
                    pattern=[[1, SLOT_Q]],
                    channel_multiplier=-1,
                )
        return et

    mu_tiles = {}

    def av_round(s, r, et):
        ext = SLOT_EXTENTS[s]
        if r == 0:
            # [128, 512] tile: AV accumulates mu into rows 0:64; after the
            # drain, the same bank is reused for the epilogue transposes.
            mu_tiles[s] = mup.tile([QT, SLOT_Q], _F32, tag="mu", name=f"mu{s}")
        mu_ps = mu_tiles[s]
        for h in (0, 1):
            st = slot_base[s] + 2 * r + h
            nc.tensor.matmul(
                mu_ps[0:64, :],
                lhsT=vn_sb[:, st * D:(st + 1) * D],
                rhs=et[:, h * SLOT_Q:(h + 1) * SLOT_Q],
                start=(r == 0 and h == 0),
                stop=(2 * r + h == ext - 1),
            )
        if 2 * r + 1 == ext - 1:
            # drain mu to SBUF on ACT (it has the most slack)
            nc.scalar.copy(
                mu_all[:, s * SLOT_Q:(s + 1) * SLOT_Q], mu_ps[0:64, :]
            )

    def epi_lnorm(sp, q):
        """|l| = 2*mu0^2 - sum(mu_d^2) for one q-tile (l is always < 0)."""
        qt_i = sp * 4 + q
        muq = muq_all[:, qt_i * D:(qt_i + 1) * D]
        sq = smallp.tile([QT, D], _F32)
        nc.vector.tensor_mul(sq[:], muq, muq)
        red = smallp.tile([QT, 1], _F32)
        nc.vector.reduce_sum(red[:], sq[:], axis=mybir.AxisListType.X)
        nc.vector.scalar_tensor_tensor(
            out=ln_all[:, qt_i:qt_i + 1],
            in0=sq[:, 0:1],
            scalar=2.0,
            in1=red[:],
            op0=mybir.AluOpType.mult,
            op1=mybir.AluOpType.subtract,
        )

    def epi_invs(sp):
        """invs = 1/sqrt(ln) on DVE: quake-style bit seed + 1 Newton step
        (avoids any ACT table switch away from Exp)."""
        x = ln_all[:, sp * 4:(sp + 1) * 4]
        xb = smallp.tile([128, 4], _F32, tag="xb")
        nc.vector.tensor_copy(xb[:], x.bitcast(mybir.dt.uint32))
        y0 = smallp.tile([128, 4], _F32, tag="y0")
        nc.vector.tensor_scalar(
            out=y0[:].bitcast(mybir.dt.int32), in0=xb[:],
            scalar1=-0.5, scalar2=float(0x5F3759DF),
            op0=mybir.AluOpType.mult, op1=mybir.AluOpType.add,
        )
        t = smallp.tile([128, 4], _F32, tag="t")
        nc.vector.tensor_mul(t[:], y0[:], y0[:])
        nc.vector.tensor_mul(t[:], t[:], x)
        nc.vector.tensor_scalar(
            out=t[:], in0=t[:], scalar1=-0.5, scalar2=1.5,
            op0=mybir.AluOpType.mult, op1=mybir.AluOpType.add,
        )
        nc.vector.tensor_mul(invs_all[:, sp * 4:(sp + 1) * 4], y0[:], t[:])

    def epi_scale(sp):
        for q in range(4):
            qt_i = sp * 4 + q
            nc.vector.tensor_scalar_mul(
                out_sb[:, qt_i * D:(qt_i + 1) * D],
                muq_all[:, qt_i * D:(qt_i + 1) * D],
                invs_all[:, qt_i:qt_i + 1],
            )
        nc.sync.dma_start(
            aps["out"][:, sp * 256:(sp + 1) * 256],
            out_sb[:, sp * 256:(sp + 1) * 256],
        )

    def epilogue_piece(sp, r):
        """Spread one slot's tail over rounds r=0..5 of the next slot."""
        if r <= 1:
            for q in (2 * r, 2 * r + 1):  # transposes into drained mu bank
                qt_i = sp * 4 + q
                nc.tensor.transpose(
                    mu_tiles[sp][:, q * 64:(q + 1) * 64],
                    mu_all[:, qt_i * QT:(qt_i + 1) * QT], ident[:],
                )
        elif r == 2:
            # all 4 tp's -> SBUF in one DVE copy
            nc.vector.tensor_copy(
                muq_all[:, sp * 256:(sp + 1) * 256], mu_tiles[sp][:, 0:256]
            )
        elif r == 3:
            epi_lnorm(sp, 0)
            epi_lnorm(sp, 1)
        elif r == 4:
            epi_lnorm(sp, 2)
            epi_lnorm(sp, 3)
            epi_invs(sp)
        elif r == 5:
            epi_scale(sp)

    # ---- flat pipelined stream over all rounds, 2-round AV skew ----
    rounds = [(s, r) for s in range(SLOTS) for r in range(SLOT_EXTENTS[s] // 2)]
    pend = []
    for s, r in rounds:
        ps = score_round(s, r)
        if len(pend) >= 2:
            av_round(*pend.pop(0))
        if s > 0 and 2 <= r <= 7:
            epilogue_piece(s - 1, r - 2)
        et = exp_round(s, r, ps)
        pend.append((s, r, et))
    for p in pend:
        av_round(*p)
    for r in range(6):
        epilogue_piece(SLOTS - 1, r)


@with_exitstack
def _body_k66(ctx: ExitStack, tc, aps, bias_val):
    """Single K=66 score matmul per step (Lorentz-structured hi/lo),
    fp16 P/V attention matmul. Skewed pipeline."""
    nc = tc.nc
    PSUM = bass.MemorySpace.PSUM

    const = ctx.enter_context(tc.tile_pool(name="const", bufs=1))
    qdp = ctx.enter_context(tc.tile_pool(name="qdp", bufs=1))
    kdp = ctx.enter_context(tc.tile_pool(name="kdp", bufs=4))
    vnp = ctx.enter_context(tc.tile_pool(name="vnp", bufs=4))
    expp = ctx.enter_context(tc.tile_pool(name="expp", bufs=3))
    stp = ctx.enter_context(tc.tile_pool(name="stp", bufs=3, space=PSUM))
    mup = ctx.enter_context(tc.tile_pool(name="mup", bufs=2, space=PSUM))
    tpp = ctx.enter_context(tc.tile_pool(name="tpp", bufs=2, space=PSUM))
    sbp = ctx.enter_context(tc.tile_pool(name="sbp", bufs=1))
    smallp = ctx.enter_context(tc.tile_pool(name="smallp", bufs=4))
    outp = ctx.enter_context(tc.tile_pool(name="outp", bufs=3))

    ident = const.tile([64, 64], _F32)
    cmasks.make_identity(nc, ident[:])
    bias_t = const.tile([128, 1], _F32)
    nc.vector.memset(bias_t[:], float(bias_val))

    qd_sb = qdp.tile([_KSTACK, NQ_CORE], _BF16)
    for s0 in range(SLOTS):
        c0, c1 = s0 * SLOT_Q, (s0 + 1) * SLOT_Q
        nc.sync.dma_start(qd_sb[:, c0:c1], aps["qd66"][:, c0:c1])

    mu_sb = sbp.tile([64, NQ_CORE], _F32)
    muq_all = sbp.tile([128, NT_CORE * D], _F32)
    ln_all = sbp.tile([128, NT_CORE], _F32)

    step_base = 0
    for s in range(SLOTS):
        ext = SLOT_EXTENTS[s]
        q_lo = s * SLOT_Q
        mu_ps = mup.tile([QT, SLOT_Q], _F32)  # rows 0-63 hi, 64-127 lo

        def st_step(i):
            st = step_base + i
            kt = kdp.tile([_KSTACK, QT], _BF16)
            nc.sync.dma_start(kt[:], aps["kd66"][:, st * QT:(st + 1) * QT])
            ps = stp.tile([QT, SLOT_Q], _F32)
            nc.tensor.matmul(
                ps[:], lhsT=kt[:], rhs=qd_sb[:, q_lo:q_lo + SLOT_Q],
                start=True, stop=True,
            )
            return ps

        def av_step(i, ps):
            st = step_base + i
            vt = vnp.tile([QT, QT], _FP16)
            nc.sync.dma_start(vt[:], aps["vn"][st * QT:(st + 1) * QT, :])
            et = expp.tile([QT, SLOT_Q], _FP16)
            nc.scalar.activation(
                et[:], ps[:], mybir.ActivationFunctionType.Exp,
                bias=bias_t[:], scale=1.0,
            )
            if i < 4:
                nc.gpsimd.affine_select(
                    out=et[:], in_=et[:],
                    compare_op=mybir.AluOpType.is_ge,
                    fill=0.0,
                    base=-QT * (3 - i),
                    pattern=[[1, SLOT_Q]],
                    channel_multiplier=-1,
                )
            nc.tensor.matmul(
                mu_ps[:], lhsT=vt[:], rhs=et[:],
                start=(i == 0), stop=(i == ext - 1),
            )

        ps_prev = st_step(0)
        for i in range(1, ext):
            ps_i = st_step(i)
            av_step(i - 1, ps_prev)
            ps_prev = ps_i
        av_step(ext - 1, ps_prev)
        step_base += ext

        lo_sb = smallp.tile([64, SLOT_Q], _F32, tag="losb")
        nc.scalar.copy(lo_sb[:], mu_ps[64:128, :])
        nc.vector.tensor_add(mu_sb[:, q_lo:q_lo + SLOT_Q], mu_ps[0:64, :], lo_sb[:])

        for q in range(SLOT_Q // QT):
            qt_i = s * (SLOT_Q // QT) + q
            tp = tpp.tile([QT, 64], _F32)
            nc.tensor.transpose(
                tp[:], mu_sb[:, qt_i * QT:(qt_i + 1) * QT], ident[:]
            )
            muq = muq_all[:, qt_i * D:(qt_i + 1) * D]
            nc.scalar.copy(muq, tp[:, :D])
            sq = smallp.tile([QT, D], _F32)
            nc.vector.tensor_mul(sq[:], muq, muq)
            red = smallp.tile([QT, 1], _F32)
            nc.vector.reduce_sum(red[:], sq[:], axis=mybir.AxisListType.X)
            nc.vector.scalar_tensor_tensor(
                out=ln_all[:, qt_i:qt_i + 1],
                in0=sq[:, 0:1],
                scalar=2.0,
                in1=red[:],
                op0=mybir.AluOpType.mult,
                op1=mybir.AluOpType.subtract,
            )

        # per-slot normalize: 1/sqrt(x) = exp(-0.5*ln(x)); Ln and Exp share
        # one ACT table set, so no table switch and no end-of-kernel phase.
        lns = ln_all[:, s * 4:(s + 1) * 4]
        lnt = smallp.tile([128, 4], _F32, tag="lnt")
        nc.scalar.activation(lnt[:], lns, mybir.ActivationFunctionType.Ln)
        invs = smallp.tile([128, 4], _F32, tag="invs")
        nc.scalar.activation(
            invs[:], lnt[:], mybir.ActivationFunctionType.Exp,
            bias=0.0, scale=-0.5,
        )
        for q in range(SLOT_Q // QT):
            qt_i = s * (SLOT_Q // QT) + q
            ot = outp.tile([QT, D], _F32)
            nc.vector.tensor_scalar_mul(
                ot[:], muq_all[:, qt_i * D:(qt_i + 1) * D], invs[:, q:q + 1]
            )
            nc.sync.dma_start(aps["out"][qt_i * QT:(qt_i + 1) * QT, :], ot[:])


@with_exitstack
def _body_fp16(ctx: ExitStack, tc, aps, bias_val, kq_dt=_FP16):
    """hi/lo-pair strategy with software-pipelined (skewed) step loop and
    per-slot preloaded K/V (per-step DMA triggers serialize on the sync
    sequencer at ~590ns each, so they must be batched).
    kq_dt: dtype of the K/Q score operands (bf16 = PE full rate)."""
    nc = tc.nc
    PSUM = bass.MemorySpace.PSUM

    const = ctx.enter_context(tc.tile_pool(name="const", bufs=1))
    qdp = ctx.enter_context(tc.tile_pool(name="qdp", bufs=1))
    kdp = ctx.enter_context(tc.tile_pool(name="kdp", bufs=1))
    vnp = ctx.enter_context(tc.tile_pool(name="vnp", bufs=1))
    expp = ctx.enter_context(tc.tile_pool(name="expp", bufs=3))
    stp = ctx.enter_context(tc.tile_pool(name="stp", bufs=3, space=PSUM))
    mup = ctx.enter_context(tc.tile_pool(name="mup", bufs=2, space=PSUM))
    tpp = ctx.enter_context(tc.tile_pool(name="tpp", bufs=2, space=PSUM))
    sbp = ctx.enter_context(tc.tile_pool(name="sbp", bufs=1))
    smallp = ctx.enter_context(tc.tile_pool(name="smallp", bufs=4))
    outp = ctx.enter_context(tc.tile_pool(name="outp", bufs=3))

    ident = const.tile([64, 64], _F32)
    cmasks.make_identity(nc, ident[:])
    bias_t = const.tile([128, 1], _F32)
    nc.vector.memset(bias_t[:], float(bias_val))

    # PE warm-up: ~16 dummy matmuls during the initial DMA window so the
    # HAM clock-gate reaches 2.4 GHz before the first real matmul.
    wsrc = const.tile([QT, SLOT_Q], kq_dt)
    nc.gpsimd.memset(wsrc[:], 0.0)
    for w in range(16):
        wps = tpp.tile([QT, SLOT_Q], _F32, tag="warm", bufs=1)
        nc.tensor.matmul(wps[:], lhsT=wsrc[:, 0:QT], rhs=wsrc[:],
                         start=True, stop=True)

    # causal boundary masks (fp16 ones/zeros incl. diagonal triangle),
    # applied with a DVE multiply instead of a gpsimd affine_select on the
    # exp->AV critical path.
    bmask = const.tile([QT, 4, SLOT_Q], _FP16)
    nc.vector.memset(bmask[:], 1.0)
    for i in range(4):
        nc.gpsimd.affine_select(
            out=bmask[:, i, :], in_=bmask[:, i, :],
            compare_op=mybir.AluOpType.is_ge,
            fill=0.0,
            base=-QT * (3 - i),
            pattern=[[1, SLOT_Q]],
            channel_multiplier=-1,
        )

    qdh_sb = qdp.tile([128, NQ_CORE], kq_dt)
    qdl_sb = qdp.tile([64, NQ_CORE], kq_dt)
    kd_sb = {}
    vn_sb = {}
    base = 0
    for s0 in range(SLOTS):
        ext = SLOT_EXTENTS[s0]
        c0, c1 = s0 * SLOT_Q, (s0 + 1) * SLOT_Q
        nc.sync.dma_start(qdh_sb[:, c0:c1], aps["qdh"][:, c0:c1])
        nc.sync.dma_start(qdl_sb[:, c0:c1], aps["qdl"][:, c0:c1])
        kd_sb[s0] = kdp.tile([QT, ext * QT], kq_dt, tag=f"kd{s0}", name=f"kd_sb{s0}")
        nc.sync.dma_start(kd_sb[s0][:], aps["kd"][:, base * QT:(base + ext) * QT])
        vn_sb[s0] = vnp.tile([QT, ext, QT], _FP16, tag=f"vn{s0}", name=f"vn_sb{s0}")
        vsrc = aps["vn"][base * QT:(base + ext) * QT, :].rearrange(
            "(t p) c -> p t c", p=QT)
        nc.sync.dma_start(vn_sb[s0][:], vsrc)
        base += ext

    mu_sb = sbp.tile([64, NQ_CORE], _F32)
    muq_all = sbp.tile([128, NT_CORE * D], _F32)
    ln_all = sbp.tile([128, NT_CORE], _F32)

    step_base = 0
    for s in range(SLOTS):
        ext = SLOT_EXTENTS[s]
        q_lo = s * SLOT_Q
        mu_ps = mup.tile([QT, SLOT_Q], _F32)  # rows 0-63 hi, 64-127 lo

        def st_step(i):
            kt = kd_sb[s][:, i * QT:(i + 1) * QT]
            ps = stp.tile([QT, SLOT_Q], _F32)
            nc.tensor.matmul(
                ps[:], lhsT=kt, rhs=qdh_sb[:, q_lo:q_lo + SLOT_Q],
                start=True, stop=False,
            )
            nc.tensor.matmul(
                ps[:], lhsT=kt[0:64, :], rhs=qdl_sb[:, q_lo:q_lo + SLOT_Q],
                start=False, stop=True,
            )
            return ps

        def av_step(i, ps):
            vt = vn_sb[s][:, i, :]
            et = expp.tile([QT, SLOT_Q], _FP16)
            nc.scalar.activation(
                et[:], ps[:], mybir.ActivationFunctionType.Exp,
                bias=bias_t[:], scale=1.0,
            )
            if i < 4:
                # step i's k-tile is the (3-i)'th q-tile block's diagonal
                nc.vector.tensor_mul(et[:], et[:], bmask[:, i, :])
            nc.tensor.matmul(
                mu_ps[:], lhsT=vt, rhs=et[:],
                start=(i == 0), stop=(i == ext - 1),
            )

        # 2-deep skewed pipeline: S_T(i+2) runs on PE before AV(i), covering
        # the exp latency (and the boundary-mask multiply) on ACT/DVE.
        pending = [st_step(0), st_step(1)]
        for i in range(2, ext):
            pending.append(st_step(i))
            av_step(i - 2, pending.pop(0))
        av_step(ext - 2, pending.pop(0))
        av_step(ext - 1, pending.pop(0))
        step_base += ext

        # mu = hi half + lo half (one PSUM operand max per DVE op)
        lo_sb = smallp.tile([64, SLOT_Q], _F32, tag="losb")
        nc.scalar.copy(lo_sb[:], mu_ps[64:128, :])
        nc.vector.tensor_add(mu_sb[:, q_lo:q_lo + SLOT_Q], mu_ps[0:64, :], lo_sb[:])

        for q in range(SLOT_Q // QT):
            qt_i = s * (SLOT_Q // QT) + q
            tp = tpp.tile([QT, 64], _F32)
            nc.tensor.transpose(
                tp[:], mu_sb[:, qt_i * QT:(qt_i + 1) * QT], ident[:]
            )
            muq = muq_all[:, qt_i * D:(qt_i + 1) * D]
            nc.scalar.copy(muq, tp[:, :D])
            sq = smallp.tile([QT, D], _F32)
            nc.vector.tensor_mul(sq[:], muq, muq)
            red = smallp.tile([QT, 1], _F32)
            nc.vector.reduce_sum(red[:], sq[:], axis=mybir.AxisListType.X)
            # |l| = -l = 2*mu0^2 - sum(mu_d^2)  (l is always < 0 here)
            nc.vector.scalar_tensor_tensor(
                out=ln_all[:, qt_i:qt_i + 1],
                in0=sq[:, 0:1],
                scalar=2.0,
                in1=red[:],
                op0=mybir.AluOpType.mult,
                op1=mybir.AluOpType.subtract,
            )

        # per-slot normalize: 1/sqrt(x) = exp(-0.5*ln(x)); Ln and Exp share
        # one ACT table set, so no table switch and no end-of-kernel phase.
        lns = ln_all[:, s * 4:(s + 1) * 4]
        lnt = smallp.tile([128, 4], _F32, tag="lnt")
        nc.scalar.activation(lnt[:], lns, mybir.ActivationFunctionType.Ln)
        invs = smallp.tile([128, 4], _F32, tag="invs")
        nc.scalar.activation(
            invs[:], lnt[:], mybir.ActivationFunctionType.Exp,
            bias=0.0, scale=-0.5,
        )
        for q in range(SLOT_Q // QT):
            qt_i = s * (SLOT_Q // QT) + q
            ot = outp.tile([QT, D], _F32)
            nc.vector.tensor_scalar_mul(
                ot[:], muq_all[:, qt_i * D:(qt_i + 1) * D], invs[:, q:q + 1]
            )
            nc.sync.dma_start(aps["out"][qt_i * QT:(qt_i + 1) * QT, :], ot[:])


@with_exitstack
def _body_split(ctx: ExitStack, tc, aps, bias_val):
    """bf16 hi/lo strategy. aps: dict of DRAM APs."""
    nc = tc.nc
    PSUM = bass.MemorySpace.PSUM

    const = ctx.enter_context(tc.tile_pool(name="const", bufs=1))
    qdp = ctx.enter_context(tc.tile_pool(name="qdp", bufs=1))
    kdp = ctx.enter_context(tc.tile_pool(name="kdp", bufs=4))
    vnp = ctx.enter_context(tc.tile_pool(name="vnp", bufs=4))
    expp = ctx.enter_context(tc.tile_pool(name="expp", bufs=3))
    ehp = ctx.enter_context(tc.tile_pool(name="ehp", bufs=3))
    elp = ctx.enter_context(tc.tile_pool(name="elp", bufs=3))
    stp = ctx.enter_context(tc.tile_pool(name="stp", bufs=2, space=PSUM))
    mup = ctx.enter_context(tc.tile_pool(name="mup", bufs=2, space=PSUM))
    tpp = ctx.enter_context(tc.tile_pool(name="tpp", bufs=2, space=PSUM))
    sbp = ctx.enter_context(tc.tile_pool(name="sbp", bufs=1))
    smallp = ctx.enter_context(tc.tile_pool(name="smallp", bufs=4))
    outp = ctx.enter_context(tc.tile_pool(name="outp", bufs=3))

    ident = const.tile([64, 64], _F32)
    cmasks.make_identity(nc, ident[:])
    bias_t = const.tile([128, 1], _F32)
    nc.vector.memset(bias_t[:], float(bias_val))

    qdh_sb = qdp.tile([128, NQ_CORE], _BF16)
    nc.sync.dma_start(qdh_sb[:], aps["qdh"][:])
    qdl_sb = qdp.tile([64, NQ_CORE], _BF16)
    nc.sync.dma_start(qdl_sb[:], aps["qdl"][:])

    mu_sb = sbp.tile([64, NQ_CORE], _F32)
    muq_all = sbp.tile([128, NT_CORE * D], _F32)
    ln_all = sbp.tile([128, NT_CORE], _F32)

    step_base = 0
    for s in range(SLOTS):
        ext = SLOT_EXTENTS[s]
        q_lo = s * SLOT_Q
        mu_ps = mup.tile([QT, SLOT_Q], _F32)  # rows 0-63 hi, 64-127 lo
        for i in range(ext):
            st = step_base + i
            kt = kdp.tile([QT, QT], _BF16)
            nc.sync.dma_start(kt[:], aps["kd"][:, st * QT:(st + 1) * QT])
            vt = vnp.tile([QT, QT], _BF16)
            nc.sync.dma_start(vt[:], aps["vn"][st * QT:(st + 1) * QT, :])

            ps = stp.tile([QT, SLOT_Q], _F32)
            nc.tensor.matmul(
                ps[:], lhsT=kt[:], rhs=qdh_sb[:, q_lo:q_lo + SLOT_Q],
                start=True, stop=False,
            )
            nc.tensor.matmul(
                ps[:], lhsT=kt[0:64, :], rhs=qdl_sb[:, q_lo:q_lo + SLOT_Q],
                start=False, stop=True,
            )
            et = expp.tile([QT, SLOT_Q], _F32)
            nc.scalar.activation(
                et[:], ps[:], mybir.ActivationFunctionType.Exp,
                bias=bias_t[:], scale=1.0,
            )
            if i < 4:
                # step i's k-tile is the (3-i)'th q-tile block's diagonal:
                # keep element (k, q) iff q - k - 128*(3-i) >= 0
                nc.gpsimd.affine_select(
                    out=et[:], in_=et[:],
                    compare_op=mybir.AluOpType.is_ge,
                    fill=0.0,
                    base=-QT * (3 - i),
                    pattern=[[1, SLOT_Q]],
                    channel_multiplier=-1,
                )
            eth = ehp.tile([QT, SLOT_Q], _BF16)
            nc.vector.tensor_copy(eth[:], et[:])
            etl = elp.tile([QT, SLOT_Q], _BF16)
            nc.vector.tensor_sub(etl[:], et[:], eth[:])
            nc.tensor.matmul(
                mu_ps[:], lhsT=vt[:], rhs=eth[:],
                start=(i == 0), stop=False,
            )
            nc.tensor.matmul(
                mu_ps[:], lhsT=vt[:], rhs=etl[:],
                start=False, stop=(i == ext - 1),
            )
        step_base += ext

        # mu = hi half + lo half (one PSUM operand max per DVE op)
        lo_sb = smallp.tile([64, SLOT_Q], _F32, tag="losb")
        nc.scalar.copy(lo_sb[:], mu_ps[64:128, :])
        nc.vector.tensor_add(mu_sb[:, q_lo:q_lo + SLOT_Q], mu_ps[0:64, :], lo_sb[:])

        for q in range(SLOT_Q // QT):
            qt_i = s * (SLOT_Q // QT) + q
            tp = tpp.tile([QT, 64], _F32)
            nc.tensor.transpose(
                tp[:], mu_sb[:, qt_i * QT:(qt_i + 1) * QT], ident[:]
            )
            muq = muq_all[:, qt_i * D:(qt_i + 1) * D]
            nc.scalar.copy(muq, tp[:, :D])
            sq = smallp.tile([QT, D], _F32)
            nc.vector.tensor_mul(sq[:], muq, muq)
            red = smallp.tile([QT, 1], _F32)
            nc.vector.reduce_sum(red[:], sq[:], axis=mybir.AxisListType.X)
            # |l| = -l = 2*mu0^2 - sum(mu_d^2)  (l is always < 0 here)
            nc.vector.scalar_tensor_tensor(
                out=ln_all[:, qt_i:qt_i + 1],
                in0=sq[:, 0:1],
                scalar=2.0,
                in1=red[:],
                op0=mybir.AluOpType.mult,
                op1=mybir.AluOpType.subtract,
            )

    # grouped sqrt (single ACT table switch) + reciprocal + final scale
    sqv = sbp.tile([128, NT_CORE], _F32)
    nc.scalar.activation(
        sqv[:], ln_all[:], mybir.ActivationFunctionType.Sqrt,
        bias=0.0, scale=1.0,
    )
    inv = sbp.tile([128, NT_CORE], _F32)
    nc.vector.reciprocal(inv[:], sqv[:])
    for qt_i in range(NT_CORE):
        ot = outp.tile([QT, D], _F32)
        nc.vector.tensor_scalar_mul(
            ot[:], muq_all[:, qt_i * D:(qt_i + 1) * D], inv[:, qt_i:qt_i + 1]
        )
        nc.sync.dma_start(aps["out"][qt_i * QT:(qt_i + 1) * QT, :], ot[:])


@with_exitstack
def _body_f32(ctx: ExitStack, tc, aps, bias_val):
    """Exact-fp32 fallback strategy."""
    nc = tc.nc
    PSUM = bass.MemorySpace.PSUM

    const = ctx.enter_context(tc.tile_pool(name="const", bufs=1))
    qdp = ctx.enter_context(tc.tile_pool(name="qdp", bufs=1))
    kdp = ctx.enter_context(tc.tile_pool(name="kdp", bufs=4))
    vnp = ctx.enter_context(tc.tile_pool(name="vnp", bufs=4))
    expp = ctx.enter_context(tc.tile_pool(name="expp", bufs=3))
    stp = ctx.enter_context(tc.tile_pool(name="stp", bufs=2, space=PSUM))
    mup = ctx.enter_context(tc.tile_pool(name="mup", bufs=2, space=PSUM))
    tpp = ctx.enter_context(tc.tile_pool(name="tpp", bufs=2, space=PSUM))
    sbp = ctx.enter_context(tc.tile_pool(name="sbp", bufs=1))
    smallp = ctx.enter_context(tc.tile_pool(name="smallp", bufs=4))
    outp = ctx.enter_context(tc.tile_pool(name="outp", bufs=3))

    ident = const.tile([64, 64], _F32)
    cmasks.make_identity(nc, ident[:])
    bias_t = const.tile([128, 1], _F32)
    nc.vector.memset(bias_t[:], float(bias_val))

    qd_sb = qdp.tile([64, NQ_CORE], _F32)
    nc.sync.dma_start(qd_sb[:], aps["qd"][:])

    mu_sb = sbp.tile([64, NQ_CORE], _F32)
    muq_all = sbp.tile([128, NT_CORE * D], _F32)
    ln_all = sbp.tile([128, NT_CORE], _F32)

    step_base = 0
    for s in range(SLOTS):
        ext = SLOT_EXTENTS[s]
        q_lo = s * SLOT_Q
        mu_ps = mup.tile([64, SLOT_Q], _F32)
        for i in range(ext):
            st = step_base + i
            kt = kdp.tile([64, QT], _F32)
            nc.sync.dma_start(kt[:], aps["kd"][:, st * QT:(st + 1) * QT])
            vt = vnp.tile([QT, D], _F32)
            nc.sync.dma_start(vt[:], aps["vn"][st * QT:(st + 1) * QT, :])

            ps = stp.tile([QT, SLOT_Q], _F32)
            nc.tensor.matmul(
                ps[:], lhsT=kt[:], rhs=qd_sb[:, q_lo:q_lo + SLOT_Q],
                start=True, stop=True,
            )
            et = expp.tile([QT, SLOT_Q], _F32)
            nc.scalar.activation(
                et[:], ps[:], mybir.ActivationFunctionType.Exp,
                bias=bias_t[:], scale=1.0,
            )
            if i < 4:
                nc.gpsimd.affine_select(
                    out=et[:], in_=et[:],
                    compare_op=mybir.AluOpType.is_ge,
                    fill=0.0,
                    base=-QT * (3 - i),
                    pattern=[[1, SLOT_Q]],
                    channel_multiplier=-1,
                )
            nc.tensor.matmul(
                mu_ps[:], lhsT=vt[:], rhs=et[:],
                start=(i == 0), stop=(i == ext - 1),
            )
        step_base += ext

        nc.vector.tensor_copy(mu_sb[:, q_lo:q_lo + SLOT_Q], mu_ps[:])
        for q in range(SLOT_Q // QT):
            qt_i = s * (SLOT_Q // QT) + q
            tp = tpp.tile([QT, 64], _F32)
            nc.tensor.transpose(
                tp[:], mu_sb[:, qt_i * QT:(qt_i + 1) * QT], ident[:]
            )
            muq = muq_all[:, qt_i * D:(qt_i + 1) * D]
            nc.scalar.copy(muq, tp[:, :D])
            sq = smallp.tile([QT, D], _F32)
            nc.vector.tensor_mul(sq[:], muq, muq)
            red = smallp.tile([QT, 1], _F32)
            nc.vector.reduce_sum(red[:], sq[:], axis=mybir.AxisListType.X)
            nc.vector.scalar_tensor_tensor(
                out=ln_all[:, qt_i:qt_i + 1],
                in0=sq[:, 0:1],
                scalar=2.0,
                in1=red[:],
                op0=mybir.AluOpType.mult,
                op1=mybir.AluOpType.subtract,
            )

    sqv = sbp.tile([128, NT_CORE], _F32)
    nc.scalar.activation(
        sqv[:], ln_all[:], mybir.ActivationFunctionType.Sqrt,
        bias=0.0, scale=1.0,
    )
    inv = sbp.tile([128, NT_CORE], _F32)
    nc.vector.reciprocal(inv[:], sqv[:])
    for qt_i in range(NT_CORE):
        ot = outp.tile([QT, D], _F32)
        nc.vector.tensor_scalar_mul(
            ot[:], muq_all[:, qt_i * D:(qt_i + 1) * D], inv[:, qt_i:qt_i + 1]
        )
        nc.sync.dma_start(aps["out"][qt_i * QT:(qt_i + 1) * QT, :], ot[:])


def _build_program(bias_val):
    key = (round(float(bias_val), 12), _STRATEGY)
    if key in _cache:
        return _cache[key]
    nc = bacc.Bacc(
        "TRN2",
        target_bir_lowering=False,
        debug=False,
        enable_asserts=False,
    )
    aps = {}
    if _STRATEGY == "v2":
        aps["qd66"] = nc.dram_tensor("qd66", [_KSTACK, NQ_CORE], _BF16, kind="ExternalInput").ap()
        aps["kd66"] = nc.dram_tensor("kd66", [_KSTACK, TOTAL_STEPS * QT], _BF16, kind="ExternalInput").ap()
        aps["vn"] = nc.dram_tensor("vn", [QT, TOTAL_STEPS * D], _FP16, kind="ExternalInput").ap()
        aps["out"] = nc.dram_tensor("out", [128, NT_CORE * D], _F32, kind="ExternalOutput").ap()
        with tile.TileContext(nc) as tc:
            _body_v2(tc, aps, bias_val)
        nc.compile()
        _cache[key] = nc
        return nc
    if _STRATEGY == "k66":
        aps["qd66"] = nc.dram_tensor("qd66", [_KSTACK, NQ_CORE], _BF16, kind="ExternalInput").ap()
        aps["kd66"] = nc.dram_tensor("kd66", [_KSTACK, TOTAL_STEPS * QT], _BF16, kind="ExternalInput").ap()
        aps["vn"] = nc.dram_tensor("vn", [TOTAL_STEPS * QT, 128], _FP16, kind="ExternalInput").ap()
    elif _STRATEGY in ("split", "fp16", "mixed"):
        kq_dt = _BF16 if _STRATEGY in ("split", "mixed") else _FP16
        pv_dt = _BF16 if _STRATEGY == "split" else _FP16
        aps["qdh"] = nc.dram_tensor("qdh", [128, NQ_CORE], kq_dt, kind="ExternalInput").ap()
        aps["qdl"] = nc.dram_tensor("qdl", [64, NQ_CORE], kq_dt, kind="ExternalInput").ap()
        aps["kd"] = nc.dram_tensor("kd", [128, TOTAL_STEPS * QT], kq_dt, kind="ExternalInput").ap()
        aps["vn"] = nc.dram_tensor("vn", [TOTAL_STEPS * QT, 128], pv_dt, kind="ExternalInput").ap()
    else:
        aps["qd"] = nc.dram_tensor("qd", [64, NQ_CORE], _F32, kind="ExternalInput").ap()
        aps["kd"] = nc.dram_tensor("kd", [64, TOTAL_STEPS * QT], _F32, kind="ExternalInput").ap()
        aps["vn"] = nc.dram_tensor("vn", [TOTAL_STEPS * QT, D], _F32, kind="ExternalInput").ap()
    aps["out"] = nc.dram_tensor("out", [NQ_CORE, D], _F32, kind="ExternalOutput").ap()
    with tile.TileContext(nc) as tc:
        if _STRATEGY == "k66":
            _body_k66(tc, aps, bias_val)
        elif _STRATEGY == "mixed":
            _body_fp16(tc, aps, bias_val, kq_dt=_BF16)
        elif _STRATEGY == "fp16":
            _body_fp16(tc, aps, bias_val, kq_dt=_FP16)
        elif _STRATEGY == "split":
            _body_split(tc, aps, bias_val)
        else:
            _body_f32(tc, aps, bias_val)
    nc.compile()
    _cache[key] = nc
    return nc


def _hilo(x, np_dt):
    hi = x.astype(np_dt)
    lo = (x - hi.astype(np.float32)).astype(np_dt)
    return hi, lo


def _prep_core_inputs_v2(Q, b, half, a_scale):
    """v2 layouts: kd66 as in k66 but pads reuse a real k-tile (scores stay
    in the normal range); vn transposed to [128, steps*64] fp16 with zero
    pads (zero V rows nullify pad steps, so no score poison is needed)."""
    groups = HALF_GROUPS[half]
    Qb = Q[b]  # [L, D]
    qd = np.empty((64, NQ_CORE), np.float32)
    kd = np.empty((64, TOTAL_STEPS * QT), np.float32)
    vn = np.zeros((TOTAL_STEPS, QT, D), np.float32)
    blk0 = Qb[0:QT, :]  # pad k-tile: any real tile keeps scores bounded
    kdb0 = blk0.T.copy()
    kdb0[0, :] = -kdb0[0, :]
    step_base = 0
    for s, g in enumerate(groups):
        ext = SLOT_EXTENTS[s]
        qd[:, s * SLOT_Q:(s + 1) * SLOT_Q] = (
            Qb[g * SLOT_Q:(g + 1) * SLOT_Q, :].T * a_scale
        )
        n_real = 4 * g + 4
        pads = ext - n_real
        # pads FIRST (zero V nullifies them), then k-tiles ascending so the
        # 4 diagonal tiles land at static steps ext-4..ext-1 (mask steps).
        for i in range(ext):
            st = step_base + i
            c0 = st * QT
            if i >= pads:
                j = i - pads  # ascending 0..n_real-1
                blk = Qb[j * QT:(j + 1) * QT, :]
                kdb = blk.T.copy()
                kdb[0, :] = -kdb[0, :]  # Lorentz signature on time component
                kd[:, c0:c0 + QT] = kdb
                vn[st] = blk
            else:
                kd[:, c0:c0 + QT] = kdb0
                # vn stays zero
        step_base += ext
    k0h, k0l = _hilo(kd[0:1], _BF16_NP)
    q0h, q0l = _hilo(qd[0:1], _BF16_NP)
    kd66 = np.empty((_KSTACK, TOTAL_STEPS * QT), _BF16_NP)
    kd66[0] = k0h
    kd66[1] = k0l
    kd66[2] = k0h
    kd66[3:] = kd[1:].astype(_BF16_NP)
    qd66 = np.empty((_KSTACK, NQ_CORE), _BF16_NP)
    qd66[0] = q0h
    qd66[1] = q0h
    qd66[2] = q0l
    qd66[3:] = qd[1:].astype(_BF16_NP)
    # [steps, 128k, 64d] -> [128k, steps*64]
    vn_pm = np.ascontiguousarray(
        vn.transpose(1, 0, 2).reshape(QT, TOTAL_STEPS * D)
    ).astype(np.float16)
    return {"qd66": qd66, "kd66": kd66, "vn": vn_pm}


def _prep_core_inputs(Q, b, half, a_scale, poison):
    """Build per-core input arrays. a_scale folded into q."""
    groups = HALF_GROUPS[half]
    Qb = Q[b]  # [L, D]
    qd = np.empty((64, NQ_CORE), np.float32)
    kd = np.empty((64, TOTAL_STEPS * QT), np.float32)
    vn = np.zeros((TOTAL_STEPS * QT, D), np.float32)
    step_base = 0
    for s, g in enumerate(groups):
        ext = SLOT_EXTENTS[s]
        qd[:, s * SLOT_Q:(s + 1) * SLOT_Q] = (
            Qb[g * SLOT_Q:(g + 1) * SLOT_Q, :].T * a_scale
        )
        n_real = 4 * g + 4  # causal extent of this group in k-tiles
        for i in range(ext):
            c0 = (step_base + i) * QT
            if i < n_real:
                j = 4 * g + 3 - i  # descending from the diagonal
                blk = Qb[j * QT:(j + 1) * QT, :]  # [128, 64]
                kdb = blk.T.copy()
                kdb[0, :] = -kdb[0, :]  # Lorentz signature on time component
                kd[:, c0:c0 + QT] = kdb
                vn[c0:c0 + QT, :] = blk
            else:
                kd[:, c0:c0 + QT] = 0.0
                kd[0, c0:c0 + QT] = poison
                # vn stays zero
        step_base += ext
    if _STRATEGY == "k66":
        # kd rows already carry the Lorentz sign on row 0 (time).
        k0h, k0l = _hilo(kd[0:1], _BF16_NP)      # signed time component
        q0h, q0l = _hilo(qd[0:1], _BF16_NP)
        kd66 = np.empty((_KSTACK, TOTAL_STEPS * QT), _BF16_NP)
        kd66[0] = k0h
        kd66[1] = k0l
        kd66[2] = k0h
        kd66[3:] = kd[1:].astype(_BF16_NP)
        qd66 = np.empty((_KSTACK, NQ_CORE), _BF16_NP)
        qd66[0] = q0h
        qd66[1] = q0h
        qd66[2] = q0l
        qd66[3:] = qd[1:].astype(_BF16_NP)
        vh, vl = _hilo(vn, np.float16)
        vns = np.concatenate([vh, vl], axis=1)   # [steps*128, 128]
        return {"qd66": qd66, "kd66": kd66, "vn": np.ascontiguousarray(vns)}
    if _STRATEGY not in ("split", "fp16", "mixed"):
        return {"qd": qd, "kd": kd, "vn": vn}
    np_dt = _BF16_NP if _STRATEGY in ("split", "mixed") else np.float16
    pv_np = _BF16_NP if _STRATEGY == "split" else np.float16
    qh, ql = _hilo(qd, np_dt)
    kh, kl = _hilo(kd, np_dt)
    vh, vl = _hilo(vn, pv_np)
    qdh = np.empty((128, NQ_CORE), np_dt)
    qdh[0:64] = qh
    qdh[64:128] = qh  # replicated: both halves of the K-stack see Q_hi
    kds = np.concatenate([kh, kl], axis=0)       # [128, steps*128]
    vns = np.concatenate([vh, vl], axis=1)       # [steps*128, 128]
    return {"qdh": qdh, "qdl": ql, "kd": np.ascontiguousarray(kds),
            "vn": np.ascontiguousarray(vns)}


def _mask_fixup(out, Q, mask, scale_v, bias_v):
    """Reference masks by QUERY row: a masked row becomes a uniform softmax
    over ALL L keys (causal entries equally -inf). Recompute those rows."""
    for b in range(B):
        rows = np.nonzero(mask[b])[0]
        if len(rows) == 0:
            continue
        mu = Q[b].mean(axis=0)  # uniform attention over all keys
        l_norm = -mu[0] ** 2 + np.sum(mu[1:] ** 2)
        denom = np.sqrt(max(abs(l_norm), EPS))
        out[b, rows, :] = (mu / denom)[None, :]
    return out


LAST_EXEC_NS = None
LAST_RESULTS = None


def kernel(Q, mask, scale, bias, _trace=False):
    global LAST_EXEC_NS, LAST_RESULTS
    Q = np.ascontiguousarray(np.asarray(Q, dtype=np.float32))
    mask_np = np.asarray(mask).astype(bool).reshape(B, L)
    scale_v = float(np.asarray(scale).reshape(-1)[0])
    bias_v = float(np.asarray(bias).reshape(-1)[0]) if np.asarray(bias).size else float(bias)

    a_scale = 2.0 / scale_v              # folded into q host-side
    b0 = 2.0 / scale_v + bias_v          # activation bias immediate
    poison = -(500.0 + abs(b0)) / a_scale

    if _trace:
        _ensure_ntff_hook()
    nc = _build_program(b0)

    in_maps = []
    for c in range(N_CORES):
        if _STRATEGY == "v2":
            in_maps.append(_prep_core_inputs_v2(Q, c // 2, c % 2, a_scale))
        else:
            in_maps.append(_prep_core_inputs(Q, c // 2, c % 2, a_scale, poison))

    res = bass_utils.run_bass_kernel_spmd(
        nc, in_maps, core_ids=list(range(N_CORES)), trace=_trace
    )
    LAST_EXEC_NS = res.exec_time_ns
    LAST_RESULTS = res

    out = np.empty((B, L, D), np.float32)
    for c in range(N_CORES):
        o = res.results[c]["out"]
        if _STRATEGY == "v2":
            # [128, 16*64] p-major -> [2048, 64]
            o = o.reshape(QT, NT_CORE, D).transpose(1, 0, 2).reshape(NQ_CORE, D)
        b, half = c // 2, c % 2
        for s, g in enumerate(HALF_GROUPS[half]):
            out[b, g * SLOT_Q:(g + 1) * SLOT_Q, :] = o[s * SLOT_Q:(s + 1) * SLOT_Q, :]

    if mask_np.any():
        out = _mask_fixup(out, Q, mask_np, scale_v, bias_v)
    return out



# revision 31
# speedup vs baseline: 1.4798x; 1.4707x over previous
"""Trainium2 Bass kernel for causal Lorentz self-attention.

Problem: B=4, L=4096, D=64 single-head self-attention where
  scores = (2 + 2*<q,k>_L) / scale + bias   (Lorentz inner product)
  causal mask (strict upper triangle) + per-query pad-mask
  attn = softmax(scores);  mu = attn @ v
  out = mu / sqrt(max(|<mu,mu>_L|, eps))

Key algebraic fact used: the softmax denominator cancels in the final
normalization (out = mu_raw / sqrt(|<mu_raw,mu_raw>_L|)), so no row-sum
is computed on device.

Sharding: 2 cores per batch. Each core runs an IDENTICAL static program of
4 "slots" (512 queries each) with static k-extents (8,16,24,32) steps of 128
keys. Which query tiles a slot owns, and where the causal boundary falls, is
encoded purely in host-prepared per-core input data:
  - k iterated DESCENDING from the diagonal, so the 4 boundary steps are
    always steps 0..3 of a slot (static affine_select masks),
  - slots whose causal extent is shorter than the static extent get
    "poison" K columns (huge negative score -> exp underflows to 0) and
    zero V rows.

Precision strategy "split" (default): all matmuls run in bf16 with hi/lo
decomposition (x = bf16(x) + bf16(x - bf16(x)), ~2^-17 operand precision):
  - scores: lhsT = [K_hi; K_lo] stacked on the contraction dim (K=128)
    against Q_hi replicated, plus a K=64 correction matmul K_hi x Q_lo.
  - attn@V: lhsT = [V_hi | V_lo] stacked on the output dim (M=128),
    two moving passes with P_hi and P_lo; the hi/lo output halves are
    summed once per slot (linearity lets them accumulate separately).
Strategy "f32" is the exact-fp32 fallback (4x slower matmuls).
"""

import os
import numpy as np
import ml_dtypes

import concourse.bass as bass
import concourse.bacc as bacc
import concourse.tile as tile
from concourse import mybir
from concourse import masks as cmasks
from concourse import bass_utils
from concourse._compat import with_exitstack
from contextlib import ExitStack

B, L, D = 4, 4096, 64
EPS = 1e-8
N_CORES = 8
QT = 128                       # queries per q-tile / keys per k-step
SLOT_Q = 512                   # queries per slot (4 q-tiles)
SLOTS = 4                      # slots per core
NQ_CORE = SLOTS * SLOT_Q       # 2048 queries per core
NT_CORE = NQ_CORE // QT        # 16 q-tiles per core
SLOT_EXTENTS = (8, 16, 24, 32)  # static k-steps per slot
TOTAL_STEPS = sum(SLOT_EXTENTS)  # 80
# groups of 4 consecutive q-tiles; group g covers q-tiles 4g..4g+3 and needs
# 4g+4 k-tiles. Half 0 gets groups (0,3,4,7) -> extents (4,16,20,32), half 1
# gets (1,2,5,6) -> (8,12,24,28); both fit elementwise under SLOT_EXTENTS.
HALF_GROUPS = ((0, 3, 4, 7), (1, 2, 5, 6))

_F32 = mybir.dt.float32
_BF16 = mybir.dt.bfloat16
_FP16 = mybir.dt.float16
_U16 = mybir.dt.uint16
_BF16_NP = ml_dtypes.bfloat16
_LOG2E = 1.4426950408889634
_SCHRAUDOLPH_A = 1024.0 * _LOG2E          # fp16-bits slope
_SCHRAUDOLPH_B0 = 1024.0 * 15.0 - 44.0    # fp16-bits intercept (C=44 minimax)
_ACT_COLS = 640                           # exp cols on ACT; rest on DVE
# strategy:
#   "k66"   - exploit Lorentz structure: time component (the only large
#             score term) as bf16 hi/lo cross-terms, spatial components as
#             single bf16 -> ONE K=66 score matmul. P fp16 from ACT, V fp16
#             hi/lo stack. 2 MMs/step total.
#   "mixed" - bf16 hi/lo pairs for K/Q, P fp16, V fp16 stack. 3 MMs/step.
#   "fp16"  - fp16 hi/lo pairs for K/Q, P fp16, V fp16 stack. 3 MMs/step,
#             but fp16 matmuls are half-rate on PE.
#   "split" - bf16 hi/lo everywhere incl. P (4 MMs/step + DVE splits)
#   "f32"   - exact fp32 fallback (4x slower matmuls)
_STRATEGY = os.environ.get("KERNEL_MM_DT", "v2")
_KSTACK = 66  # rows: [-k0h, -k0l, -k0h, k_space(63)] x [q0h, q0h, q0l, q_space]
_KPAD = 128   # contraction rows padded to full 128 partitions: sub-128
              # partition matmuls cap the PE clock at the mid p-state

_cache = {}


def _ensure_ntff_hook():
    """The agent image lacks ``antenv.axon_hooks``; synthesize it using the
    ctypes NTFF driver from trn_agent_boot so trace=True works."""
    import sys as _sys
    if "antenv.axon_hooks" in _sys.modules:
        return
    try:
        import types as _types
        import antenv  # noqa: F401
        from trn_agent_boot.trn_boot import _ntff_profile_via_ctypes
        hook = _ntff_profile_via_ctypes("/opt/axon/libaxon_pjrt.so")
        m = _types.ModuleType("antenv.axon_hooks")
        m.get_axon_ntff_profile_hook = lambda: hook
        m.set_axon_ntff_profile_hook = lambda h: None
        _sys.modules["antenv.axon_hooks"] = m
    except Exception:
        pass


@with_exitstack
def _body_v2(ctx: ExitStack, tc, aps, bias_val):
    """Fully SBUF-resident K/V/Q, K=66 Lorentz-structured score matmul,
    single-fp16 V. Exp of each 2-step round is split between ACT (true exp,
    cols 0:ACT_COLS) and DVE (Schraudolph fp16-bits exp, rest). Diagonal
    (masked) steps sit at slot END (pads first) so slot starts never stall;
    rounds stream across slot boundaries with a 1-round skew; epilogues are
    spread in small chunks between rounds. rsqrt via DVE bit trick + one
    Newton step keeps the ACT Exp table resident the whole kernel."""
    nc = tc.nc
    PSUM = bass.MemorySpace.PSUM

    const = ctx.enter_context(tc.tile_pool(name="const", bufs=1))
    datap = ctx.enter_context(tc.tile_pool(name="datap", bufs=1))
    expp = ctx.enter_context(tc.tile_pool(name="expp", bufs=3))
    stp = ctx.enter_context(tc.tile_pool(name="stp", bufs=3, space=PSUM))
    mup = ctx.enter_context(tc.tile_pool(name="mup", bufs=2, space=PSUM))
    sbp = ctx.enter_context(tc.tile_pool(name="sbp", bufs=1))
    smallp = ctx.enter_context(tc.tile_pool(name="smallp", bufs=4))
    outp = ctx.enter_context(tc.tile_pool(name="outp", bufs=1))

    ident = const.tile([128, 128], _F32)
    cmasks.make_identity(nc, ident[:])
    bias_t = const.tile([128, 1], _F32)
    nc.vector.memset(bias_t[:], float(bias_val))
    # DVE Schraudolph intercept: bits = A*(ps + b) + B0 = A*ps + (B0 + A*b)
    dve_b = _SCHRAUDOLPH_B0 + _SCHRAUDOLPH_A * float(bias_val)

    # ---- bulk preloads; first two triggers cover slot 0's working set ----
    slot_base = [sum(SLOT_EXTENTS[:i]) for i in range(SLOTS + 1)]
    kd_sb = datap.tile([_KPAD, TOTAL_STEPS * QT], _BF16)
    nc.sync.dma_start(kd_sb[:, 0:slot_base[1] * QT],
                      aps["kd66"][:, 0:slot_base[1] * QT])
    qd_sb = datap.tile([_KPAD, NQ_CORE], _BF16)
    nc.sync.dma_start(qd_sb[:, 0:SLOT_Q], aps["qd66"][:, 0:SLOT_Q])
    vn_sb = datap.tile([QT, TOTAL_STEPS * D], _FP16)
    nc.sync.dma_start(vn_sb[:, 0:slot_base[1] * D],
                      aps["vn"][:, 0:slot_base[1] * D])
    nc.sync.dma_start(qd_sb[:, SLOT_Q:], aps["qd66"][:, SLOT_Q:])
    for s0 in range(1, SLOTS):
        nc.sync.dma_start(
            kd_sb[:, slot_base[s0] * QT:slot_base[s0 + 1] * QT],
            aps["kd66"][:, slot_base[s0] * QT:slot_base[s0 + 1] * QT],
        )
        nc.sync.dma_start(
            vn_sb[:, slot_base[s0] * D:slot_base[s0 + 1] * D],
            aps["vn"][:, slot_base[s0] * D:slot_base[s0 + 1] * D],
        )

    # ---- PE warm-up: short matmuls ramp the clock during the DMA fill ----
    wsrc = const.tile([QT, SLOT_Q], _BF16)
    nc.vector.memset(wsrc[:], 0.0)
    n_warm = int(os.environ.get("KERNEL_N_WARM", "6"))
    for w in range(n_warm):
        wps = stp.tile([QT, 2 * SLOT_Q], _F32, tag="ps", name=f"warm{w}")
        nc.tensor.matmul(wps[:, 0:QT], lhsT=wsrc[:, 0:QT],
                         rhs=wsrc[:, 0:QT], start=True, stop=True)

    # mu_all padded to 128 partitions (rows 64:128 zeroed once) so the
    # epilogue transposes are full-partition ops (sub-128 caps the PE clock)
    mu_all = sbp.tile([128, NQ_CORE], _F32)
    nc.vector.memset(mu_all[64:128, :], 0.0)
    muq_all = sbp.tile([128, NT_CORE * D], _F32)
    ln_all = sbp.tile([128, NT_CORE], _F32)
    invs_all = sbp.tile([128, NT_CORE], _F32)
    out_sb = outp.tile([128, NT_CORE * D], _F32)

    def score_round(s, r):
        ps = stp.tile([QT, 2 * SLOT_Q], _F32)
        qblk = qd_sb[:, s * SLOT_Q:(s + 1) * SLOT_Q]
        for h in (0, 1):
            st = slot_base[s] + 2 * r + h
            nc.tensor.matmul(
                ps[:, h * SLOT_Q:(h + 1) * SLOT_Q],
                lhsT=kd_sb[:, st * QT:(st + 1) * QT],
                rhs=qblk,
                start=True, stop=True,
            )
        return ps

    def exp_round(s, r, ps):
        ext = SLOT_EXTENTS[s]
        et = expp.tile([QT, 2 * SLOT_Q], _FP16)
        # step h=0 -> ACT true exp, step h=1 -> DVE Schraudolph bits exp.
        # Diagonal (masked) steps: columns q' < 128*m are fully above the
        # diagonal — skip their exp; the affine_select zero-fills them.
        m0 = 2 * r - (ext - 4)
        if m0 < 0:
            # unmasked round: ACT takes step 0 plus 128 cols of step 1
            act_lo, act_hi = 0, _ACT_COLS
            dve_lo = _ACT_COLS
        else:
            act_lo, act_hi = QT * m0, SLOT_Q
            dve_lo = SLOT_Q + QT * (m0 + 1)
        nc.scalar.activation(
            et[:, act_lo:act_hi], ps[:, act_lo:act_hi],
            mybir.ActivationFunctionType.Exp,
            bias=bias_t[:], scale=1.0,
        )
        nc.vector.tensor_scalar(
            out=et[:, dve_lo:].bitcast(_U16),
            in0=ps[:, dve_lo:],
            scalar1=_SCHRAUDOLPH_A,
            scalar2=dve_b,
            op0=mybir.AluOpType.mult,
            op1=mybir.AluOpType.add,
        )
        for h in (0, 1):
            m = 2 * r + h - (ext - 4)
            if m >= 0:
                nc.gpsimd.affine_select(
                    out=et[:, h * SLOT_Q:(h + 1) * SLOT_Q],
                    in_=et[:, h * SLOT_Q:(h + 1) * SLOT_Q],
                    compare_op=mybir.AluOpType.is_ge,
                    fill=0.0,
                    base=-QT * m,
                    pattern=[[1, SLOT_Q]],
                    channel_multiplier=-1,
                )
        return et

    mu_tiles = {}
    epi_queue = []

    def av_round(s, r, et):
        ext = SLOT_EXTENTS[s]
        if r == 0:
            # [128, 512] tile: AV accumulates mu into rows 0:64; after the
            # drain, the same bank is reused for the epilogue transposes.
            mu_tiles[s] = mup.tile([QT, SLOT_Q], _F32, tag="mu", name=f"mu{s}")
        mu_ps = mu_tiles[s]
        for h in (0, 1):
            st = slot_base[s] + 2 * r + h
            nc.tensor.matmul(
                mu_ps[0:64, :],
                lhsT=vn_sb[:, st * D:(st + 1) * D],
                rhs=et[:, h * SLOT_Q:(h + 1) * SLOT_Q],
                start=(r == 0 and h == 0),
                stop=(2 * r + h == ext - 1),
            )
        if 2 * r + 1 == ext - 1:
            # drain mu to SBUF on ACT (it has the most slack)
            nc.scalar.copy(
                mu_all[0:64, s * SLOT_Q:(s + 1) * SLOT_Q], mu_ps[0:64, :]
            )
            for p in range(7):
                epi_queue.append((s, p))

    def epi_lnorm(sp, q):
        """|l| = 2*mu0^2 - sum(mu_d^2) for one q-tile (l is always < 0)."""
        qt_i = sp * 4 + q
        muq = muq_all[:, qt_i * D:(qt_i + 1) * D]
        sq = smallp.tile([QT, D], _F32)
        nc.vector.tensor_mul(sq[:], muq, muq)
        red = smallp.tile([QT, 1], _F32)
        nc.vector.reduce_sum(red[:], sq[:], axis=mybir.AxisListType.X)
        nc.vector.scalar_tensor_tensor(
            out=ln_all[:, qt_i:qt_i + 1],
            in0=sq[:, 0:1],
            scalar=2.0,
            in1=red[:],
            op0=mybir.AluOpType.mult,
            op1=mybir.AluOpType.subtract,
        )

    def epi_invs(sp):
        """invs = 1/sqrt(ln) on DVE: quake-style bit seed + 1 Newton step
        (avoids any ACT table switch away from Exp)."""
        x = ln_all[:, sp * 4:(sp + 1) * 4]
        xb = smallp.tile([128, 4], _F32, tag="xb")
        nc.vector.tensor_copy(xb[:], x.bitcast(mybir.dt.uint32))
        y0 = smallp.tile([128, 4], _F32, tag="y0")
        nc.vector.tensor_scalar(
            out=y0[:].bitcast(mybir.dt.int32), in0=xb[:],
            scalar1=-0.5, scalar2=float(0x5F3759DF),
            op0=mybir.AluOpType.mult, op1=mybir.AluOpType.add,
        )
        t = smallp.tile([128, 4], _F32, tag="t")
        nc.vector.tensor_mul(t[:], y0[:], y0[:])
        nc.vector.tensor_mul(t[:], t[:], x)
        nc.vector.tensor_scalar(
            out=t[:], in0=t[:], scalar1=-0.5, scalar2=1.5,
            op0=mybir.AluOpType.mult, op1=mybir.AluOpType.add,
        )
        nc.vector.tensor_mul(invs_all[:, sp * 4:(sp + 1) * 4], y0[:], t[:])

    def epi_scale(sp):
        for q in range(4):
            qt_i = sp * 4 + q
            nc.vector.tensor_scalar_mul(
                out_sb[:, qt_i * D:(qt_i + 1) * D],
                muq_all[:, qt_i * D:(qt_i + 1) * D],
                invs_all[:, qt_i:qt_i + 1],
            )
        nc.sync.dma_start(
            aps["out"][:, sp * 256:(sp + 1) * 256],
            out_sb[:, sp * 256:(sp + 1) * 256],
        )

    def epilogue_piece(sp, p):
        """One slot-tail piece; at most one is emitted per round."""
        if p <= 1:
            for q in (2 * p, 2 * p + 1):  # transposes into drained mu bank
                qt_i = sp * 4 + q
                nc.tensor.transpose(
                    mu_tiles[sp][:, q * QT:(q + 1) * QT],
                    mu_all[:, qt_i * QT:(qt_i + 1) * QT], ident[:],
                )
        elif p == 2:
            # all 4 tp's (real data in cols 0:64 of each 128-block) -> SBUF
            nc.vector.tensor_copy(
                muq_all[:, sp * 256:(sp + 1) * 256]
                .rearrange("p (q d) -> p q d", d=64),
                mu_tiles[sp][:].rearrange("p (q d) -> p q d", d=128)[:, :, 0:64],
            )
        elif p == 3:
            epi_lnorm(sp, 0)
            epi_lnorm(sp, 1)
        elif p == 4:
            epi_lnorm(sp, 2)
            epi_lnorm(sp, 3)
        elif p == 5:
            epi_invs(sp)
        elif p == 6:
            epi_scale(sp)

    # ---- flat pipelined stream over all rounds, 2-round AV skew;
    # epilogue pieces drip out one per round from a global queue ----
    rounds = [(s, r) for s in range(SLOTS) for r in range(SLOT_EXTENTS[s] // 2)]
    pend = []
    for s, r in rounds:
        ps = score_round(s, r)
        if len(pend) >= 2:
            av_round(*pend.pop(0))
        if epi_queue:
            epilogue_piece(*epi_queue.pop(0))
        et = exp_round(s, r, ps)
        pend.append((s, r, et))
    for p in pend:
        av_round(*p)
    while epi_queue:
        epilogue_piece(*epi_queue.pop(0))


@with_exitstack
def _body_k66(ctx: ExitStack, tc, aps, bias_val):
    """Single K=66 score matmul per step (Lorentz-structured hi/lo),
    fp16 P/V attention matmul. Skewed pipeline."""
    nc = tc.nc
    PSUM = bass.MemorySpace.PSUM

    const = ctx.enter_context(tc.tile_pool(name="const", bufs=1))
    qdp = ctx.enter_context(tc.tile_pool(name="qdp", bufs=1))
    kdp = ctx.enter_context(tc.tile_pool(name="kdp", bufs=4))
    vnp = ctx.enter_context(tc.tile_pool(name="vnp", bufs=4))
    expp = ctx.enter_context(tc.tile_pool(name="expp", bufs=3))
    stp = ctx.enter_context(tc.tile_pool(name="stp", bufs=3, space=PSUM))
    mup = ctx.enter_context(tc.tile_pool(name="mup", bufs=2, space=PSUM))
    tpp = ctx.enter_context(tc.tile_pool(name="tpp", bufs=2, space=PSUM))
    sbp = ctx.enter_context(tc.tile_pool(name="sbp", bufs=1))
    smallp = ctx.enter_context(tc.tile_pool(name="smallp", bufs=4))
    outp = ctx.enter_context(tc.tile_pool(name="outp", bufs=3))

    ident = const.tile([64, 64], _F32)
    cmasks.make_identity(nc, ident[:])
    bias_t = const.tile([128, 1], _F32)
    nc.vector.memset(bias_t[:], float(bias_val))

    qd_sb = qdp.tile([_KSTACK, NQ_CORE], _BF16)
    for s0 in range(SLOTS):
        c0, c1 = s0 * SLOT_Q, (s0 + 1) * SLOT_Q
        nc.sync.dma_start(qd_sb[:, c0:c1], aps["qd66"][:, c0:c1])

    mu_sb = sbp.tile([64, NQ_CORE], _F32)
    muq_all = sbp.tile([128, NT_CORE * D], _F32)
    ln_all = sbp.tile([128, NT_CORE], _F32)

    step_base = 0
    for s in range(SLOTS):
        ext = SLOT_EXTENTS[s]
        q_lo = s * SLOT_Q
        mu_ps = mup.tile([QT, SLOT_Q], _F32)  # rows 0-63 hi, 64-127 lo

        def st_step(i):
            st = step_base + i
            kt = kdp.tile([_KSTACK, QT], _BF16)
            nc.sync.dma_start(kt[:], aps["kd66"][:, st * QT:(st + 1) * QT])
            ps = stp.tile([QT, SLOT_Q], _F32)
            nc.tensor.matmul(
                ps[:], lhsT=kt[:], rhs=qd_sb[:, q_lo:q_lo + SLOT_Q],
                start=True, stop=True,
            )
            return ps

        def av_step(i, ps):
            st = step_base + i
            vt = vnp.tile([QT, QT], _FP16)
            nc.sync.dma_start(vt[:], aps["vn"][st * QT:(st + 1) * QT, :])
            et = expp.tile([QT, SLOT_Q], _FP16)
            nc.scalar.activation(
                et[:], ps[:], mybir.ActivationFunctionType.Exp,
                bias=bias_t[:], scale=1.0,
            )
            if i < 4:
                nc.gpsimd.affine_select(
                    out=et[:], in_=et[:],
                    compare_op=mybir.AluOpType.is_ge,
                    fill=0.0,
                    base=-QT * (3 - i),
                    pattern=[[1, SLOT_Q]],
                    channel_multiplier=-1,
                )
            nc.tensor.matmul(
                mu_ps[:], lhsT=vt[:], rhs=et[:],
                start=(i == 0), stop=(i == ext - 1),
            )

        ps_prev = st_step(0)
        for i in range(1, ext):
            ps_i = st_step(i)
            av_step(i - 1, ps_prev)
            ps_prev = ps_i
        av_step(ext - 1, ps_prev)
        step_base += ext

        lo_sb = smallp.tile([64, SLOT_Q], _F32, tag="losb")
        nc.scalar.copy(lo_sb[:], mu_ps[64:128, :])
        nc.vector.tensor_add(mu_sb[:, q_lo:q_lo + SLOT_Q], mu_ps[0:64, :], lo_sb[:])

        for q in range(SLOT_Q // QT):
            qt_i = s * (SLOT_Q // QT) + q
            tp = tpp.tile([QT, 64], _F32)
            nc.tensor.transpose(
                tp[:], mu_sb[:, qt_i * QT:(qt_i + 1) * QT], ident[:]
            )
            muq = muq_all[:, qt_i * D:(qt_i + 1) * D]
            nc.scalar.copy(muq, tp[:, :D])
            sq = smallp.tile([QT, D], _F32)
            nc.vector.tensor_mul(sq[:], muq, muq)
            red = smallp.tile([QT, 1], _F32)
            nc.vector.reduce_sum(red[:], sq[:], axis=mybir.AxisListType.X)
            nc.vector.scalar_tensor_tensor(
                out=ln_all[:, qt_i:qt_i + 1],
                in0=sq[:, 0:1],
                scalar=2.0,
                in1=red[:],
                op0=mybir.AluOpType.mult,
                op1=mybir.AluOpType.subtract,
            )

        # per-slot normalize: 1/sqrt(x) = exp(-0.5*ln(x)); Ln and Exp share
        # one ACT table set, so no table switch and no end-of-kernel phase.
        lns = ln_all[:, s * 4:(s + 1) * 4]
        lnt = smallp.tile([128, 4], _F32, tag="lnt")
        nc.scalar.activation(lnt[:], lns, mybir.ActivationFunctionType.Ln)
        invs = smallp.tile([128, 4], _F32, tag="invs")
        nc.scalar.activation(
            invs[:], lnt[:], mybir.ActivationFunctionType.Exp,
            bias=0.0, scale=-0.5,
        )
        for q in range(SLOT_Q // QT):
            qt_i = s * (SLOT_Q // QT) + q
            ot = outp.tile([QT, D], _F32)
            nc.vector.tensor_scalar_mul(
                ot[:], muq_all[:, qt_i * D:(qt_i + 1) * D], invs[:, q:q + 1]
            )
            nc.sync.dma_start(aps["out"][qt_i * QT:(qt_i + 1) * QT, :], ot[:])


@with_exitstack
def _body_fp16(ctx: ExitStack, tc, aps, bias_val, kq_dt=_FP16):
    """hi/lo-pair strategy with software-pipelined (skewed) step loop and
    per-slot preloaded K/V (per-step DMA triggers serialize on the sync
    sequencer at ~590ns each, so they must be batched).
    kq_dt: dtype of the K/Q score operands (bf16 = PE full rate)."""
    nc = tc.nc
    PSUM = bass.MemorySpace.PSUM

    const = ctx.enter_context(tc.tile_pool(name="const", bufs=1))
    qdp = ctx.enter_context(tc.tile_pool(name="qdp", bufs=1))
    kdp = ctx.enter_context(tc.tile_pool(name="kdp", bufs=1))
    vnp = ctx.enter_context(tc.tile_pool(name="vnp", bufs=1))
    expp = ctx.enter_context(tc.tile_pool(name="expp", bufs=3))
    stp = ctx.enter_context(tc.tile_pool(name="stp", bufs=3, space=PSUM))
    mup = ctx.enter_context(tc.tile_pool(name="mup", bufs=2, space=PSUM))
    tpp = ctx.enter_context(tc.tile_pool(name="tpp", bufs=2, space=PSUM))
    sbp = ctx.enter_context(tc.tile_pool(name="sbp", bufs=1))
    smallp = ctx.enter_context(tc.tile_pool(name="smallp", bufs=4))
    outp = ctx.enter_context(tc.tile_pool(name="outp", bufs=3))

    ident = const.tile([64, 64], _F32)
    cmasks.make_identity(nc, ident[:])
    bias_t = const.tile([128, 1], _F32)
    nc.vector.memset(bias_t[:], float(bias_val))

    # PE warm-up: ~16 dummy matmuls during the initial DMA window so the
    # HAM clock-gate reaches 2.4 GHz before the first real matmul.
    wsrc = const.tile([QT, SLOT_Q], kq_dt)
    nc.gpsimd.memset(wsrc[:], 0.0)
    for w in range(16):
        wps = tpp.tile([QT, SLOT_Q], _F32, tag="warm", bufs=1)
        nc.tensor.matmul(wps[:], lhsT=wsrc[:, 0:QT], rhs=wsrc[:],
                         start=True, stop=True)

    # causal boundary masks (fp16 ones/zeros incl. diagonal triangle),
    # applied with a DVE multiply instead of a gpsimd affine_select on the
    # exp->AV critical path.
    bmask = const.tile([QT, 4, SLOT_Q], _FP16)
    nc.vector.memset(bmask[:], 1.0)
    for i in range(4):
        nc.gpsimd.affine_select(
            out=bmask[:, i, :], in_=bmask[:, i, :],
            compare_op=mybir.AluOpType.is_ge,
            fill=0.0,
            base=-QT * (3 - i),
            pattern=[[1, SLOT_Q]],
            channel_multiplier=-1,
        )

    qdh_sb = qdp.tile([128, NQ_CORE], kq_dt)
    qdl_sb = qdp.tile([64, NQ_CORE], kq_dt)
    kd_sb = {}
    vn_sb = {}
    base = 0
    for s0 in range(SLOTS):
        ext = SLOT_EXTENTS[s0]
        c0, c1 = s0 * SLOT_Q, (s0 + 1) * SLOT_Q
        nc.sync.dma_start(qdh_sb[:, c0:c1], aps["qdh"][:, c0:c1])
        nc.sync.dma_start(qdl_sb[:, c0:c1], aps["qdl"][:, c0:c1])
        kd_sb[s0] = kdp.tile([QT, ext * QT], kq_dt, tag=f"kd{s0}", name=f"kd_sb{s0}")
        nc.sync.dma_start(kd_sb[s0][:], aps["kd"][:, base * QT:(base + ext) * QT])
        vn_sb[s0] = vnp.tile([QT, ext, QT], _FP16, tag=f"vn{s0}", name=f"vn_sb{s0}")
        vsrc = aps["vn"][base * QT:(base + ext) * QT, :].rearrange(
            "(t p) c -> p t c", p=QT)
        nc.sync.dma_start(vn_sb[s0][:], vsrc)
        base += ext

    mu_sb = sbp.tile([64, NQ_CORE], _F32)
    muq_all = sbp.tile([128, NT_CORE * D], _F32)
    ln_all = sbp.tile([128, NT_CORE], _F32)

    step_base = 0
    for s in range(SLOTS):
        ext = SLOT_EXTENTS[s]
        q_lo = s * SLOT_Q
        mu_ps = mup.tile([QT, SLOT_Q], _F32)  # rows 0-63 hi, 64-127 lo

        def st_step(i):
            kt = kd_sb[s][:, i * QT:(i + 1) * QT]
            ps = stp.tile([QT, SLOT_Q], _F32)
            nc.tensor.matmul(
                ps[:], lhsT=kt, rhs=qdh_sb[:, q_lo:q_lo + SLOT_Q],
                start=True, stop=False,
            )
            nc.tensor.matmul(
                ps[:], lhsT=kt[0:64, :], rhs=qdl_sb[:, q_lo:q_lo + SLOT_Q],
                start=False, stop=True,
            )
            return ps

        def av_step(i, ps):
            vt = vn_sb[s][:, i, :]
            et = expp.tile([QT, SLOT_Q], _FP16)
            nc.scalar.activation(
                et[:], ps[:], mybir.ActivationFunctionType.Exp,
                bias=bias_t[:], scale=1.0,
            )
            if i < 4:
                # step i's k-tile is the (3-i)'th q-tile block's diagonal
                nc.vector.tensor_mul(et[:], et[:], bmask[:, i, :])
            nc.tensor.matmul(
                mu_ps[:], lhsT=vt, rhs=et[:],
                start=(i == 0), stop=(i == ext - 1),
            )

        # 2-deep skewed pipeline: S_T(i+2) runs on PE before AV(i), covering
        # the exp latency (and the boundary-mask multiply) on ACT/DVE.
        pending = [st_step(0), st_step(1)]
        for i in range(2, ext):
            pending.append(st_step(i))
            av_step(i - 2, pending.pop(0))
        av_step(ext - 2, pending.pop(0))
        av_step(ext - 1, pending.pop(0))
        step_base += ext

        # mu = hi half + lo half (one PSUM operand max per DVE op)
        lo_sb = smallp.tile([64, SLOT_Q], _F32, tag="losb")
        nc.scalar.copy(lo_sb[:], mu_ps[64:128, :])
        nc.vector.tensor_add(mu_sb[:, q_lo:q_lo + SLOT_Q], mu_ps[0:64, :], lo_sb[:])

        for q in range(SLOT_Q // QT):
            qt_i = s * (SLOT_Q // QT) + q
            tp = tpp.tile([QT, 64], _F32)
            nc.tensor.transpose(
                tp[:], mu_sb[:, qt_i * QT:(qt_i + 1) * QT], ident[:]
            )
            muq = muq_all[:, qt_i * D:(qt_i + 1) * D]
            nc.scalar.copy(muq, tp[:, :D])
            sq = smallp.tile([QT, D], _F32)
            nc.vector.tensor_mul(sq[:], muq, muq)
            red = smallp.tile([QT, 1], _F32)
            nc.vector.reduce_sum(red[:], sq[:], axis=mybir.AxisListType.X)
            # |l| = -l = 2*mu0^2 - sum(mu_d^2)  (l is always < 0 here)
            nc.vector.scalar_tensor_tensor(
                out=ln_all[:, qt_i:qt_i + 1],
                in0=sq[:, 0:1],
                scalar=2.0,
                in1=red[:],
                op0=mybir.AluOpType.mult,
                op1=mybir.AluOpType.subtract,
            )

        # per-slot normalize: 1/sqrt(x) = exp(-0.5*ln(x)); Ln and Exp share
        # one ACT table set, so no table switch and no end-of-kernel phase.
        lns = ln_all[:, s * 4:(s + 1) * 4]
        lnt = smallp.tile([128, 4], _F32, tag="lnt")
        nc.scalar.activation(lnt[:], lns, mybir.ActivationFunctionType.Ln)
        invs = smallp.tile([128, 4], _F32, tag="invs")
        nc.scalar.activation(
            invs[:], lnt[:], mybir.ActivationFunctionType.Exp,
            bias=0.0, scale=-0.5,
        )
        for q in range(SLOT_Q // QT):
            qt_i = s * (SLOT_Q // QT) + q
            ot = outp.tile([QT, D], _F32)
            nc.vector.tensor_scalar_mul(
                ot[:], muq_all[:, qt_i * D:(qt_i + 1) * D], invs[:, q:q + 1]
            )
            nc.sync.dma_start(aps["out"][qt_i * QT:(qt_i + 1) * QT, :], ot[:])


@with_exitstack
def _body_split(ctx: ExitStack, tc, aps, bias_val):
    """bf16 hi/lo strategy. aps: dict of DRAM APs."""
    nc = tc.nc
    PSUM = bass.MemorySpace.PSUM

    const = ctx.enter_context(tc.tile_pool(name="const", bufs=1))
    qdp = ctx.enter_context(tc.tile_pool(name="qdp", bufs=1))
    kdp = ctx.enter_context(tc.tile_pool(name="kdp", bufs=4))
    vnp = ctx.enter_context(tc.tile_pool(name="vnp", bufs=4))
    expp = ctx.enter_context(tc.tile_pool(name="expp", bufs=3))
    ehp = ctx.enter_context(tc.tile_pool(name="ehp", bufs=3))
    elp = ctx.enter_context(tc.tile_pool(name="elp", bufs=3))
    stp = ctx.enter_context(tc.tile_pool(name="stp", bufs=2, space=PSUM))
    mup = ctx.enter_context(tc.tile_pool(name="mup", bufs=2, space=PSUM))
    tpp = ctx.enter_context(tc.tile_pool(name="tpp", bufs=2, space=PSUM))
    sbp = ctx.enter_context(tc.tile_pool(name="sbp", bufs=1))
    smallp = ctx.enter_context(tc.tile_pool(name="smallp", bufs=4))
    outp = ctx.enter_context(tc.tile_pool(name="outp", bufs=3))

    ident = const.tile([64, 64], _F32)
    cmasks.make_identity(nc, ident[:])
    bias_t = const.tile([128, 1], _F32)
    nc.vector.memset(bias_t[:], float(bias_val))

    qdh_sb = qdp.tile([128, NQ_CORE], _BF16)
    nc.sync.dma_start(qdh_sb[:], aps["qdh"][:])
    qdl_sb = qdp.tile([64, NQ_CORE], _BF16)
    nc.sync.dma_start(qdl_sb[:], aps["qdl"][:])

    mu_sb = sbp.tile([64, NQ_CORE], _F32)
    muq_all = sbp.tile([128, NT_CORE * D], _F32)
    ln_all = sbp.tile([128, NT_CORE], _F32)

    step_base = 0
    for s in range(SLOTS):
        ext = SLOT_EXTENTS[s]
        q_lo = s * SLOT_Q
        mu_ps = mup.tile([QT, SLOT_Q], _F32)  # rows 0-63 hi, 64-127 lo
        for i in range(ext):
            st = step_base + i
            kt = kdp.tile([QT, QT], _BF16)
            nc.sync.dma_start(kt[:], aps["kd"][:, st * QT:(st + 1) * QT])
            vt = vnp.tile([QT, QT], _BF16)
            nc.sync.dma_start(vt[:], aps["vn"][st * QT:(st + 1) * QT, :])

            ps = stp.tile([QT, SLOT_Q], _F32)
            nc.tensor.matmul(
                ps[:], lhsT=kt[:], rhs=qdh_sb[:, q_lo:q_lo + SLOT_Q],
                start=True, stop=False,
            )
            nc.tensor.matmul(
                ps[:], lhsT=kt[0:64, :], rhs=qdl_sb[:, q_lo:q_lo + SLOT_Q],
                start=False, stop=True,
            )
            et = expp.tile([QT, SLOT_Q], _F32)
            nc.scalar.activation(
                et[:], ps[:], mybir.ActivationFunctionType.Exp,
                bias=bias_t[:], scale=1.0,
            )
            if i < 4:
                # step i's k-tile is the (3-i)'th q-tile block's diagonal:
                # keep element (k, q) iff q - k - 128*(3-i) >= 0
                nc.gpsimd.affine_select(
                    out=et[:], in_=et[:],
                    compare_op=mybir.AluOpType.is_ge,
                    fill=0.0,
                    base=-QT * (3 - i),
                    pattern=[[1, SLOT_Q]],
                    channel_multiplier=-1,
                )
            eth = ehp.tile([QT, SLOT_Q], _BF16)
            nc.vector.tensor_copy(eth[:], et[:])
            etl = elp.tile([QT, SLOT_Q], _BF16)
            nc.vector.tensor_sub(etl[:], et[:], eth[:])
            nc.tensor.matmul(
                mu_ps[:], lhsT=vt[:], rhs=eth[:],
                start=(i == 0), stop=False,
            )
            nc.tensor.matmul(
                mu_ps[:], lhsT=vt[:], rhs=etl[:],
                start=False, stop=(i == ext - 1),
            )
        step_base += ext

        # mu = hi half + lo half (one PSUM operand max per DVE op)
        lo_sb = smallp.tile([64, SLOT_Q], _F32, tag="losb")
        nc.scalar.copy(lo_sb[:], mu_ps[64:128, :])
        nc.vector.tensor_add(mu_sb[:, q_lo:q_lo + SLOT_Q], mu_ps[0:64, :], lo_sb[:])

        for q in range(SLOT_Q // QT):
            qt_i = s * (SLOT_Q // QT) + q
            tp = tpp.tile([QT, 64], _F32)
            nc.tensor.transpose(
                tp[:], mu_sb[:, qt_i * QT:(qt_i + 1) * QT], ident[:]
            )
            muq = muq_all[:, qt_i * D:(qt_i + 1) * D]
            nc.scalar.copy(muq, tp[:, :D])
            sq = smallp.tile([QT, D], _F32)
            nc.vector.tensor_mul(sq[:], muq, muq)
            red = smallp.tile([QT, 1], _F32)
            nc.vector.reduce_sum(red[:], sq[:], axis=mybir.AxisListType.X)
            # |l| = -l = 2*mu0^2 - sum(mu_d^2)  (l is always < 0 here)
            nc.vector.scalar_tensor_tensor(
                out=ln_all[:, qt_i:qt_i + 1],
                in0=sq[:, 0:1],
                scalar=2.0,
                in1=red[:],
                op0=mybir.AluOpType.mult,
                op1=mybir.AluOpType.subtract,
            )

    # grouped sqrt (single ACT table switch) + reciprocal + final scale
    sqv = sbp.tile([128, NT_CORE], _F32)
    nc.scalar.activation(
        sqv[:], ln_all[:], mybir.ActivationFunctionType.Sqrt,
        bias=0.0, scale=1.0,
    )
    inv = sbp.tile([128, NT_CORE], _F32)
    nc.vector.reciprocal(inv[:], sqv[:])
    for qt_i in range(NT_CORE):
        ot = outp.tile([QT, D], _F32)
        nc.vector.tensor_scalar_mul(
            ot[:], muq_all[:, qt_i * D:(qt_i + 1) * D], inv[:, qt_i:qt_i + 1]
        )
        nc.sync.dma_start(aps["out"][qt_i * QT:(qt_i + 1) * QT, :], ot[:])


@with_exitstack
def _body_f32(ctx: ExitStack, tc, aps, bias_val):
    """Exact-fp32 fallback strategy."""
    nc = tc.nc
    PSUM = bass.MemorySpace.PSUM

    const = ctx.enter_context(tc.tile_pool(name="const", bufs=1))
    qdp = ctx.enter_context(tc.tile_pool(name="qdp", bufs=1))
    kdp = ctx.enter_context(tc.tile_pool(name="kdp", bufs=4))
    vnp = ctx.enter_context(tc.tile_pool(name="vnp", bufs=4))
    expp = ctx.enter_context(tc.tile_pool(name="expp", bufs=3))
    stp = ctx.enter_context(tc.tile_pool(name="stp", bufs=2, space=PSUM))
    mup = ctx.enter_context(tc.tile_pool(name="mup", bufs=2, space=PSUM))
    tpp = ctx.enter_context(tc.tile_pool(name="tpp", bufs=2, space=PSUM))
    sbp = ctx.enter_context(tc.tile_pool(name="sbp", bufs=1))
    smallp = ctx.enter_context(tc.tile_pool(name="smallp", bufs=4))
    outp = ctx.enter_context(tc.tile_pool(name="outp", bufs=3))

    ident = const.tile([64, 64], _F32)
    cmasks.make_identity(nc, ident[:])
    bias_t = const.tile([128, 1], _F32)
    nc.vector.memset(bias_t[:], float(bias_val))

    qd_sb = qdp.tile([64, NQ_CORE], _F32)
    nc.sync.dma_start(qd_sb[:], aps["qd"][:])

    mu_sb = sbp.tile([64, NQ_CORE], _F32)
    muq_all = sbp.tile([128, NT_CORE * D], _F32)
    ln_all = sbp.tile([128, NT_CORE], _F32)

    step_base = 0
    for s in range(SLOTS):
        ext = SLOT_EXTENTS[s]
        q_lo = s * SLOT_Q
        mu_ps = mup.tile([64, SLOT_Q], _F32)
        for i in range(ext):
            st = step_base + i
            kt = kdp.tile([64, QT], _F32)
            nc.sync.dma_start(kt[:], aps["kd"][:, st * QT:(st + 1) * QT])
            vt = vnp.tile([QT, D], _F32)
            nc.sync.dma_start(vt[:], aps["vn"][st * QT:(st + 1) * QT, :])

            ps = stp.tile([QT, SLOT_Q], _F32)
            nc.tensor.matmul(
                ps[:], lhsT=kt[:], rhs=qd_sb[:, q_lo:q_lo + SLOT_Q],
                start=True, stop=True,
            )
            et = expp.tile([QT, SLOT_Q], _F32)
            nc.scalar.activation(
                et[:], ps[:], mybir.ActivationFunctionType.Exp,
                bias=bias_t[:], scale=1.0,
            )
            if i < 4:
                nc.gpsimd.affine_select(
                    out=et[:], in_=et[:],
                    compare_op=mybir.AluOpType.is_ge,
                    fill=0.0,
                    base=-QT * (3 - i),
                    pattern=[[1, SLOT_Q]],
                    channel_multiplier=-1,
                )
            nc.tensor.matmul(
                mu_ps[:], lhsT=vt[:], rhs=et[:],
                start=(i == 0), stop=(i == ext - 1),
            )
        step_base += ext

        nc.vector.tensor_copy(mu_sb[:, q_lo:q_lo + SLOT_Q], mu_ps[:])
        for q in range(SLOT_Q // QT):
            qt_i = s * (SLOT_Q // QT) + q
            tp = tpp.tile([QT, 64], _F32)
            nc.tensor.transpose(
                tp[:], mu_sb[:, qt_i * QT:(qt_i + 1) * QT], ident[:]
            )
            muq = muq_all[:, qt_i * D:(qt_i + 1) * D]
            nc.scalar.copy(muq, tp[:, :D])
            sq = smallp.tile([QT, D], _F32)
            nc.vector.tensor_mul(sq[:], muq, muq)
            red = smallp.tile([QT, 1], _F32)
            nc.vector.reduce_sum(red[:], sq[:], axis=mybir.AxisListType.X)
            nc.vector.scalar_tensor_tensor(
                out=ln_all[:, qt_i:qt_i + 1],
                in0=sq[:, 0:1],
                scalar=2.0,
                in1=red[:],
                op0=mybir.AluOpType.mult,
                op1=mybir.AluOpType.subtract,
            )

    sqv = sbp.tile([128, NT_CORE], _F32)
    nc.scalar.activation(
        sqv[:], ln_all[:], mybir.ActivationFunctionType.Sqrt,
        bias=0.0, scale=1.0,
    )
    inv = sbp.tile([128, NT_CORE], _F32)
    nc.vector.reciprocal(inv[:], sqv[:])
    for qt_i in range(NT_CORE):
        ot = outp.tile([QT, D], _F32)
        nc.vector.tensor_scalar_mul(
            ot[:], muq_all[:, qt_i * D:(qt_i + 1) * D], inv[:, qt_i:qt_i + 1]
        )
        nc.sync.dma_start(aps["out"][qt_i * QT:(qt_i + 1) * QT, :], ot[:])


def _build_program(bias_val):
    key = (round(float(bias_val), 12), _STRATEGY)
    if key in _cache:
        return _cache[key]
    nc = bacc.Bacc(
        "TRN2",
        target_bir_lowering=False,
        debug=False,
        enable_asserts=False,
    )
    aps = {}
    if _STRATEGY == "v2":
        aps["qd66"] = nc.dram_tensor("qd66", [_KPAD, NQ_CORE], _BF16, kind="ExternalInput").ap()
        aps["kd66"] = nc.dram_tensor("kd66", [_KPAD, TOTAL_STEPS * QT], _BF16, kind="ExternalInput").ap()
        aps["vn"] = nc.dram_tensor("vn", [QT, TOTAL_STEPS * D], _FP16, kind="ExternalInput").ap()
        aps["out"] = nc.dram_tensor("out", [128, NT_CORE * D], _F32, kind="ExternalOutput").ap()
        with tile.TileContext(nc) as tc:
            _body_v2(tc, aps, bias_val)
        nc.compile()
        _cache[key] = nc
        return nc
    if _STRATEGY == "k66":
        aps["qd66"] = nc.dram_tensor("qd66", [_KSTACK, NQ_CORE], _BF16, kind="ExternalInput").ap()
        aps["kd66"] = nc.dram_tensor("kd66", [_KSTACK, TOTAL_STEPS * QT], _BF16, kind="ExternalInput").ap()
        aps["vn"] = nc.dram_tensor("vn", [TOTAL_STEPS * QT, 128], _FP16, kind="ExternalInput").ap()
    elif _STRATEGY in ("split", "fp16", "mixed"):
        kq_dt = _BF16 if _STRATEGY in ("split", "mixed") else _FP16
        pv_dt = _BF16 if _STRATEGY == "split" else _FP16
        aps["qdh"] = nc.dram_tensor("qdh", [128, NQ_CORE], kq_dt, kind="ExternalInput").ap()
        aps["qdl"] = nc.dram_tensor("qdl", [64, NQ_CORE], kq_dt, kind="ExternalInput").ap()
        aps["kd"] = nc.dram_tensor("kd", [128, TOTAL_STEPS * QT], kq_dt, kind="ExternalInput").ap()
        aps["vn"] = nc.dram_tensor("vn", [TOTAL_STEPS * QT, 128], pv_dt, kind="ExternalInput").ap()
    else:
        aps["qd"] = nc.dram_tensor("qd", [64, NQ_CORE], _F32, kind="ExternalInput").ap()
        aps["kd"] = nc.dram_tensor("kd", [64, TOTAL_STEPS * QT], _F32, kind="ExternalInput").ap()
        aps["vn"] = nc.dram_tensor("vn", [TOTAL_STEPS * QT, D], _F32, kind="ExternalInput").ap()
    aps["out"] = nc.dram_tensor("out", [NQ_CORE, D], _F32, kind="ExternalOutput").ap()
    with tile.TileContext(nc) as tc:
        if _STRATEGY == "k66":
            _body_k66(tc, aps, bias_val)
        elif _STRATEGY == "mixed":
            _body_fp16(tc, aps, bias_val, kq_dt=_BF16)
        elif _STRATEGY == "fp16":
            _body_fp16(tc, aps, bias_val, kq_dt=_FP16)
        elif _STRATEGY == "split":
            _body_split(tc, aps, bias_val)
        else:
            _body_f32(tc, aps, bias_val)
    nc.compile()
    _cache[key] = nc
    return nc


def _hilo(x, np_dt):
    hi = x.astype(np_dt)
    lo = (x - hi.astype(np.float32)).astype(np_dt)
    return hi, lo


def _prep_core_inputs_v2(Q, b, half, a_scale):
    """v2 layouts: kd66 as in k66 but pads reuse a real k-tile (scores stay
    in the normal range); vn transposed to [128, steps*64] fp16 with zero
    pads (zero V rows nullify pad steps, so no score poison is needed)."""
    groups = HALF_GROUPS[half]
    Qb = Q[b]  # [L, D]
    qd = np.empty((64, NQ_CORE), np.float32)
    kd = np.empty((64, TOTAL_STEPS * QT), np.float32)
    vn = np.zeros((TOTAL_STEPS, QT, D), np.float32)
    blk0 = Qb[0:QT, :]  # pad k-tile: any real tile keeps scores bounded
    kdb0 = blk0.T.copy()
    kdb0[0, :] = -kdb0[0, :]
    step_base = 0
    for s, g in enumerate(groups):
        ext = SLOT_EXTENTS[s]
        qd[:, s * SLOT_Q:(s + 1) * SLOT_Q] = (
            Qb[g * SLOT_Q:(g + 1) * SLOT_Q, :].T * a_scale
        )
        n_real = 4 * g + 4
        pads = ext - n_real
        # pads FIRST (zero V nullifies them), then k-tiles ascending so the
        # 4 diagonal tiles land at static steps ext-4..ext-1 (mask steps).
        for i in range(ext):
            st = step_base + i
            c0 = st * QT
            if i >= pads:
                j = i - pads  # ascending 0..n_real-1
                blk = Qb[j * QT:(j + 1) * QT, :]
                kdb = blk.T.copy()
                kdb[0, :] = -kdb[0, :]  # Lorentz signature on time component
                kd[:, c0:c0 + QT] = kdb
                vn[st] = blk
            else:
                kd[:, c0:c0 + QT] = kdb0
                # vn stays zero
        step_base += ext
    k0h, k0l = _hilo(kd[0:1], _BF16_NP)
    q0h, q0l = _hilo(qd[0:1], _BF16_NP)
    kd66 = np.zeros((_KPAD, TOTAL_STEPS * QT), _BF16_NP)
    kd66[0] = k0h
    kd66[1] = k0l
    kd66[2] = k0h
    kd66[3:_KSTACK] = kd[1:].astype(_BF16_NP)
    qd66 = np.zeros((_KPAD, NQ_CORE), _BF16_NP)
    qd66[0] = q0h
    qd66[1] = q0h
    qd66[2] = q0l
    qd66[3:_KSTACK] = qd[1:].astype(_BF16_NP)
    # [steps, 128k, 64d] -> [128k, steps*64]
    vn_pm = np.ascontiguousarray(
        vn.transpose(1, 0, 2).reshape(QT, TOTAL_STEPS * D)
    ).astype(np.float16)
    return {"qd66": qd66, "kd66": kd66, "vn": vn_pm}


def _prep_core_inputs(Q, b, half, a_scale, poison):
    """Build per-core input arrays. a_scale folded into q."""
    groups = HALF_GROUPS[half]
    Qb = Q[b]  # [L, D]
    qd = np.empty((64, NQ_CORE), np.float32)
    kd = np.empty((64, TOTAL_STEPS * QT), np.float32)
    vn = np.zeros((TOTAL_STEPS * QT, D), np.float32)
    step_base = 0
    for s, g in enumerate(groups):
        ext = SLOT_EXTENTS[s]
        qd[:, s * SLOT_Q:(s + 1) * SLOT_Q] = (
            Qb[g * SLOT_Q:(g + 1) * SLOT_Q, :].T * a_scale
        )
        n_real = 4 * g + 4  # causal extent of this group in k-tiles
        for i in range(ext):
            c0 = (step_base + i) * QT
            if i < n_real:
                j = 4 * g + 3 - i  # descending from the diagonal
                blk = Qb[j * QT:(j + 1) * QT, :]  # [128, 64]
                kdb = blk.T.copy()
                kdb[0, :] = -kdb[0, :]  # Lorentz signature on time component
                kd[:, c0:c0 + QT] = kdb
                vn[c0:c0 + QT, :] = blk
            else:
                kd[:, c0:c0 + QT] = 0.0
                kd[0, c0:c0 + QT] = poison
                # vn stays zero
        step_base += ext
    if _STRATEGY == "k66":
        # kd rows already carry the Lorentz sign on row 0 (time).
        k0h, k0l = _hilo(kd[0:1], _BF16_NP)      # signed time component
        q0h, q0l = _hilo(qd[0:1], _BF16_NP)
        kd66 = np.empty((_KSTACK, TOTAL_STEPS * QT), _BF16_NP)
        kd66[0] = k0h
        kd66[1] = k0l
        kd66[2] = k0h
        kd66[3:] = kd[1:].astype(_BF16_NP)
        qd66 = np.empty((_KSTACK, NQ_CORE), _BF16_NP)
        qd66[0] = q0h
        qd66[1] = q0h
        qd66[2] = q0l
        qd66[3:] = qd[1:].astype(_BF16_NP)
        vh, vl = _hilo(vn, np.float16)
        vns = np.concatenate([vh, vl], axis=1)   # [steps*128, 128]
        return {"qd66": qd66, "kd66": kd66, "vn": np.ascontiguousarray(vns)}
    if _STRATEGY not in ("split", "fp16", "mixed"):
        return {"qd": qd, "kd": kd, "vn": vn}
    np_dt = _BF16_NP if _STRATEGY in ("split", "mixed") else np.float16
    pv_np = _BF16_NP if _STRATEGY == "split" else np.float16
    qh, ql = _hilo(qd, np_dt)
    kh, kl = _hilo(kd, np_dt)
    vh, vl = _hilo(vn, pv_np)
    qdh = np.empty((128, NQ_CORE), np_dt)
    qdh[0:64] = qh
    qdh[64:128] = qh  # replicated: both halves of the K-stack see Q_hi
    kds = np.concatenate([kh, kl], axis=0)       # [128, steps*128]
    vns = np.concatenate([vh, vl], axis=1)       # [steps*128, 128]
    return {"qdh": qdh, "qdl": ql, "kd": np.ascontiguousarray(kds),
            "vn": np.ascontiguousarray(vns)}


def _mask_fixup(out, Q, mask, scale_v, bias_v):
    """Reference masks by QUERY row: a masked row becomes a uniform softmax
    over ALL L keys (causal entries equally -inf). Recompute those rows."""
    for b in range(B):
        rows = np.nonzero(mask[b])[0]
        if len(rows) == 0:
            continue
        mu = Q[b].mean(axis=0)  # uniform attention over all keys
        l_norm = -mu[0] ** 2 + np.sum(mu[1:] ** 2)
        denom = np.sqrt(max(abs(l_norm), EPS))
        out[b, rows, :] = (mu / denom)[None, :]
    return out


LAST_EXEC_NS = None
LAST_RESULTS = None


def kernel(Q, mask, scale, bias, _trace=False):
    global LAST_EXEC_NS, LAST_RESULTS
    Q = np.ascontiguousarray(np.asarray(Q, dtype=np.float32))
    mask_np = np.asarray(mask).astype(bool).reshape(B, L)
    scale_v = float(np.asarray(scale).reshape(-1)[0])
    bias_v = float(np.asarray(bias).reshape(-1)[0]) if np.asarray(bias).size else float(bias)

    a_scale = 2.0 / scale_v              # folded into q host-side
    b0 = 2.0 / scale_v + bias_v          # activation bias immediate
    poison = -(500.0 + abs(b0)) / a_scale

    if _trace:
        _ensure_ntff_hook()
    nc = _build_program(b0)

    in_maps = []
    for c in range(N_CORES):
        if _STRATEGY == "v2":
            in_maps.append(_prep_core_inputs_v2(Q, c // 2, c % 2, a_scale))
        else:
            in_maps.append(_prep_core_inputs(Q, c // 2, c % 2, a_scale, poison))

    res = bass_utils.run_bass_kernel_spmd(
        nc, in_maps, core_ids=list(range(N_CORES)), trace=_trace
    )
    LAST_EXEC_NS = res.exec_time_ns
    LAST_RESULTS = res

    out = np.empty((B, L, D), np.float32)
    for c in range(N_CORES):
        o = res.results[c]["out"]
        if _STRATEGY == "v2":
            # [128, 16*64] p-major -> [2048, 64]
            o = o.reshape(QT, NT_CORE, D).transpose(1, 0, 2).reshape(NQ_CORE, D)
        b, half = c // 2, c % 2
        for s, g in enumerate(HALF_GROUPS[half]):
            out[b, g * SLOT_Q:(g + 1) * SLOT_Q, :] = o[s * SLOT_Q:(s + 1) * SLOT_Q, :]

    if mask_np.any():
        out = _mask_fixup(out, Q, mask_np, scale_v, bias_v)
    return out



# revision 32
# speedup vs baseline: 1.4847x; 1.0034x over previous
"""Trainium2 Bass kernel for causal Lorentz self-attention.

Problem: B=4, L=4096, D=64 single-head self-attention where
  scores = (2 + 2*<q,k>_L) / scale + bias   (Lorentz inner product)
  causal mask (strict upper triangle) + per-query pad-mask
  attn = softmax(scores);  mu = attn @ v
  out = mu / sqrt(max(|<mu,mu>_L|, eps))

Key algebraic fact used: the softmax denominator cancels in the final
normalization (out = mu_raw / sqrt(|<mu_raw,mu_raw>_L|)), so no row-sum
is computed on device.

Sharding: 2 cores per batch. Each core runs an IDENTICAL static program of
4 "slots" (512 queries each) with static k-extents (8,16,24,32) steps of 128
keys. Which query tiles a slot owns, and where the causal boundary falls, is
encoded purely in host-prepared per-core input data:
  - k iterated DESCENDING from the diagonal, so the 4 boundary steps are
    always steps 0..3 of a slot (static affine_select masks),
  - slots whose causal extent is shorter than the static extent get
    "poison" K columns (huge negative score -> exp underflows to 0) and
    zero V rows.

Precision strategy "split" (default): all matmuls run in bf16 with hi/lo
decomposition (x = bf16(x) + bf16(x - bf16(x)), ~2^-17 operand precision):
  - scores: lhsT = [K_hi; K_lo] stacked on the contraction dim (K=128)
    against Q_hi replicated, plus a K=64 correction matmul K_hi x Q_lo.
  - attn@V: lhsT = [V_hi | V_lo] stacked on the output dim (M=128),
    two moving passes with P_hi and P_lo; the hi/lo output halves are
    summed once per slot (linearity lets them accumulate separately).
Strategy "f32" is the exact-fp32 fallback (4x slower matmuls).
"""

import os
import numpy as np
import ml_dtypes

import concourse.bass as bass
import concourse.bacc as bacc
import concourse.tile as tile
from concourse import mybir
from concourse import masks as cmasks
from concourse import bass_utils
from concourse._compat import with_exitstack
from contextlib import ExitStack

B, L, D = 4, 4096, 64
EPS = 1e-8
N_CORES = 8
QT = 128                       # queries per q-tile / keys per k-step
SLOT_Q = 512                   # queries per slot (4 q-tiles)
SLOTS = 4                      # slots per core
NQ_CORE = SLOTS * SLOT_Q       # 2048 queries per core
NT_CORE = NQ_CORE // QT        # 16 q-tiles per core
SLOT_EXTENTS = (8, 16, 24, 32)  # static k-steps per slot
TOTAL_STEPS = sum(SLOT_EXTENTS)  # 80
# groups of 4 consecutive q-tiles; group g covers q-tiles 4g..4g+3 and needs
# 4g+4 k-tiles. Half 0 gets groups (0,3,4,7) -> extents (4,16,20,32), half 1
# gets (1,2,5,6) -> (8,12,24,28); both fit elementwise under SLOT_EXTENTS.
HALF_GROUPS = ((0, 3, 4, 7), (1, 2, 5, 6))

_F32 = mybir.dt.float32
_BF16 = mybir.dt.bfloat16
_FP16 = mybir.dt.float16
_U16 = mybir.dt.uint16
_BF16_NP = ml_dtypes.bfloat16
_LOG2E = 1.4426950408889634
_SCHRAUDOLPH_A = 1024.0 * _LOG2E          # fp16-bits slope
_SCHRAUDOLPH_B0 = 1024.0 * 15.0 - 44.0    # fp16-bits intercept (C=44 minimax)
_ACT_COLS = 640                           # exp cols on ACT; rest on DVE
# strategy:
#   "k66"   - exploit Lorentz structure: time component (the only large
#             score term) as bf16 hi/lo cross-terms, spatial components as
#             single bf16 -> ONE K=66 score matmul. P fp16 from ACT, V fp16
#             hi/lo stack. 2 MMs/step total.
#   "mixed" - bf16 hi/lo pairs for K/Q, P fp16, V fp16 stack. 3 MMs/step.
#   "fp16"  - fp16 hi/lo pairs for K/Q, P fp16, V fp16 stack. 3 MMs/step,
#             but fp16 matmuls are half-rate on PE.
#   "split" - bf16 hi/lo everywhere incl. P (4 MMs/step + DVE splits)
#   "f32"   - exact fp32 fallback (4x slower matmuls)
_STRATEGY = os.environ.get("KERNEL_MM_DT", "v2")
_KSTACK = 66  # rows: [-k0h, -k0l, -k0h, k_space(63)] x [q0h, q0h, q0l, q_space]
_KPAD = 128   # contraction rows padded to full 128 partitions: sub-128
              # partition matmuls cap the PE clock at the mid p-state

_cache = {}


def _ensure_ntff_hook():
    """The agent image lacks ``antenv.axon_hooks``; synthesize it using the
    ctypes NTFF driver from trn_agent_boot so trace=True works."""
    import sys as _sys
    if "antenv.axon_hooks" in _sys.modules:
        return
    try:
        import types as _types
        import antenv  # noqa: F401
        from trn_agent_boot.trn_boot import _ntff_profile_via_ctypes
        hook = _ntff_profile_via_ctypes("/opt/axon/libaxon_pjrt.so")
        m = _types.ModuleType("antenv.axon_hooks")
        m.get_axon_ntff_profile_hook = lambda: hook
        m.set_axon_ntff_profile_hook = lambda h: None
        _sys.modules["antenv.axon_hooks"] = m
    except Exception:
        pass


@with_exitstack
def _body_v2(ctx: ExitStack, tc, aps, bias_val):
    """Fully SBUF-resident K/V/Q, K=66 Lorentz-structured score matmul,
    single-fp16 V. Exp of each 2-step round is split between ACT (true exp,
    cols 0:ACT_COLS) and DVE (Schraudolph fp16-bits exp, rest). Diagonal
    (masked) steps sit at slot END (pads first) so slot starts never stall;
    rounds stream across slot boundaries with a 1-round skew; epilogues are
    spread in small chunks between rounds. rsqrt via DVE bit trick + one
    Newton step keeps the ACT Exp table resident the whole kernel."""
    nc = tc.nc
    PSUM = bass.MemorySpace.PSUM

    const = ctx.enter_context(tc.tile_pool(name="const", bufs=1))
    datap = ctx.enter_context(tc.tile_pool(name="datap", bufs=1))
    expp = ctx.enter_context(tc.tile_pool(name="expp", bufs=4))
    stp = ctx.enter_context(tc.tile_pool(name="stp", bufs=3, space=PSUM))
    mup = ctx.enter_context(tc.tile_pool(name="mup", bufs=2, space=PSUM))
    sbp = ctx.enter_context(tc.tile_pool(name="sbp", bufs=1))
    smallp = ctx.enter_context(tc.tile_pool(name="smallp", bufs=4))
    outp = ctx.enter_context(tc.tile_pool(name="outp", bufs=1))

    ident = const.tile([128, 128], _F32)
    cmasks.make_identity(nc, ident[:])
    bias_t = const.tile([128, 1], _F32)
    nc.vector.memset(bias_t[:], float(bias_val))
    # DVE Schraudolph intercept: bits = A*(ps + b) + B0 = A*ps + (B0 + A*b)
    dve_b = _SCHRAUDOLPH_B0 + _SCHRAUDOLPH_A * float(bias_val)

    # ---- bulk preloads; first two triggers cover slot 0's working set ----
    slot_base = [sum(SLOT_EXTENTS[:i]) for i in range(SLOTS + 1)]
    kd_sb = datap.tile([_KPAD, TOTAL_STEPS * QT], _BF16)
    nc.sync.dma_start(kd_sb[:, 0:slot_base[1] * QT],
                      aps["kd66"][:, 0:slot_base[1] * QT])
    qd_sb = datap.tile([_KPAD, NQ_CORE], _BF16)
    nc.sync.dma_start(qd_sb[:, 0:SLOT_Q], aps["qd66"][:, 0:SLOT_Q])
    vn_sb = datap.tile([QT, TOTAL_STEPS * D], _FP16)
    nc.sync.dma_start(vn_sb[:, 0:slot_base[1] * D],
                      aps["vn"][:, 0:slot_base[1] * D])
    nc.sync.dma_start(qd_sb[:, SLOT_Q:], aps["qd66"][:, SLOT_Q:])
    for s0 in range(1, SLOTS):
        nc.sync.dma_start(
            kd_sb[:, slot_base[s0] * QT:slot_base[s0 + 1] * QT],
            aps["kd66"][:, slot_base[s0] * QT:slot_base[s0 + 1] * QT],
        )
        nc.sync.dma_start(
            vn_sb[:, slot_base[s0] * D:slot_base[s0 + 1] * D],
            aps["vn"][:, slot_base[s0] * D:slot_base[s0 + 1] * D],
        )

    # ---- PE warm-up: short matmuls ramp the clock during the DMA fill ----
    wsrc = const.tile([QT, SLOT_Q], _BF16)
    nc.vector.memset(wsrc[:], 0.0)
    n_warm = int(os.environ.get("KERNEL_N_WARM", "6"))
    for w in range(n_warm):
        wps = stp.tile([QT, 2 * SLOT_Q], _F32, tag="ps", name=f"warm{w}")
        nc.tensor.matmul(wps[:, 0:QT], lhsT=wsrc[:, 0:QT],
                         rhs=wsrc[:, 0:QT], start=True, stop=True)

    # mu_all padded to 128 partitions (rows 64:128 zeroed once) so the
    # epilogue transposes are full-partition ops (sub-128 caps the PE clock)
    mu_all = sbp.tile([128, NQ_CORE], _F32)
    nc.vector.memset(mu_all[64:128, :], 0.0)
    muq_all = sbp.tile([128, NT_CORE * D], _F32)
    ln_all = sbp.tile([128, NT_CORE], _F32)
    invs_all = sbp.tile([128, NT_CORE], _F32)
    out_sb = outp.tile([128, NT_CORE * D], _F32)

    def score_round(s, r):
        ps = stp.tile([QT, 2 * SLOT_Q], _F32)
        qblk = qd_sb[:, s * SLOT_Q:(s + 1) * SLOT_Q]
        for h in (0, 1):
            st = slot_base[s] + 2 * r + h
            nc.tensor.matmul(
                ps[:, h * SLOT_Q:(h + 1) * SLOT_Q],
                lhsT=kd_sb[:, st * QT:(st + 1) * QT],
                rhs=qblk,
                start=True, stop=True,
            )
        return ps

    def exp_round(s, r, ps):
        ext = SLOT_EXTENTS[s]
        et = expp.tile([QT, 2 * SLOT_Q], _FP16)
        # step h=0 -> ACT true exp, step h=1 -> DVE Schraudolph bits exp.
        # Diagonal (masked) steps: columns q' < 128*m are fully above the
        # diagonal — skip their exp; the affine_select zero-fills them.
        m0 = 2 * r - (ext - 4)
        if m0 < 0:
            # unmasked round: ACT takes step 0 plus 128 cols of step 1
            act_lo, act_hi = 0, _ACT_COLS
            dve_lo = _ACT_COLS
        else:
            act_lo, act_hi = QT * m0, SLOT_Q
            dve_lo = SLOT_Q + QT * (m0 + 1)
        nc.scalar.activation(
            et[:, act_lo:act_hi], ps[:, act_lo:act_hi],
            mybir.ActivationFunctionType.Exp,
            bias=bias_t[:], scale=1.0,
        )
        nc.vector.tensor_scalar(
            out=et[:, dve_lo:].bitcast(_U16),
            in0=ps[:, dve_lo:],
            scalar1=_SCHRAUDOLPH_A,
            scalar2=dve_b,
            op0=mybir.AluOpType.mult,
            op1=mybir.AluOpType.add,
        )
        for h in (0, 1):
            m = 2 * r + h - (ext - 4)
            if m >= 0:
                nc.gpsimd.affine_select(
                    out=et[:, h * SLOT_Q:(h + 1) * SLOT_Q],
                    in_=et[:, h * SLOT_Q:(h + 1) * SLOT_Q],
                    compare_op=mybir.AluOpType.is_ge,
                    fill=0.0,
                    base=-QT * m,
                    pattern=[[1, SLOT_Q]],
                    channel_multiplier=-1,
                )
        return et

    mu_tiles = {}
    epi_queue = []

    def av_round(s, r, et):
        ext = SLOT_EXTENTS[s]
        if r == 0:
            # [128, 512] tile: AV accumulates mu into rows 0:64; after the
            # drain, the same bank is reused for the epilogue transposes.
            mu_tiles[s] = mup.tile([QT, SLOT_Q], _F32, tag="mu", name=f"mu{s}")
        mu_ps = mu_tiles[s]
        for h in (0, 1):
            st = slot_base[s] + 2 * r + h
            nc.tensor.matmul(
                mu_ps[0:64, :],
                lhsT=vn_sb[:, st * D:(st + 1) * D],
                rhs=et[:, h * SLOT_Q:(h + 1) * SLOT_Q],
                start=(r == 0 and h == 0),
                stop=(2 * r + h == ext - 1),
            )
        if 2 * r + 1 == ext - 1:
            # drain mu to SBUF on ACT (it has the most slack)
            nc.scalar.copy(
                mu_all[0:64, s * SLOT_Q:(s + 1) * SLOT_Q], mu_ps[0:64, :]
            )
            for p in range(7):
                epi_queue.append((s, p))

    def epi_lnorm(sp, q):
        """|l| = 2*mu0^2 - sum(mu_d^2) for one q-tile (l is always < 0)."""
        qt_i = sp * 4 + q
        muq = muq_all[:, qt_i * D:(qt_i + 1) * D]
        sq = smallp.tile([QT, D], _F32)
        nc.vector.tensor_mul(sq[:], muq, muq)
        red = smallp.tile([QT, 1], _F32)
        nc.vector.reduce_sum(red[:], sq[:], axis=mybir.AxisListType.X)
        nc.vector.scalar_tensor_tensor(
            out=ln_all[:, qt_i:qt_i + 1],
            in0=sq[:, 0:1],
            scalar=2.0,
            in1=red[:],
            op0=mybir.AluOpType.mult,
            op1=mybir.AluOpType.subtract,
        )

    def epi_invs(sp):
        """invs = 1/sqrt(ln) on DVE: quake-style bit seed + 1 Newton step
        (avoids any ACT table switch away from Exp)."""
        x = ln_all[:, sp * 4:(sp + 1) * 4]
        xb = smallp.tile([128, 4], _F32, tag="xb")
        nc.vector.tensor_copy(xb[:], x.bitcast(mybir.dt.uint32))
        y0 = smallp.tile([128, 4], _F32, tag="y0")
        nc.vector.tensor_scalar(
            out=y0[:].bitcast(mybir.dt.int32), in0=xb[:],
            scalar1=-0.5, scalar2=float(0x5F3759DF),
            op0=mybir.AluOpType.mult, op1=mybir.AluOpType.add,
        )
        t = smallp.tile([128, 4], _F32, tag="t")
        nc.vector.tensor_mul(t[:], y0[:], y0[:])
        nc.vector.tensor_mul(t[:], t[:], x)
        nc.vector.tensor_scalar(
            out=t[:], in0=t[:], scalar1=-0.5, scalar2=1.5,
            op0=mybir.AluOpType.mult, op1=mybir.AluOpType.add,
        )
        nc.vector.tensor_mul(invs_all[:, sp * 4:(sp + 1) * 4], y0[:], t[:])

    def epi_scale(sp):
        for q in range(4):
            qt_i = sp * 4 + q
            nc.vector.tensor_scalar_mul(
                out_sb[:, qt_i * D:(qt_i + 1) * D],
                muq_all[:, qt_i * D:(qt_i + 1) * D],
                invs_all[:, qt_i:qt_i + 1],
            )
        nc.sync.dma_start(
            aps["out"][:, sp * 256:(sp + 1) * 256],
            out_sb[:, sp * 256:(sp + 1) * 256],
        )

    def epilogue_piece(sp, p):
        """One slot-tail piece; at most one is emitted per round."""
        if p <= 1:
            for q in (2 * p, 2 * p + 1):  # transposes into drained mu bank
                qt_i = sp * 4 + q
                nc.tensor.transpose(
                    mu_tiles[sp][:, q * QT:(q + 1) * QT],
                    mu_all[:, qt_i * QT:(qt_i + 1) * QT], ident[:],
                )
        elif p == 2:
            # all 4 tp's (real data in cols 0:64 of each 128-block) -> SBUF
            nc.vector.tensor_copy(
                muq_all[:, sp * 256:(sp + 1) * 256]
                .rearrange("p (q d) -> p q d", d=64),
                mu_tiles[sp][:].rearrange("p (q d) -> p q d", d=128)[:, :, 0:64],
            )
        elif p == 3:
            epi_lnorm(sp, 0)
            epi_lnorm(sp, 1)
        elif p == 4:
            epi_lnorm(sp, 2)
            epi_lnorm(sp, 3)
        elif p == 5:
            epi_invs(sp)
        elif p == 6:
            epi_scale(sp)

    # ---- flat pipelined stream over all rounds, 2-round AV skew;
    # epilogue pieces drip out one per round from a global queue ----
    rounds = [(s, r) for s in range(SLOTS) for r in range(SLOT_EXTENTS[s] // 2)]
    pend = []
    for s, r in rounds:
        ps = score_round(s, r)
        if len(pend) >= 2:
            av_round(*pend.pop(0))
        # exp BEFORE the epilogue piece: keeps exp at the head of the
        # in-order ACT/DVE queues so PE-dependent epilogue minis never
        # delay the next round's probabilities
        et = exp_round(s, r, ps)
        if epi_queue:
            epilogue_piece(*epi_queue.pop(0))
        pend.append((s, r, et))
    for p in pend:
        av_round(*p)
    while epi_queue:
        epilogue_piece(*epi_queue.pop(0))


@with_exitstack
def _body_k66(ctx: ExitStack, tc, aps, bias_val):
    """Single K=66 score matmul per step (Lorentz-structured hi/lo),
    fp16 P/V attention matmul. Skewed pipeline."""
    nc = tc.nc
    PSUM = bass.MemorySpace.PSUM

    const = ctx.enter_context(tc.tile_pool(name="const", bufs=1))
    qdp = ctx.enter_context(tc.tile_pool(name="qdp", bufs=1))
    kdp = ctx.enter_context(tc.tile_pool(name="kdp", bufs=4))
    vnp = ctx.enter_context(tc.tile_pool(name="vnp", bufs=4))
    expp = ctx.enter_context(tc.tile_pool(name="expp", bufs=4))
    stp = ctx.enter_context(tc.tile_pool(name="stp", bufs=3, space=PSUM))
    mup = ctx.enter_context(tc.tile_pool(name="mup", bufs=2, space=PSUM))
    tpp = ctx.enter_context(tc.tile_pool(name="tpp", bufs=2, space=PSUM))
    sbp = ctx.enter_context(tc.tile_pool(name="sbp", bufs=1))
    smallp = ctx.enter_context(tc.tile_pool(name="smallp", bufs=4))
    outp = ctx.enter_context(tc.tile_pool(name="outp", bufs=3))

    ident = const.tile([64, 64], _F32)
    cmasks.make_identity(nc, ident[:])
    bias_t = const.tile([128, 1], _F32)
    nc.vector.memset(bias_t[:], float(bias_val))

    qd_sb = qdp.tile([_KSTACK, NQ_CORE], _BF16)
    for s0 in range(SLOTS):
        c0, c1 = s0 * SLOT_Q, (s0 + 1) * SLOT_Q
        nc.sync.dma_start(qd_sb[:, c0:c1], aps["qd66"][:, c0:c1])

    mu_sb = sbp.tile([64, NQ_CORE], _F32)
    muq_all = sbp.tile([128, NT_CORE * D], _F32)
    ln_all = sbp.tile([128, NT_CORE], _F32)

    step_base = 0
    for s in range(SLOTS):
        ext = SLOT_EXTENTS[s]
        q_lo = s * SLOT_Q
        mu_ps = mup.tile([QT, SLOT_Q], _F32)  # rows 0-63 hi, 64-127 lo

        def st_step(i):
            st = step_base + i
            kt = kdp.tile([_KSTACK, QT], _BF16)
            nc.sync.dma_start(kt[:], aps["kd66"][:, st * QT:(st + 1) * QT])
            ps = stp.tile([QT, SLOT_Q], _F32)
            nc.tensor.matmul(
                ps[:], lhsT=kt[:], rhs=qd_sb[:, q_lo:q_lo + SLOT_Q],
                start=True, stop=True,
            )
            return ps

        def av_step(i, ps):
            st = step_base + i
            vt = vnp.tile([QT, QT], _FP16)
            nc.sync.dma_start(vt[:], aps["vn"][st * QT:(st + 1) * QT, :])
            et = expp.tile([QT, SLOT_Q], _FP16)
            nc.scalar.activation(
                et[:], ps[:], mybir.ActivationFunctionType.Exp,
                bias=bias_t[:], scale=1.0,
            )
            if i < 4:
                nc.gpsimd.affine_select(
                    out=et[:], in_=et[:],
                    compare_op=mybir.AluOpType.is_ge,
                    fill=0.0,
                    base=-QT * (3 - i),
                    pattern=[[1, SLOT_Q]],
                    channel_multiplier=-1,
                )
            nc.tensor.matmul(
                mu_ps[:], lhsT=vt[:], rhs=et[:],
                start=(i == 0), stop=(i == ext - 1),
            )

        ps_prev = st_step(0)
        for i in range(1, ext):
            ps_i = st_step(i)
            av_step(i - 1, ps_prev)
            ps_prev = ps_i
        av_step(ext - 1, ps_prev)
        step_base += ext

        lo_sb = smallp.tile([64, SLOT_Q], _F32, tag="losb")
        nc.scalar.copy(lo_sb[:], mu_ps[64:128, :])
        nc.vector.tensor_add(mu_sb[:, q_lo:q_lo + SLOT_Q], mu_ps[0:64, :], lo_sb[:])

        for q in range(SLOT_Q // QT):
            qt_i = s * (SLOT_Q // QT) + q
            tp = tpp.tile([QT, 64], _F32)
            nc.tensor.transpose(
                tp[:], mu_sb[:, qt_i * QT:(qt_i + 1) * QT], ident[:]
            )
            muq = muq_all[:, qt_i * D:(qt_i + 1) * D]
            nc.scalar.copy(muq, tp[:, :D])
            sq = smallp.tile([QT, D], _F32)
            nc.vector.tensor_mul(sq[:], muq, muq)
            red = smallp.tile([QT, 1], _F32)
            nc.vector.reduce_sum(red[:], sq[:], axis=mybir.AxisListType.X)
            nc.vector.scalar_tensor_tensor(
                out=ln_all[:, qt_i:qt_i + 1],
                in0=sq[:, 0:1],
                scalar=2.0,
                in1=red[:],
                op0=mybir.AluOpType.mult,
                op1=mybir.AluOpType.subtract,
            )

        # per-slot normalize: 1/sqrt(x) = exp(-0.5*ln(x)); Ln and Exp share
        # one ACT table set, so no table switch and no end-of-kernel phase.
        lns = ln_all[:, s * 4:(s + 1) * 4]
        lnt = smallp.tile([128, 4], _F32, tag="lnt")
        nc.scalar.activation(lnt[:], lns, mybir.ActivationFunctionType.Ln)
        invs = smallp.tile([128, 4], _F32, tag="invs")
        nc.scalar.activation(
            invs[:], lnt[:], mybir.ActivationFunctionType.Exp,
            bias=0.0, scale=-0.5,
        )
        for q in range(SLOT_Q // QT):
            qt_i = s * (SLOT_Q // QT) + q
            ot = outp.tile([QT, D], _F32)
            nc.vector.tensor_scalar_mul(
                ot[:], muq_all[:, qt_i * D:(qt_i + 1) * D], invs[:, q:q + 1]
            )
            nc.sync.dma_start(aps["out"][qt_i * QT:(qt_i + 1) * QT, :], ot[:])


@with_exitstack
def _body_fp16(ctx: ExitStack, tc, aps, bias_val, kq_dt=_FP16):
    """hi/lo-pair strategy with software-pipelined (skewed) step loop and
    per-slot preloaded K/V (per-step DMA triggers serialize on the sync
    sequencer at ~590ns each, so they must be batched).
    kq_dt: dtype of the K/Q score operands (bf16 = PE full rate)."""
    nc = tc.nc
    PSUM = bass.MemorySpace.PSUM

    const = ctx.enter_context(tc.tile_pool(name="const", bufs=1))
    qdp = ctx.enter_context(tc.tile_pool(name="qdp", bufs=1))
    kdp = ctx.enter_context(tc.tile_pool(name="kdp", bufs=1))
    vnp = ctx.enter_context(tc.tile_pool(name="vnp", bufs=1))
    expp = ctx.enter_context(tc.tile_pool(name="expp", bufs=4))
    stp = ctx.enter_context(tc.tile_pool(name="stp", bufs=3, space=PSUM))
    mup = ctx.enter_context(tc.tile_pool(name="mup", bufs=2, space=PSUM))
    tpp = ctx.enter_context(tc.tile_pool(name="tpp", bufs=2, space=PSUM))
    sbp = ctx.enter_context(tc.tile_pool(name="sbp", bufs=1))
    smallp = ctx.enter_context(tc.tile_pool(name="smallp", bufs=4))
    outp = ctx.enter_context(tc.tile_pool(name="outp", bufs=3))

    ident = const.tile([64, 64], _F32)
    cmasks.make_identity(nc, ident[:])
    bias_t = const.tile([128, 1], _F32)
    nc.vector.memset(bias_t[:], float(bias_val))

    # PE warm-up: ~16 dummy matmuls during the initial DMA window so the
    # HAM clock-gate reaches 2.4 GHz before the first real matmul.
    wsrc = const.tile([QT, SLOT_Q], kq_dt)
    nc.gpsimd.memset(wsrc[:], 0.0)
    for w in range(16):
        wps = tpp.tile([QT, SLOT_Q], _F32, tag="warm", bufs=1)
        nc.tensor.matmul(wps[:], lhsT=wsrc[:, 0:QT], rhs=wsrc[:],
                         start=True, stop=True)

    # causal boundary masks (fp16 ones/zeros incl. diagonal triangle),
    # applied with a DVE multiply instead of a gpsimd affine_select on the
    # exp->AV critical path.
    bmask = const.tile([QT, 4, SLOT_Q], _FP16)
    nc.vector.memset(bmask[:], 1.0)
    for i in range(4):
        nc.gpsimd.affine_select(
            out=bmask[:, i, :], in_=bmask[:, i, :],
            compare_op=mybir.AluOpType.is_ge,
            fill=0.0,
            base=-QT * (3 - i),
            pattern=[[1, SLOT_Q]],
            channel_multiplier=-1,
        )

    qdh_sb = qdp.tile([128, NQ_CORE], kq_dt)
    qdl_sb = qdp.tile([64, NQ_CORE], kq_dt)
    kd_sb = {}
    vn_sb = {}
    base = 0
    for s0 in range(SLOTS):
        ext = SLOT_EXTENTS[s0]
        c0, c1 = s0 * SLOT_Q, (s0 + 1) * SLOT_Q
        nc.sync.dma_start(qdh_sb[:, c0:c1], aps["qdh"][:, c0:c1])
        nc.sync.dma_start(qdl_sb[:, c0:c1], aps["qdl"][:, c0:c1])
        kd_sb[s0] = kdp.tile([QT, ext * QT], kq_dt, tag=f"kd{s0}", name=f"kd_sb{s0}")
        nc.sync.dma_start(kd_sb[s0][:], aps["kd"][:, base * QT:(base + ext) * QT])
        vn_sb[s0] = vnp.tile([QT, ext, QT], _FP16, tag=f"vn{s0}", name=f"vn_sb{s0}")
        vsrc = aps["vn"][base * QT:(base + ext) * QT, :].rearrange(
            "(t p) c -> p t c", p=QT)
        nc.sync.dma_start(vn_sb[s0][:], vsrc)
        base += ext

    mu_sb = sbp.tile([64, NQ_CORE], _F32)
    muq_all = sbp.tile([128, NT_CORE * D], _F32)
    ln_all = sbp.tile([128, NT_CORE], _F32)

    step_base = 0
    for s in range(SLOTS):
        ext = SLOT_EXTENTS[s]
        q_lo = s * SLOT_Q
        mu_ps = mup.tile([QT, SLOT_Q], _F32)  # rows 0-63 hi, 64-127 lo

        def st_step(i):
            kt = kd_sb[s][:, i * QT:(i + 1) * QT]
            ps = stp.tile([QT, SLOT_Q], _F32)
            nc.tensor.matmul(
                ps[:], lhsT=kt, rhs=qdh_sb[:, q_lo:q_lo + SLOT_Q],
                start=True, stop=False,
            )
            nc.tensor.matmul(
                ps[:], lhsT=kt[0:64, :], rhs=qdl_sb[:, q_lo:q_lo + SLOT_Q],
                start=False, stop=True,
            )
            return ps

        def av_step(i, ps):
            vt = vn_sb[s][:, i, :]
            et = expp.tile([QT, SLOT_Q], _FP16)
            nc.scalar.activation(
                et[:], ps[:], mybir.ActivationFunctionType.Exp,
                bias=bias_t[:], scale=1.0,
            )
            if i < 4:
                # step i's k-tile is the (3-i)'th q-tile block's diagonal
                nc.vector.tensor_mul(et[:], et[:], bmask[:, i, :])
            nc.tensor.matmul(
                mu_ps[:], lhsT=vt, rhs=et[:],
                start=(i == 0), stop=(i == ext - 1),
            )

        # 2-deep skewed pipeline: S_T(i+2) runs on PE before AV(i), covering
        # the exp latency (and the boundary-mask multiply) on ACT/DVE.
        pending = [st_step(0), st_step(1)]
        for i in range(2, ext):
            pending.append(st_step(i))
            av_step(i - 2, pending.pop(0))
        av_step(ext - 2, pending.pop(0))
        av_step(ext - 1, pending.pop(0))
        step_base += ext

        # mu = hi half + lo half (one PSUM operand max per DVE op)
        lo_sb = smallp.tile([64, SLOT_Q], _F32, tag="losb")
        nc.scalar.copy(lo_sb[:], mu_ps[64:128, :])
        nc.vector.tensor_add(mu_sb[:, q_lo:q_lo + SLOT_Q], mu_ps[0:64, :], lo_sb[:])

        for q in range(SLOT_Q // QT):
            qt_i = s * (SLOT_Q // QT) + q
            tp = tpp.tile([QT, 64], _F32)
            nc.tensor.transpose(
                tp[:], mu_sb[:, qt_i * QT:(qt_i + 1) * QT], ident[:]
            )
            muq = muq_all[:, qt_i * D:(qt_i + 1) * D]
            nc.scalar.copy(muq, tp[:, :D])
            sq = smallp.tile([QT, D], _F32)
            nc.vector.tensor_mul(sq[:], muq, muq)
            red = smallp.tile([QT, 1], _F32)
            nc.vector.reduce_sum(red[:], sq[:], axis=mybir.AxisListType.X)
            # |l| = -l = 2*mu0^2 - sum(mu_d^2)  (l is always < 0 here)
            nc.vector.scalar_tensor_tensor(
                out=ln_all[:, qt_i:qt_i + 1],
                in0=sq[:, 0:1],
                scalar=2.0,
                in1=red[:],
                op0=mybir.AluOpType.mult,
                op1=mybir.AluOpType.subtract,
            )

        # per-slot normalize: 1/sqrt(x) = exp(-0.5*ln(x)); Ln and Exp share
        # one ACT table set, so no table switch and no end-of-kernel phase.
        lns = ln_all[:, s * 4:(s + 1) * 4]
        lnt = smallp.tile([128, 4], _F32, tag="lnt")
        nc.scalar.activation(lnt[:], lns, mybir.ActivationFunctionType.Ln)
        invs = smallp.tile([128, 4], _F32, tag="invs")
        nc.scalar.activation(
            invs[:], lnt[:], mybir.ActivationFunctionType.Exp,
            bias=0.0, scale=-0.5,
        )
        for q in range(SLOT_Q // QT):
            qt_i = s * (SLOT_Q // QT) + q
            ot = outp.tile([QT, D], _F32)
            nc.vector.tensor_scalar_mul(
                ot[:], muq_all[:, qt_i * D:(qt_i + 1) * D], invs[:, q:q + 1]
            )
            nc.sync.dma_start(aps["out"][qt_i * QT:(qt_i + 1) * QT, :], ot[:])


@with_exitstack
def _body_split(ctx: ExitStack, tc, aps, bias_val):
    """bf16 hi/lo strategy. aps: dict of DRAM APs."""
    nc = tc.nc
    PSUM = bass.MemorySpace.PSUM

    const = ctx.enter_context(tc.tile_pool(name="const", bufs=1))
    qdp = ctx.enter_context(tc.tile_pool(name="qdp", bufs=1))
    kdp = ctx.enter_context(tc.tile_pool(name="kdp", bufs=4))
    vnp = ctx.enter_context(tc.tile_pool(name="vnp", bufs=4))
    expp = ctx.enter_context(tc.tile_pool(name="expp", bufs=4))
    ehp = ctx.enter_context(tc.tile_pool(name="ehp", bufs=3))
    elp = ctx.enter_context(tc.tile_pool(name="elp", bufs=3))
    stp = ctx.enter_context(tc.tile_pool(name="stp", bufs=2, space=PSUM))
    mup = ctx.enter_context(tc.tile_pool(name="mup", bufs=2, space=PSUM))
    tpp = ctx.enter_context(tc.tile_pool(name="tpp", bufs=2, space=PSUM))
    sbp = ctx.enter_context(tc.tile_pool(name="sbp", bufs=1))
    smallp = ctx.enter_context(tc.tile_pool(name="smallp", bufs=4))
    outp = ctx.enter_context(tc.tile_pool(name="outp", bufs=3))

    ident = const.tile([64, 64], _F32)
    cmasks.make_identity(nc, ident[:])
    bias_t = const.tile([128, 1], _F32)
    nc.vector.memset(bias_t[:], float(bias_val))

    qdh_sb = qdp.tile([128, NQ_CORE], _BF16)
    nc.sync.dma_start(qdh_sb[:], aps["qdh"][:])
    qdl_sb = qdp.tile([64, NQ_CORE], _BF16)
    nc.sync.dma_start(qdl_sb[:], aps["qdl"][:])

    mu_sb = sbp.tile([64, NQ_CORE], _F32)
    muq_all = sbp.tile([128, NT_CORE * D], _F32)
    ln_all = sbp.tile([128, NT_CORE], _F32)

    step_base = 0
    for s in range(SLOTS):
        ext = SLOT_EXTENTS[s]
        q_lo = s * SLOT_Q
        mu_ps = mup.tile([QT, SLOT_Q], _F32)  # rows 0-63 hi, 64-127 lo
        for i in range(ext):
            st = step_base + i
            kt = kdp.tile([QT, QT], _BF16)
            nc.sync.dma_start(kt[:], aps["kd"][:, st * QT:(st + 1) * QT])
            vt = vnp.tile([QT, QT], _BF16)
            nc.sync.dma_start(vt[:], aps["vn"][st * QT:(st + 1) * QT, :])

            ps = stp.tile([QT, SLOT_Q], _F32)
            nc.tensor.matmul(
                ps[:], lhsT=kt[:], rhs=qdh_sb[:, q_lo:q_lo + SLOT_Q],
                start=True, stop=False,
            )
            nc.tensor.matmul(
                ps[:], lhsT=kt[0:64, :], rhs=qdl_sb[:, q_lo:q_lo + SLOT_Q],
                start=False, stop=True,
            )
            et = expp.tile([QT, SLOT_Q], _F32)
            nc.scalar.activation(
                et[:], ps[:], mybir.ActivationFunctionType.Exp,
                bias=bias_t[:], scale=1.0,
            )
            if i < 4:
                # step i's k-tile is the (3-i)'th q-tile block's diagonal:
                # keep element (k, q) iff q - k - 128*(3-i) >= 0
                nc.gpsimd.affine_select(
                    out=et[:], in_=et[:],
                    compare_op=mybir.AluOpType.is_ge,
                    fill=0.0,
                    base=-QT * (3 - i),
                    pattern=[[1, SLOT_Q]],
                    channel_multiplier=-1,
                )
            eth = ehp.tile([QT, SLOT_Q], _BF16)
            nc.vector.tensor_copy(eth[:], et[:])
            etl = elp.tile([QT, SLOT_Q], _BF16)
            nc.vector.tensor_sub(etl[:], et[:], eth[:])
            nc.tensor.matmul(
                mu_ps[:], lhsT=vt[:], rhs=eth[:],
                start=(i == 0), stop=False,
            )
            nc.tensor.matmul(
                mu_ps[:], lhsT=vt[:], rhs=etl[:],
                start=False, stop=(i == ext - 1),
            )
        step_base += ext

        # mu = hi half + lo half (one PSUM operand max per DVE op)
        lo_sb = smallp.tile([64, SLOT_Q], _F32, tag="losb")
        nc.scalar.copy(lo_sb[:], mu_ps[64:128, :])
        nc.vector.tensor_add(mu_sb[:, q_lo:q_lo + SLOT_Q], mu_ps[0:64, :], lo_sb[:])

        for q in range(SLOT_Q // QT):
            qt_i = s * (SLOT_Q // QT) + q
            tp = tpp.tile([QT, 64], _F32)
            nc.tensor.transpose(
                tp[:], mu_sb[:, qt_i * QT:(qt_i + 1) * QT], ident[:]
            )
            muq = muq_all[:, qt_i * D:(qt_i + 1) * D]
            nc.scalar.copy(muq, tp[:, :D])
            sq = smallp.tile([QT, D], _F32)
            nc.vector.tensor_mul(sq[:], muq, muq)
            red = smallp.tile([QT, 1], _F32)
            nc.vector.reduce_sum(red[:], sq[:], axis=mybir.AxisListType.X)
            # |l| = -l = 2*mu0^2 - sum(mu_d^2)  (l is always < 0 here)
            nc.vector.scalar_tensor_tensor(
                out=ln_all[:, qt_i:qt_i + 1],
                in0=sq[:, 0:1],
                scalar=2.0,
                in1=red[:],
                op0=mybir.AluOpType.mult,
                op1=mybir.AluOpType.subtract,
            )

    # grouped sqrt (single ACT table switch) + reciprocal + final scale
    sqv = sbp.tile([128, NT_CORE], _F32)
    nc.scalar.activation(
        sqv[:], ln_all[:], mybir.ActivationFunctionType.Sqrt,
        bias=0.0, scale=1.0,
    )
    inv = sbp.tile([128, NT_CORE], _F32)
    nc.vector.reciprocal(inv[:], sqv[:])
    for qt_i in range(NT_CORE):
        ot = outp.tile([QT, D], _F32)
        nc.vector.tensor_scalar_mul(
            ot[:], muq_all[:, qt_i * D:(qt_i + 1) * D], inv[:, qt_i:qt_i + 1]
        )
        nc.sync.dma_start(aps["out"][qt_i * QT:(qt_i + 1) * QT, :], ot[:])


@with_exitstack
def _body_f32(ctx: ExitStack, tc, aps, bias_val):
    """Exact-fp32 fallback strategy."""
    nc = tc.nc
    PSUM = bass.MemorySpace.PSUM

    const = ctx.enter_context(tc.tile_pool(name="const", bufs=1))
    qdp = ctx.enter_context(tc.tile_pool(name="qdp", bufs=1))
    kdp = ctx.enter_context(tc.tile_pool(name="kdp", bufs=4))
    vnp = ctx.enter_context(tc.tile_pool(name="vnp", bufs=4))
    expp = ctx.enter_context(tc.tile_pool(name="expp", bufs=4))
    stp = ctx.enter_context(tc.tile_pool(name="stp", bufs=2, space=PSUM))
    mup = ctx.enter_context(tc.tile_pool(name="mup", bufs=2, space=PSUM))
    tpp = ctx.enter_context(tc.tile_pool(name="tpp", bufs=2, space=PSUM))
    sbp = ctx.enter_context(tc.tile_pool(name="sbp", bufs=1))
    smallp = ctx.enter_context(tc.tile_pool(name="smallp", bufs=4))
    outp = ctx.enter_context(tc.tile_pool(name="outp", bufs=3))

    ident = const.tile([64, 64], _F32)
    cmasks.make_identity(nc, ident[:])
    bias_t = const.tile([128, 1], _F32)
    nc.vector.memset(bias_t[:], float(bias_val))

    qd_sb = qdp.tile([64, NQ_CORE], _F32)
    nc.sync.dma_start(qd_sb[:], aps["qd"][:])

    mu_sb = sbp.tile([64, NQ_CORE], _F32)
    muq_all = sbp.tile([128, NT_CORE * D], _F32)
    ln_all = sbp.tile([128, NT_CORE], _F32)

    step_base = 0
    for s in range(SLOTS):
        ext = SLOT_EXTENTS[s]
        q_lo = s * SLOT_Q
        mu_ps = mup.tile([64, SLOT_Q], _F32)
        for i in range(ext):
            st = step_base + i
            kt = kdp.tile([64, QT], _F32)
            nc.sync.dma_start(kt[:], aps["kd"][:, st * QT:(st + 1) * QT])
            vt = vnp.tile([QT, D], _F32)
            nc.sync.dma_start(vt[:], aps["vn"][st * QT:(st + 1) * QT, :])

            ps = stp.tile([QT, SLOT_Q], _F32)
            nc.tensor.matmul(
                ps[:], lhsT=kt[:], rhs=qd_sb[:, q_lo:q_lo + SLOT_Q],
                start=True, stop=True,
            )
            et = expp.tile([QT, SLOT_Q], _F32)
            nc.scalar.activation(
                et[:], ps[:], mybir.ActivationFunctionType.Exp,
                bias=bias_t[:], scale=1.0,
            )
            if i < 4:
                nc.gpsimd.affine_select(
                    out=et[:], in_=et[:],
                    compare_op=mybir.AluOpType.is_ge,
                    fill=0.0,
                    base=-QT * (3 - i),
                    pattern=[[1, SLOT_Q]],
                    channel_multiplier=-1,
                )
            nc.tensor.matmul(
                mu_ps[:], lhsT=vt[:], rhs=et[:],
                start=(i == 0), stop=(i == ext - 1),
            )
        step_base += ext

        nc.vector.tensor_copy(mu_sb[:, q_lo:q_lo + SLOT_Q], mu_ps[:])
        for q in range(SLOT_Q // QT):
            qt_i = s * (SLOT_Q // QT) + q
            tp = tpp.tile([QT, 64], _F32)
            nc.tensor.transpose(
                tp[:], mu_sb[:, qt_i * QT:(qt_i + 1) * QT], ident[:]
            )
            muq = muq_all[:, qt_i * D:(qt_i + 1) * D]
            nc.scalar.copy(muq, tp[:, :D])
            sq = smallp.tile([QT, D], _F32)
            nc.vector.tensor_mul(sq[:], muq, muq)
            red = smallp.tile([QT, 1], _F32)
            nc.vector.reduce_sum(red[:], sq[:], axis=mybir.AxisListType.X)
            nc.vector.scalar_tensor_tensor(
                out=ln_all[:, qt_i:qt_i + 1],
                in0=sq[:, 0:1],
                scalar=2.0,
                in1=red[:],
                op0=mybir.AluOpType.mult,
                op1=mybir.AluOpType.subtract,
            )

    sqv = sbp.tile([128, NT_CORE], _F32)
    nc.scalar.activation(
        sqv[:], ln_all[:], mybir.ActivationFunctionType.Sqrt,
        bias=0.0, scale=1.0,
    )
    inv = sbp.tile([128, NT_CORE], _F32)
    nc.vector.reciprocal(inv[:], sqv[:])
    for qt_i in range(NT_CORE):
        ot = outp.tile([QT, D], _F32)
        nc.vector.tensor_scalar_mul(
            ot[:], muq_all[:, qt_i * D:(qt_i + 1) * D], inv[:, qt_i:qt_i + 1]
        )
        nc.sync.dma_start(aps["out"][qt_i * QT:(qt_i + 1) * QT, :], ot[:])


def _build_program(bias_val):
    key = (round(float(bias_val), 12), _STRATEGY)
    if key in _cache:
        return _cache[key]
    nc = bacc.Bacc(
        "TRN2",
        target_bir_lowering=False,
        debug=False,
        enable_asserts=False,
    )
    aps = {}
    if _STRATEGY == "v2":
        aps["qd66"] = nc.dram_tensor("qd66", [_KPAD, NQ_CORE], _BF16, kind="ExternalInput").ap()
        aps["kd66"] = nc.dram_tensor("kd66", [_KPAD, TOTAL_STEPS * QT], _BF16, kind="ExternalInput").ap()
        aps["vn"] = nc.dram_tensor("vn", [QT, TOTAL_STEPS * D], _FP16, kind="ExternalInput").ap()
        aps["out"] = nc.dram_tensor("out", [128, NT_CORE * D], _F32, kind="ExternalOutput").ap()
        with tile.TileContext(nc) as tc:
            _body_v2(tc, aps, bias_val)
        nc.compile()
        _cache[key] = nc
        return nc
    if _STRATEGY == "k66":
        aps["qd66"] = nc.dram_tensor("qd66", [_KSTACK, NQ_CORE], _BF16, kind="ExternalInput").ap()
        aps["kd66"] = nc.dram_tensor("kd66", [_KSTACK, TOTAL_STEPS * QT], _BF16, kind="ExternalInput").ap()
        aps["vn"] = nc.dram_tensor("vn", [TOTAL_STEPS * QT, 128], _FP16, kind="ExternalInput").ap()
    elif _STRATEGY in ("split", "fp16", "mixed"):
        kq_dt = _BF16 if _STRATEGY in ("split", "mixed") else _FP16
        pv_dt = _BF16 if _STRATEGY == "split" else _FP16
        aps["qdh"] = nc.dram_tensor("qdh", [128, NQ_CORE], kq_dt, kind="ExternalInput").ap()
        aps["qdl"] = nc.dram_tensor("qdl", [64, NQ_CORE], kq_dt, kind="ExternalInput").ap()
        aps["kd"] = nc.dram_tensor("kd", [128, TOTAL_STEPS * QT], kq_dt, kind="ExternalInput").ap()
        aps["vn"] = nc.dram_tensor("vn", [TOTAL_STEPS * QT, 128], pv_dt, kind="ExternalInput").ap()
    else:
        aps["qd"] = nc.dram_tensor("qd", [64, NQ_CORE], _F32, kind="ExternalInput").ap()
        aps["kd"] = nc.dram_tensor("kd", [64, TOTAL_STEPS * QT], _F32, kind="ExternalInput").ap()
        aps["vn"] = nc.dram_tensor("vn", [TOTAL_STEPS * QT, D], _F32, kind="ExternalInput").ap()
    aps["out"] = nc.dram_tensor("out", [NQ_CORE, D], _F32, kind="ExternalOutput").ap()
    with tile.TileContext(nc) as tc:
        if _STRATEGY == "k66":
            _body_k66(tc, aps, bias_val)
        elif _STRATEGY == "mixed":
            _body_fp16(tc, aps, bias_val, kq_dt=_BF16)
        elif _STRATEGY == "fp16":
            _body_fp16(tc, aps, bias_val, kq_dt=_FP16)
        elif _STRATEGY == "split":
            _body_split(tc, aps, bias_val)
        else:
            _body_f32(tc, aps, bias_val)
    nc.compile()
    _cache[key] = nc
    return nc


def _hilo(x, np_dt):
    hi = x.astype(np_dt)
    lo = (x - hi.astype(np.float32)).astype(np_dt)
    return hi, lo


def _prep_core_inputs_v2(Q, b, half, a_scale):
    """v2 layouts: kd66 as in k66 but pads reuse a real k-tile (scores stay
    in the normal range); vn transposed to [128, steps*64] fp16 with zero
    pads (zero V rows nullify pad steps, so no score poison is needed)."""
    groups = HALF_GROUPS[half]
    Qb = Q[b]  # [L, D]
    qd = np.empty((64, NQ_CORE), np.float32)
    kd = np.empty((64, TOTAL_STEPS * QT), np.float32)
    vn = np.zeros((TOTAL_STEPS, QT, D), np.float32)
    blk0 = Qb[0:QT, :]  # pad k-tile: any real tile keeps scores bounded
    kdb0 = blk0.T.copy()
    kdb0[0, :] = -kdb0[0, :]
    step_base = 0
    for s, g in enumerate(groups):
        ext = SLOT_EXTENTS[s]
        qd[:, s * SLOT_Q:(s + 1) * SLOT_Q] = (
            Qb[g * SLOT_Q:(g + 1) * SLOT_Q, :].T * a_scale
        )
        n_real = 4 * g + 4
        pads = ext - n_real
        # pads FIRST (zero V nullifies them), then k-tiles ascending so the
        # 4 diagonal tiles land at static steps ext-4..ext-1 (mask steps).
        for i in range(ext):
            st = step_base + i
            c0 = st * QT
            if i >= pads:
                j = i - pads  # ascending 0..n_real-1
                blk = Qb[j * QT:(j + 1) * QT, :]
                kdb = blk.T.copy()
                kdb[0, :] = -kdb[0, :]  # Lorentz signature on time component
                kd[:, c0:c0 + QT] = kdb
                vn[st] = blk
            else:
                kd[:, c0:c0 + QT] = kdb0
                # vn stays zero
        step_base += ext
    k0h, k0l = _hilo(kd[0:1], _BF16_NP)
    q0h, q0l = _hilo(qd[0:1], _BF16_NP)
    kd66 = np.zeros((_KPAD, TOTAL_STEPS * QT), _BF16_NP)
    kd66[0] = k0h
    kd66[1] = k0l
    kd66[2] = k0h
    kd66[3:_KSTACK] = kd[1:].astype(_BF16_NP)
    qd66 = np.zeros((_KPAD, NQ_CORE), _BF16_NP)
    qd66[0] = q0h
    qd66[1] = q0h
    qd66[2] = q0l
    qd66[3:_KSTACK] = qd[1:].astype(_BF16_NP)
    # [steps, 128k, 64d] -> [128k, steps*64]
    vn_pm = np.ascontiguousarray(
        vn.transpose(1, 0, 2).reshape(QT, TOTAL_STEPS * D)
    ).astype(np.float16)
    return {"qd66": qd66, "kd66": kd66, "vn": vn_pm}


def _prep_core_inputs(Q, b, half, a_scale, poison):
    """Build per-core input arrays. a_scale folded into q."""
    groups = HALF_GROUPS[half]
    Qb = Q[b]  # [L, D]
    qd = np.empty((64, NQ_CORE), np.float32)
    kd = np.empty((64, TOTAL_STEPS * QT), np.float32)
    vn = np.zeros((TOTAL_STEPS * QT, D), np.float32)
    step_base = 0
    for s, g in enumerate(groups):
        ext = SLOT_EXTENTS[s]
        qd[:, s * SLOT_Q:(s + 1) * SLOT_Q] = (
            Qb[g * SLOT_Q:(g + 1) * SLOT_Q, :].T * a_scale
        )
        n_real = 4 * g + 4  # causal extent of this group in k-tiles
        for i in range(ext):
            c0 = (step_base + i) * QT
            if i < n_real:
                j = 4 * g + 3 - i  # descending from the diagonal
                blk = Qb[j * QT:(j + 1) * QT, :]  # [128, 64]
                kdb = blk.T.copy()
                kdb[0, :] = -kdb[0, :]  # Lorentz signature on time component
                kd[:, c0:c0 + QT] = kdb
                vn[c0:c0 + QT, :] = blk
            else:
                kd[:, c0:c0 + QT] = 0.0
                kd[0, c0:c0 + QT] = poison
                # vn stays zero
        step_base += ext
    if _STRATEGY == "k66":
        # kd rows already carry the Lorentz sign on row 0 (time).
        k0h, k0l = _hilo(kd[0:1], _BF16_NP)      # signed time component
        q0h, q0l = _hilo(qd[0:1], _BF16_NP)
        kd66 = np.empty((_KSTACK, TOTAL_STEPS * QT), _BF16_NP)
        kd66[0] = k0h
        kd66[1] = k0l
        kd66[2] = k0h
        kd66[3:] = kd[1:].astype(_BF16_NP)
        qd66 = np.empty((_KSTACK, NQ_CORE), _BF16_NP)
        qd66[0] = q0h
        qd66[1] = q0h
        qd66[2] = q0l
        qd66[3:] = qd[1:].astype(_BF16_NP)
        vh, vl = _hilo(vn, np.float16)
        vns = np.concatenate([vh, vl], axis=1)   # [steps*128, 128]
        return {"qd66": qd66, "kd66": kd66, "vn": np.ascontiguousarray(vns)}
    if _STRATEGY not in ("split", "fp16", "mixed"):
        return {"qd": qd, "kd": kd, "vn": vn}
    np_dt = _BF16_NP if _STRATEGY in ("split", "mixed") else np.float16
    pv_np = _BF16_NP if _STRATEGY == "split" else np.float16
    qh, ql = _hilo(qd, np_dt)
    kh, kl = _hilo(kd, np_dt)
    vh, vl = _hilo(vn, pv_np)
    qdh = np.empty((128, NQ_CORE), np_dt)
    qdh[0:64] = qh
    qdh[64:128] = qh  # replicated: both halves of the K-stack see Q_hi
    kds = np.concatenate([kh, kl], axis=0)       # [128, steps*128]
    vns = np.concatenate([vh, vl], axis=1)       # [steps*128, 128]
    return {"qdh": qdh, "qdl": ql, "kd": np.ascontiguousarray(kds),
            "vn": np.ascontiguousarray(vns)}


def _mask_fixup(out, Q, mask, scale_v, bias_v):
    """Reference masks by QUERY row: a masked row becomes a uniform softmax
    over ALL L keys (causal entries equally -inf). Recompute those rows."""
    for b in range(B):
        rows = np.nonzero(mask[b])[0]
        if len(rows) == 0:
            continue
        mu = Q[b].mean(axis=0)  # uniform attention over all keys
        l_norm = -mu[0] ** 2 + np.sum(mu[1:] ** 2)
        denom = np.sqrt(max(abs(l_norm), EPS))
        out[b, rows, :] = (mu / denom)[None, :]
    return out


LAST_EXEC_NS = None
LAST_RESULTS = None


def kernel(Q, mask, scale, bias, _trace=False):
    global LAST_EXEC_NS, LAST_RESULTS
    Q = np.ascontiguousarray(np.asarray(Q, dtype=np.float32))
    mask_np = np.asarray(mask).astype(bool).reshape(B, L)
    scale_v = float(np.asarray(scale).reshape(-1)[0])
    bias_v = float(np.asarray(bias).reshape(-1)[0]) if np.asarray(bias).size else float(bias)

    a_scale = 2.0 / scale_v              # folded into q host-side
    b0 = 2.0 / scale_v + bias_v          # activation bias immediate
    poison = -(500.0 + abs(b0)) / a_scale

    if _trace:
        _ensure_ntff_hook()
    nc = _build_program(b0)

    in_maps = []
    for c in range(N_CORES):
        if _STRATEGY == "v2":
            in_maps.append(_prep_core_inputs_v2(Q, c // 2, c % 2, a_scale))
        else:
            in_maps.append(_prep_core_inputs(Q, c // 2, c % 2, a_scale, poison))

    res = bass_utils.run_bass_kernel_spmd(
        nc, in_maps, core_ids=list(range(N_CORES)), trace=_trace
    )
    LAST_EXEC_NS = res.exec_time_ns
    LAST_RESULTS = res

    out = np.empty((B, L, D), np.float32)
    for c in range(N_CORES):
        o = res.results[c]["out"]
        if _STRATEGY == "v2":
            # [128, 16*64] p-major -> [2048, 64]
            o = o.reshape(QT, NT_CORE, D).transpose(1, 0, 2).reshape(NQ_CORE, D)
        b, half = c // 2, c % 2
        for s, g in enumerate(HALF_GROUPS[half]):
            out[b, g * SLOT_Q:(g + 1) * SLOT_Q, :] = o[s * SLOT_Q:(s + 1) * SLOT_Q, :]

    if mask_np.any():
        out = _mask_fixup(out, Q, mask_np, scale_v, bias_v)
    return out

